# revision 9
# baseline (speedup 1.0000x reference)
"""AdaptiveSpanAttention TRN2 kernel: 8-way batch-parallel Bass/Tile kernel.

Structure exploited: fs = sigmoid(z)*decay_s can only exceed adaptive_threshold
where decay_s > threshold (since sigmoid < 1). For the reference scalars
(decay 0.95, thr 0.3) that is only the last 24 of 1024 positions, so the big
relevance MLP runs only on that row suffix. The threshold mask is evaluated on
the pre-sigmoid logit z against host-precomputed c_s = logit(thr/decay_s),
which is exactly monotone-equivalent and immune to LUT rounding at the
discontinuity. All matmuls run in true fp32 on the PE (4 cyc/row).

Sharding: batch dim (axis 1 of long_x/encoded_x) across the 8 NeuronCores,
weights replicated, outputs concatenated on host (pure data parallel).
"""
import sys

import numpy as np

sys.path.insert(0, "/opt/trn_rl_repo")

import concourse.tile as tile  # noqa: E402
from concourse import bacc, mybir  # noqa: E402
from concourse.bass_utils import run_bass_kernel_spmd  # noqa: E402

F32 = mybir.dt.float32
I32 = mybir.dt.int32
AF = mybir.ActivationFunctionType
ALU = mybir.AluOpType
AXL = mybir.AxisListType

S, B, E, H = 1024, 64, 1024, 16
NCORES = 8
BL = B // NCORES  # 8 batches per core
MAX_SPAN, MIN_SPAN = 1024, 8
LAST_RESULT = None


def _build(S_active, hs_mean, regp):
    """Per-core Bass graph. Row order r = b*S_active + s_local (b-major),
    R = BL*S_active rows, MLP processed in r-blocks of <=256."""
    R = BL * S_active
    NQ = R // 128
    RB = 256 if R >= 256 else R
    NRB = R // RB
    s_lo = S - S_active
    ZROWS = s_lo * BL

    nc = bacc.Bacc("TRN2", target_bir_lowering=False, debug=False,
                   num_devices=NCORES)

    def inp(name, shape, dt=F32):
        return nc.declare_dram_parameter(name, shape, dt, isOutput=False)

    lx = inp("lx", [BL, S_active, E])
    enc = inp("enc", [256, BL, E])
    rs_w1 = inp("rs_w1", [2 * E, E])
    rs_w2 = inp("rs_w2", [E, E // 2])
    rs_w3 = inp("rs_w3", [E // 2, 1])
    rs_b1 = inp("rs_b1", [E])
    rs_b2 = inp("rs_b2", [E // 2])
    rs_b3 = inp("rs_b3", [1])
    sp_w1 = inp("sp_w1", [E, E // 2])
    sp_w2 = inp("sp_w2", [E // 2, E // 4])
    sp_w3 = inp("sp_w3", [E // 4, 1])
    sp_b1 = inp("sp_b1", [E // 2])
    sp_b2 = inp("sp_b2", [E // 4])
    sp_b3 = inp("sp_b3", [1])
    ident = inp("ident", [128, 128])
    patt8 = inp("patt8", [128, BL])
    onehot = inp("onehot", [BL, R])
    bdm = inp("bdm", [R, BL])
    decay_r = inp("decay_r", [R])
    cthr_r = inp("cthr_r", [R])
    c2_r = inp("c2_r", [R])

    attn_out = nc.declare_dram_parameter("attn_out", [S, BL], F32, isOutput=True)
    feats_out = nc.declare_dram_parameter("feats_out", [BL, E], F32, isOutput=True)
    spans_out = nc.declare_dram_parameter("spans_out", [BL], I32, isOutput=True)

    enc_flat = enc.rearrange("s b e -> (s b) e")     # [2048, E]
    lx_flat = lx.rearrange("b s e -> (b s) e")       # [R, E], b-major rows

    with tile.TileContext(nc) as tc, \
            tc.tile_pool(name="wts", bufs=1) as wp, \
            tc.tile_pool(name="per", bufs=1) as pp, \
            tc.tile_pool(name="acts", bufs=1) as ap, \
            tc.tile_pool(name="encs", bufs=4) as ep, \
            tc.tile_pool(name="cyc", bufs=3) as cy, \
            tc.tile_pool(name="ptr", bufs=2, space="PSUM") as ptr, \
            tc.tile_pool(name="pacc", bufs=2, space="PSUM") as pacc, \
            tc.tile_pool(name="pwide", bufs=2, space="PSUM") as pwide, \
            tc.tile_pool(name="psml", bufs=2, space="PSUM") as psml:

        # ---- constants / biases / small inputs ----
        ident_t = pp.tile([128, 128], F32, tag="ident")
        nc.sync.dma_start(ident_t[:], ident[:])
        patt8_t = pp.tile([128, BL], F32, tag="patt8")
        nc.sync.dma_start(patt8_t[:], patt8[:])
        onehot_t = pp.tile([BL, R], F32, tag="onehot")
        nc.sync.dma_start(onehot_t[:], onehot[:])
        bdm_t = [pp.tile([128, BL], F32, tag=f"bdm{q}", name=f"bdm{q}") for q in range(NQ)]
        for q in range(NQ):
            nc.sync.dma_start(bdm_t[q][:], bdm[128 * q:128 * (q + 1), :])
        dec_t = pp.tile([1, R], F32, tag="dec")
        nc.sync.dma_start(dec_t[:], decay_r[:].rearrange("(o r) -> o r", o=1))
        cthr_t = pp.tile([1, R], F32, tag="cthr")
        nc.sync.dma_start(cthr_t[:], cthr_r[:].rearrange("(o r) -> o r", o=1))
        c2_t = pp.tile([1, R], F32, tag="c2")
        nc.sync.dma_start(c2_t[:], c2_r[:].rearrange("(o r) -> o r", o=1))
        rb1_t = pp.tile([128, 8], F32, tag="rb1")
        nc.sync.dma_start(rb1_t[:], rs_b1[:].rearrange("(m p) -> p m", p=128))
        rb2_t = pp.tile([128, 4], F32, tag="rb2")
        nc.sync.dma_start(rb2_t[:], rs_b2[:].rearrange("(m p) -> p m", p=128))
        rb3_t = pp.tile([1, 1], F32, tag="rb3")
        nc.sync.dma_start(rb3_t[:], rs_b3[:].rearrange("(p o) -> p o", p=1))
        sb1_t = pp.tile([128, 4], F32, tag="sb1")
        nc.sync.dma_start(sb1_t[:], sp_b1[:].rearrange("(m p) -> p m", p=128))
        sb2_t = pp.tile([128, 2], F32, tag="sb2")
        nc.sync.dma_start(sb2_t[:], sp_b2[:].rearrange("(m p) -> p m", p=128))
        sb3_t = pp.tile([1, 1], F32, tag="sb3")
        nc.sync.dma_start(sb3_t[:], sp_b3[:].rearrange("(p o) -> p o", p=1))

        # ---- long_x rows (b-major) ----
        lx_t = [pp.tile([128, E], F32, tag=f"lx{q}", name=f"lx{q}") for q in range(NQ)]
        for q in range(NQ):
            nc.sync.dma_start(lx_t[q][:], lx_flat[128 * q:128 * (q + 1), :])

        # ---- rs_w1: 16 row tiles [128, E] (top 8 = x-part, bottom 8 = ctx) ----
        w1_t = [wp.tile([128, E], F32, tag=f"w1_{k}", name=f"w1_{k}") for k in range(16)]
        for k in range(16):
            nc.sync.dma_start(w1_t[k][:], rs_w1[128 * k:128 * (k + 1), :])

        # ---- encoded_x: stream 16 chunks, DVE-accumulate ----
        acc = pp.tile([128, E], F32, tag="acc")
        for t in range(16):
            et = ep.tile([128, E], F32, tag="enc")
            nc.sync.dma_start(et[:], enc_flat[128 * t:128 * (t + 1), :])
            if t == 0:
                nc.vector.tensor_copy(acc[:], et[:])
            else:
                nc.vector.tensor_tensor(acc[:], acc[:], et[:], ALU.add)

        # ---- remaining weights ----
        w2_t = [wp.tile([128, E // 2], F32, tag=f"w2_{k}", name=f"w2_{k}") for k in range(8)]
        for k in range(8):
            nc.sync.dma_start(w2_t[k][:], rs_w2[128 * k:128 * (k + 1), :])
        w3_t = wp.tile([128, 4], F32, tag="w3")
        nc.sync.dma_start(w3_t[:], rs_w3[:].rearrange("(k p) o -> p (k o)", p=128))
        sw1_t = [wp.tile([128, E // 2], F32, tag=f"sw1_{k}", name=f"sw1_{k}") for k in range(8)]
        for k in range(8):
            nc.sync.dma_start(sw1_t[k][:], sp_w1[128 * k:128 * (k + 1), :])
        sw2_t = [wp.tile([128, E // 4], F32, tag=f"sw2_{k}", name=f"sw2_{k}") for k in range(4)]
        for k in range(4):
            nc.sync.dma_start(sw2_t[k][:], sp_w2[128 * k:128 * (k + 1), :])
        sw3_t = wp.tile([128, 2], F32, tag="sw3")
        nc.sync.dma_start(sw3_t[:], sp_w3[:].rearrange("(k p) o -> p (k o)", p=128))

        # ---- ctx = mean_s(encoded) : [BL, E] via pattern matmul + scale ----
        ctx_t = pp.tile([BL, E], F32, tag="ctx")
        for h in range(2):
            cps = pwide.tile([BL, 512], F32, tag="pwide")
            nc.tensor.matmul(cps[:], patt8_t[:], acc[:, 512 * h:512 * (h + 1)],
                             start=True, stop=True)
            nc.scalar.activation(ctx_t[:, 512 * h:512 * (h + 1)], cps[:],
                                 AF.Copy, scale=1.0 / 256.0)

        # ---- ctxT [E, BL] as [128, 8*8] (chunk c at cols 8c:8c+8) ----
        ctxT_t = pp.tile([128, 8 * BL], F32, tag="ctxT")
        for c in range(8):
            tps = ptr.tile([128, BL], F32, tag="ptr")
            nc.tensor.matmul(tps[:], ctx_t[0:BL, 128 * c:128 * (c + 1)],
                             ident_t[0:BL, 0:BL], is_transpose=True,
                             start=True, stop=True)
            nc.scalar.copy(ctxT_t[:, BL * c:BL * (c + 1)], tps[:])

        # ---- span predictor MLP (fp32, N=BL) ----
        h1_t = pp.tile([128, 4 * BL], F32, tag="h1")
        for m in range(4):
            ps = ptr.tile([128, BL], F32, tag="ptr")
            for k in range(8):
                nc.tensor.matmul(ps[:], sw1_t[k][:, 128 * m:128 * (m + 1)],
                                 ctxT_t[:, BL * k:BL * (k + 1)],
                                 start=(k == 0), stop=(k == 7))
            nc.scalar.activation(h1_t[:, BL * m:BL * (m + 1)], ps[:], AF.Gelu,
                                 bias=sb1_t[:, m:m + 1])
        h2_t = pp.tile([128, 2 * BL], F32, tag="h2")
        for m in range(2):
            ps = ptr.tile([128, BL], F32, tag="ptr")
            for k in range(4):
                nc.tensor.matmul(ps[:], sw2_t[k][:, 128 * m:128 * (m + 1)],
                                 h1_t[:, BL * k:BL * (k + 1)],
                                 start=(k == 0), stop=(k == 3))
            nc.scalar.activation(h2_t[:, BL * m:BL * (m + 1)], ps[:], AF.Gelu,
                                 bias=sb2_t[:, m:m + 1])
        ups = psml.tile([1, BL], F32, tag="psml")
        for k in range(2):
            nc.tensor.matmul(ups[:], sw3_t[:, k:k + 1],
                             h2_t[:, BL * k:BL * (k + 1)],
                             start=(k == 0), stop=(k == 1))
        sl_t = pp.tile([1, BL], F32, tag="sl")
        nc.scalar.activation(sl_t[:], ups[:], AF.Sigmoid, bias=sb3_t[0:1, 0:1])
        # spans = floor(sl*1016+8)  (round-to-nearest cast of x-0.5 == floor)
        tspan = pp.tile([1, BL], F32, tag="tspan")
        nc.vector.tensor_scalar(tspan[:], sl_t[:], float(MAX_SPAN - MIN_SPAN),
                                float(MIN_SPAN), ALU.mult, ALU.add)
        tsh = pp.tile([1, BL], F32, tag="tsh")
        nc.vector.tensor_scalar(tsh[:], tspan[:], 0.5, None, ALU.subtract)
        spans_i = pp.tile([1, BL], I32, tag="spans_i")
        nc.vector.tensor_copy(spans_i[:], tsh[:])
        span_f = pp.tile([1, BL], F32, tag="span_f")
        nc.vector.tensor_copy(span_f[:], spans_i[:])
        nc.sync.dma_start(spans_out[:].rearrange("(o b) -> o b", o=1), spans_i[:])

        # ---- c1 = ctx @ rs_w1[E:] in lhsT layout [BL, E] ----
        c1n_t = pp.tile([BL, E], F32, tag="c1n")
        for m in range(8):
            cps = ptr.tile([128, BL], F32, tag="ptr")
            for k in range(8):
                nc.tensor.matmul(cps[:], w1_t[8 + k][:, 128 * m:128 * (m + 1)],
                                 ctxT_t[:, BL * k:BL * (k + 1)],
                                 start=(k == 0), stop=(k == 7))
            c1T = cy.tile([128, BL], F32, tag="c1T")
            nc.scalar.copy(c1T[:], cps[:])
            nps = psml.tile([BL, 128], F32, tag="psml")
            nc.tensor.matmul(nps[:], c1T[:], ident_t[:, :], is_transpose=True,
                             start=True, stop=True)
            nc.scalar.copy(c1n_t[:, 128 * m:128 * (m + 1)], nps[:])

        # ---- relevance MLP over r-blocks ----
        zb_t = pp.tile([1, R], F32, tag="zb")
        rel_t = pp.tile([1, R], F32, tag="rel")
        for rb in range(NRB):
            r0 = rb * RB
            q0 = r0 // 128
            nqb = RB // 128
            # A0 = lx_block^T : 8 tiles [128, RB] (e-chunk c)
            a0 = [ap.tile([128, RB], F32, tag=f"a0_{c}", name=f"a0_{c}") for c in range(8)]
            for c in range(8):
                for qq in range(nqb):
                    tps = ptr.tile([128, 128], F32, tag="ptr")
                    nc.tensor.matmul(tps[:],
                                     lx_t[q0 + qq][:, 128 * c:128 * (c + 1)],
                                     ident_t[:, :], is_transpose=True,
                                     start=True, stop=True)
                    nc.scalar.copy(a0[c][:, 128 * qq:128 * (qq + 1)], tps[:])
            # L1: A1 = gelu(W1x^T A0 + c1 + b1)
            a1 = [ap.tile([128, RB], F32, tag=f"a1_{m}", name=f"a1_{m}") for m in range(8)]
            for m in range(8):
                ps = pacc.tile([128, RB], F32, tag="pacc")
                for k in range(8):
                    nc.tensor.matmul(ps[:], w1_t[k][:, 128 * m:128 * (m + 1)],
                                     a0[k][:], start=(k == 0), stop=False)
                nc.tensor.matmul(ps[:], c1n_t[0:BL, 128 * m:128 * (m + 1)],
                                 onehot_t[0:BL, r0:r0 + RB],
                                 start=False, stop=True)
                nc.scalar.activation(a1[m][:], ps[:], AF.Gelu,
                                     bias=rb1_t[:, m:m + 1])
            # L2: A2 = gelu(W2^T A1 + b2)
            a2 = [ap.tile([128, RB], F32, tag=f"a2_{m}", name=f"a2_{m}") for m in range(4)]
            for m in range(4):
                ps = pacc.tile([128, RB], F32, tag="pacc")
                for k in range(8):
                    nc.tensor.matmul(ps[:], w2_t[k][:, 128 * m:128 * (m + 1)],
                                     a1[k][:], start=(k == 0), stop=(k == 7))
                nc.scalar.activation(a2[m][:], ps[:], AF.Gelu,
                                     bias=rb2_t[:, m:m + 1])
            # L3: z = W3^T A2  -> zb = z + b3 ; rel = sigmoid(z + b3)
            zps = psml.tile([1, RB], F32, tag="psml")
            for k in range(4):
                nc.tensor.matmul(zps[:], w3_t[:, k:k + 1], a2[k][:],
                                 start=(k == 0), stop=(k == 3))
            nc.vector.tensor_scalar(zb_t[:, r0:r0 + RB], zps[:],
                                    rb3_t[0:1, 0:1], None, ALU.add)
            nc.scalar.activation(rel_t[:, r0:r0 + RB], zps[:], AF.Sigmoid,
                                 bias=rb3_t[0:1, 0:1])

        # ---- scores ----
        mask_t = pp.tile([1, R], F32, tag="mask")
        nc.vector.tensor_tensor(mask_t[:], zb_t[:], cthr_t[:], ALU.is_gt)
        # span broadcast per b-segment: spanb[r] = span[b(r)]
        stps = psml.tile([BL, 1], F32, tag="psml")
        nc.tensor.matmul(stps[:], span_f[0:1, 0:BL], ident_t[0:1, 0:1],
                         is_transpose=True, start=True, stop=True)
        spanT = pp.tile([BL, 1], F32, tag="spanT")
        nc.scalar.copy(spanT[:], stps[:])
        valid_t = pp.tile([1, R], F32, tag="valid")
        for n0 in range(0, R, 512):
            nn = min(512, R - n0)
            bps = psml.tile([1, 512], F32, tag="psml")
            nc.tensor.matmul(bps[:, 0:nn], spanT[:], onehot_t[:, n0:n0 + nn],
                             start=True, stop=True)
            nc.vector.tensor_tensor(valid_t[:, n0:n0 + nn], bps[:, 0:nn],
                                    c2_t[:, n0:n0 + nn], ALU.is_gt)
        fs_t = pp.tile([1, R], F32, tag="fs")
        nc.vector.tensor_tensor(fs_t[:], rel_t[:], dec_t[:], ALU.mult)
        nc.vector.tensor_tensor(fs_t[:], fs_t[:], mask_t[:], ALU.mult)
        nc.vector.tensor_scalar(fs_t[:], fs_t[:], float(hs_mean), None, ALU.mult)
        nc.vector.tensor_tensor(fs_t[:], fs_t[:], valid_t[:], ALU.mult)
        total_t = pp.tile([1, BL], F32, tag="total")
        nc.vector.tensor_reduce(total_t[:],
                                fs_t[:].rearrange("p (b s) -> p b s", b=BL),
                                AXL.X, ALU.add)
        den_t = pp.tile([1, BL], F32, tag="den")
        nc.vector.tensor_scalar(den_t[:], total_t[:], float(regp), None, ALU.add)
        recip_t = pp.tile([1, BL], F32, tag="recip")
        nc.vector.reciprocal(recip_t[:], den_t[:])
        rtps = psml.tile([BL, 1], F32, tag="psml")
        nc.tensor.matmul(rtps[:], recip_t[0:1, 0:BL], ident_t[0:1, 0:1],
                         is_transpose=True, start=True, stop=True)
        recipT = pp.tile([BL, 1], F32, tag="recipT")
        nc.scalar.copy(recipT[:], rtps[:])
        fsn_t = pp.tile([1, R], F32, tag="fsn")
        for n0 in range(0, R, 512):
            nn = min(512, R - n0)
            rps = psml.tile([1, 512], F32, tag="psml")
            nc.tensor.matmul(rps[:, 0:nn], recipT[:], onehot_t[:, n0:n0 + nn],
                             start=True, stop=True)
            nc.vector.tensor_tensor(fsn_t[:, n0:n0 + nn], fs_t[:, n0:n0 + nn],
                                    rps[:, 0:nn], ALU.mult)

        # ---- attn output ----
        for b in range(BL):
            nc.sync.dma_start(
                attn_out[S - S_active:S, b:b + 1].rearrange("s x -> (s x)")
                .rearrange("(o f) -> o f", o=1),
                fsn_t[:, S_active * b:S_active * (b + 1)])
        if ZROWS > 0:
            zcols = ZROWS // 128
            zt = pp.tile([128, zcols], F32, tag="zt")
            nc.vector.memset(zt[:], 0.0)
            nc.sync.dma_start(
                attn_out[0:S - S_active, :].rearrange("s b -> (s b)")
                .rearrange("(p f) -> p f", p=128), zt[:])

        # ---- feats = sum_s attn * long_x  (block-diag packed matmul) ----
        psf = [pwide.tile([BL, 512], F32, tag="pwide", name="psf") for _ in range(2)]
        for q in range(NQ):
            fps = ptr.tile([128, 1], F32, tag="ptr")
            nc.tensor.matmul(fps[:], fsn_t[:, 128 * q:128 * (q + 1)],
                             ident_t[0:1, 0:1], is_transpose=True,
                             start=True, stop=True)
            fsnT = cy.tile([128, 1], F32, tag="fsnT")
            nc.scalar.copy(fsnT[:], fps[:])
            bd = cy.tile([128, BL], F32, tag="bd")
            nc.vector.tensor_scalar(bd[:], bdm_t[q][:], fsnT[:], None, ALU.mult)
            for h in range(2):
                nc.tensor.matmul(psf[h][:], bd[:],
                                 lx_t[q][:, 512 * h:512 * (h + 1)],
                                 start=(q == 0), stop=(q == NQ - 1))
        feats_t = pp.tile([BL, E], F32, tag="feats")
        for h in range(2):
            nc.scalar.copy(feats_t[:, 512 * h:512 * (h + 1)], psf[h][:])
        nc.sync.dma_start(feats_out[:], feats_t[:])

    nc.compile()
    return nc


def _host_consts(S_active, td, thr):
    """Host-side constant tensors for one core (b-major rows)."""
    R = BL * S_active
    s_lo = S - S_active
    s_idx = np.arange(S)
    decay_full = (np.float64(td) ** (S - 1 - s_idx)).astype(np.float32)
    s_of_r = s_lo + (np.arange(R) % S_active)
    dec_r = decay_full[s_of_r]
    q = np.float64(thr) / dec_r.astype(np.float64)
    with np.errstate(divide="ignore", invalid="ignore"):
        logit = np.log(q / (1.0 - q))
    cthr = np.where(q >= 1.0, 1e30, np.where(q <= 0.0, -1e30, logit))
    cthr = cthr.astype(np.float32)
    c2 = (S - 1 - s_of_r).astype(np.float32)
    b_of_r = np.arange(R) // S_active
    onehot = (b_of_r[None, :] == np.arange(BL)[:, None]).astype(np.float32)
    bdm = onehot.T.copy()
    ident = np.eye(128, dtype=np.float32)
    patt8 = (np.arange(128)[:, None] % BL == np.arange(BL)[None, :]).astype(
        np.float32)
    return dict(ident=ident, patt8=patt8, onehot=onehot, bdm=bdm,
                decay_r=dec_r, cthr_r=cthr, c2_r=c2)


def _uniform_fallback(npv, spans):
    """Exact outputs when no position can pass the threshold (total==0
    everywhere): attn = valid/span, feats accordingly."""
    lx = npv["long_x"].astype(np.float32)
    start = (S - spans).astype(np.int32)
    pos = np.arange(S)
    valid = (pos[:, None] >= start[None, :]).astype(np.float32)
    attn = valid / spans[None, :].astype(np.float32)
    feats = np.einsum("sb,sbe->be", attn, lx).astype(np.float32)
    return attn, feats, spans


def _numpy_full_reference(npv):
    """Exact float32 numpy replica of the reference (slow, safety net for
    near-dense threshold patterns that the sparse device kernel doesn't
    cover)."""
    from scipy.special import erf
    f32 = np.float32
    lx = npv["long_x"].astype(f32)
    ctx = npv["encoded_x"].astype(f32).mean(axis=0, dtype=f32)

    def gelu(x):
        return (0.5 * x * (1 + erf(x / np.sqrt(2)))).astype(f32)

    spans = _numpy_spans(npv)
    start = S - spans
    pos = np.arange(S)
    valid = (pos[:, None] >= start[None, :]).astype(f32)
    comb = np.concatenate([lx, np.broadcast_to(ctx[None], lx.shape)], axis=-1)
    r = gelu(comb.reshape(-1, 2 * E) @ npv["rs_w1"].astype(f32) +
             npv["rs_b1"].astype(f32))
    r = gelu(r @ npv["rs_w2"].astype(f32) + npv["rs_b2"].astype(f32))
    rel = 1 / (1 + np.exp(-(r @ npv["rs_w3"].astype(f32) +
                            npv["rs_b3"].astype(f32))))
    rel = rel[:, 0].reshape(S, B).astype(f32)
    td = f32(npv["temporal_decay"])
    decay = (np.float64(td) ** (S - 1 - pos)).astype(f32)
    fs = rel * decay[:, None]
    fs = fs * (fs > f32(npv["adaptive_threshold"]))
    fs = fs * f32(np.asarray(npv["head_scale"], np.float64).mean())
    fs = fs * valid
    total = fs.sum(axis=0, dtype=f32)
    normed = fs / (total[None, :] + f32(npv["attention_reg"]) + f32(1e-8))
    uniform = valid / spans[None, :].astype(f32)
    attn = np.where(total[None, :] > 0, normed, uniform).astype(f32)
    feats = np.einsum("sb,sbe->be", attn, lx).astype(f32)
    return attn, feats, spans


def _numpy_spans(npv):
    from scipy.special import erf
    ctx = npv["encoded_x"].astype(np.float64).mean(axis=0)

    def gelu(x):
        return 0.5 * x * (1 + erf(x / np.sqrt(2)))

    h = gelu(ctx @ npv["sp_w1"].astype(np.float64) + npv["sp_b1"])
    h = gelu(h @ npv["sp_w2"].astype(np.float64) + npv["sp_b2"])
    sl = 1 / (1 + np.exp(-(h @ npv["sp_w3"].astype(np.float64) + npv["sp_b3"])))
    sl = sl[:, 0].astype(np.float32)
    spans = np.minimum((sl * (MAX_SPAN - MIN_SPAN) + MIN_SPAN).astype(np.int32),
                       S)
    return spans


def kernel(**inputs):
    npv = {k: np.asarray(v) for k, v in inputs.items()}
    f32 = np.float32
    td = float(f32(npv["temporal_decay"]))
    thr = float(f32(npv["adaptive_threshold"]))
    reg = float(f32(npv["attention_reg"]))
    hs_mean = float(f32(np.asarray(npv["head_scale"], np.float64).mean()))
    regp = float(f32(reg) + f32(1e-8))

    decay_full = (np.float64(td) ** (S - 1 - np.arange(S))).astype(np.float32)
    passing = decay_full > f32(thr)
    if not passing.any():
        return _uniform_fallback(npv, _numpy_spans(npv))
    s_min = int(np.nonzero(passing)[0].min())
    S_active = min(S, int(np.ceil((S - s_min + 8) / 32.0)) * 32)
    if S_active > 128:
        return _numpy_full_reference(npv)
    s_lo = S - S_active

    nc = _build(S_active, hs_mean, regp)
    consts = _host_consts(S_active, td, thr)

    def c(a):
        return np.ascontiguousarray(a, dtype=np.float32)

    shared = dict(
        rs_w1=c(npv["rs_w1"]), rs_w2=c(npv["rs_w2"]), rs_w3=c(npv["rs_w3"]),
        rs_b1=c(npv["rs_b1"]), rs_b2=c(npv["rs_b2"]), rs_b3=c(npv["rs_b3"]),
        sp_w1=c(npv["sp_w1"]), sp_w2=c(npv["sp_w2"]), sp_w3=c(npv["sp_w3"]),
        sp_b1=c(npv["sp_b1"]), sp_b2=c(npv["sp_b2"]), sp_b3=c(npv["sp_b3"]),
        **{k: c(v) for k, v in consts.items()})
    in_maps = []
    for i in range(NCORES):
        bs = slice(i * BL, (i + 1) * BL)
        in_maps.append(dict(
            lx=c(npv["long_x"][s_lo:, bs, :].transpose(1, 0, 2)),
            enc=c(npv["encoded_x"][:, bs, :]),
            **shared))

    res = run_bass_kernel_spmd(nc, in_maps, core_ids=list(range(NCORES)))
    global LAST_RESULT
    LAST_RESULT = res
    rs = res.results
    attn = np.concatenate([r["attn_out"] for r in rs], axis=1)
    feats = np.concatenate([r["feats_out"] for r in rs], axis=0)
    spans = np.concatenate([r["spans_out"] for r in rs], axis=0).astype(np.int32)
    return attn, feats, spans


if __name__ == "__main__":
    rng = np.random.default_rng(0)
    print("smoke test requires reference inputs; use test.py")


# revision 10
# speedup vs baseline: 1.3262x; 1.3262x over previous
"""AdaptiveSpanAttention TRN2 kernel: 8-way batch-parallel Bass/Tile kernel.

Structure exploited: fs = sigmoid(z)*decay_s can only exceed adaptive_threshold
where decay_s > threshold (since sigmoid < 1). For the reference scalars
(decay 0.95, thr 0.3) that is only the last 24 of 1024 positions, so the big
relevance MLP runs only on that row suffix. The threshold mask is evaluated on
the pre-sigmoid logit z against host-precomputed c_s = logit(thr/decay_s),
which is exactly monotone-equivalent and immune to LUT rounding at the
discontinuity. All matmuls run in true fp32 on the PE (4 cyc/row).

Sharding: batch dim (axis 1 of long_x/encoded_x) across the 8 NeuronCores,
weights replicated, outputs concatenated on host (pure data parallel).
"""
import sys

import numpy as np

sys.path.insert(0, "/opt/trn_rl_repo")

import concourse.tile as tile  # noqa: E402
from concourse import bacc, mybir  # noqa: E402
from concourse.bass_utils import run_bass_kernel_spmd  # noqa: E402

F32 = mybir.dt.float32
F32R = mybir.dt.float32r
BF16 = mybir.dt.bfloat16
I32 = mybir.dt.int32
AF = mybir.ActivationFunctionType
ALU = mybir.AluOpType
AXL = mybir.AxisListType

S, B, E, H = 1024, 64, 1024, 16
NCORES = 8
BL = B // NCORES  # 8 batches per core
MAX_SPAN, MIN_SPAN = 1024, 8
LAST_RESULT = None


def _build(S_active, hs_mean, regp):
    R = BL * S_active
    NQ = R // 128
    RB = min(R, 512)
    NRB = R // RB
    s_lo = S - S_active
    ZROWS = s_lo * BL

    nc = bacc.Bacc("TRN2", target_bir_lowering=False, debug=False,
                   num_devices=NCORES)

    def inp(name, shape, dt=F32):
        return nc.declare_dram_parameter(name, shape, dt, isOutput=False)

    lx = inp("lx", [BL, S_active, E])
    encb = inp("encb", [256, BL, E], BF16)
    w1x = inp("w1x", [E, E], F32R)          # rs_w1 top half (f32 bytes)
    w1c = inp("w1c", [E, E], BF16)          # rs_w1 bottom half, bf16
    rs_w2 = inp("rs_w2", [E, E // 2])
    rs_w3 = inp("rs_w3", [E // 2, 1])
    rs_b1 = inp("rs_b1", [E])
    rs_b2 = inp("rs_b2", [E // 2])
    rs_b3 = inp("rs_b3", [1])
    sw1 = inp("sw1", [E, E // 2], BF16)
    sw2 = inp("sw2", [E // 2, E // 4], BF16)
    sw3 = inp("sw3", [E // 4, 1], BF16)
    sp_b1 = inp("sp_b1", [E // 2])
    sp_b2 = inp("sp_b2", [E // 4])
    sp_b3 = inp("sp_b3", [1])
    ident = inp("ident", [128, 128])
    patt8 = inp("patt8", [128, BL], BF16)
    onehot = inp("onehot", [BL, R], F32R)
    bdm = inp("bdm", [R, BL])
    decay_r = inp("decay_r", [R])
    cthr_r = inp("cthr_r", [R])
    c2_r = inp("c2_r", [R])

    attn_out = nc.declare_dram_parameter("attn_out", [S, BL], F32, isOutput=True)
    feats_out = nc.declare_dram_parameter("feats_out", [BL, E], F32, isOutput=True)
    spans_out = nc.declare_dram_parameter("spans_out", [BL], I32, isOutput=True)

    enc_flat = encb.rearrange("s b e -> (s b) e")   # [2048, E]
    lx_flat = lx.rearrange("b s e -> (b s) e")      # [R, E]

    with tile.TileContext(nc) as tc, \
            tc.tile_pool(name="wts", bufs=1) as wp, \
            tc.tile_pool(name="per", bufs=1) as pp, \
            tc.tile_pool(name="acts", bufs=1) as ap, \
            tc.tile_pool(name="encs", bufs=16) as ep, \
            tc.tile_pool(name="cyc", bufs=3) as cy, \
            tc.tile_pool(name="pacc1", bufs=2, space="PSUM") as pacc1, \
            tc.tile_pool(name="pacc2", bufs=4, space="PSUM") as pacc2, \
            tc.tile_pool(name="ptr", bufs=1, space="PSUM") as ptr, \
            tc.tile_pool(name="psml", bufs=1, space="PSUM") as psml:

        # ---------------- DMAs: consts, lx, enc, weights ----------------
        ident_t = pp.tile([128, 128], F32, tag="ident")
        nc.sync.dma_start(ident_t[:], ident[:])
        patt8_t = pp.tile([128, BL], BF16, tag="patt8")
        nc.sync.dma_start(patt8_t[:], patt8[:])
        onehot_t = pp.tile([BL, R], F32R, tag="onehot")
        nc.sync.dma_start(onehot_t[:], onehot[:])
        bdm_t = [pp.tile([128, BL], F32, tag=f"bdm{q}", name=f"bdm{q}")
                 for q in range(NQ)]
        for q in range(NQ):
            nc.sync.dma_start(bdm_t[q][:], bdm[128 * q:128 * (q + 1), :])
        dec_t = pp.tile([1, R], F32, tag="dec")
        nc.sync.dma_start(dec_t[:], decay_r[:].rearrange("(o r) -> o r", o=1))
        cthr_t = pp.tile([1, R], F32, tag="cthr")
        nc.sync.dma_start(cthr_t[:], cthr_r[:].rearrange("(o r) -> o r", o=1))
        c2_t = pp.tile([1, R], F32, tag="c2")
        nc.sync.dma_start(c2_t[:], c2_r[:].rearrange("(o r) -> o r", o=1))
        rb1_t = pp.tile([128, 8], F32, tag="rb1")
        nc.sync.dma_start(rb1_t[:], rs_b1[:].rearrange("(m p) -> p m", p=128))
        rb2_t = pp.tile([128, 4], F32, tag="rb2")
        nc.sync.dma_start(rb2_t[:], rs_b2[:].rearrange("(m p) -> p m", p=128))
        rb3_t = pp.tile([1, 1], F32, tag="rb3")
        nc.sync.dma_start(rb3_t[:], rs_b3[:].rearrange("(p o) -> p o", p=1))
        sb1_t = pp.tile([128, 4], F32, tag="sb1")
        nc.sync.dma_start(sb1_t[:], sp_b1[:].rearrange("(m p) -> p m", p=128))
        sb2_t = pp.tile([128, 2], F32, tag="sb2")
        nc.sync.dma_start(sb2_t[:], sp_b2[:].rearrange("(m p) -> p m", p=128))
        sb3_t = pp.tile([1, 1], F32, tag="sb3")
        nc.sync.dma_start(sb3_t[:], sp_b3[:].rearrange("(p o) -> p o", p=1))

        lx_t = [pp.tile([128, E], F32, tag=f"lx{q}", name=f"lx{q}")
                for q in range(NQ)]
        for q in range(NQ):
            nc.sync.dma_start(lx_t[q][:], lx_flat[128 * q:128 * (q + 1), :])

        enc_tiles = []
        for t in range(16):
            et = ep.tile([128, E], BF16, tag="enc", name="enc_t")
            nc.sync.dma_start(et[:], enc_flat[128 * t:128 * (t + 1), :])
            enc_tiles.append(et)

        # column-tiled rs_w1 halves + row-tiled rs_w2, interleaved per m
        wc_t = [wp.tile([128, E], BF16, tag=f"wc{m}", name=f"wc{m}")
                for m in range(8)]
        wm_t = [wp.tile([128, E], F32R, tag=f"wm{m}", name=f"wm{m}")
                for m in range(8)]
        w2_t = [wp.tile([128, E // 2], F32, tag=f"w2_{k}", name=f"w2_{k}")
                for k in range(8)]
        for m in range(8):
            for k in range(8):
                nc.sync.dma_start(
                    wc_t[m][:, 128 * k:128 * (k + 1)],
                    w1c[128 * k:128 * (k + 1), 128 * m:128 * (m + 1)])
            for k in range(8):
                nc.sync.dma_start(
                    wm_t[m][:, 128 * k:128 * (k + 1)],
                    w1x[128 * k:128 * (k + 1), 128 * m:128 * (m + 1)])
            nc.sync.dma_start(w2_t[m][:], rs_w2[128 * m:128 * (m + 1), :])
        w3_t = wp.tile([128, 4], F32, tag="w3")
        nc.sync.dma_start(w3_t[:], rs_w3[:].rearrange("(k p) o -> p (k o)", p=128))
        sw1_t = [wp.tile([128, E // 2], BF16, tag=f"sw1_{k}", name=f"sw1_{k}")
                 for k in range(8)]
        for k in range(8):
            nc.sync.dma_start(sw1_t[k][:], sw1[128 * k:128 * (k + 1), :])
        sw2_t = [wp.tile([128, E // 4], BF16, tag=f"sw2_{k}", name=f"sw2_{k}")
                 for k in range(4)]
        for k in range(4):
            nc.sync.dma_start(sw2_t[k][:], sw2[128 * k:128 * (k + 1), :])
        sw3_t = wp.tile([128, 2], BF16, tag="sw3")
        nc.sync.dma_start(sw3_t[:], sw3[:].rearrange("(k p) o -> p (k o)", p=128))

        # ---------------- PE: A0 transposes (lx^T), fp32 -> f32r --------
        a0 = [ap.tile([128, R], F32R, tag=f"a0_{c}", name=f"a0_{c}")
              for c in range(8)]
        for c in range(8):
            for q in range(NQ):
                tps = ptr.tile([128, 128], F32, tag="ptr", name="tps")
                nc.tensor.matmul(tps[:], lx_t[q][:, 128 * c:128 * (c + 1)],
                                 ident_t[:, :], is_transpose=True,
                                 start=True, stop=True)
                nc.scalar.copy(a0[c][:, 128 * q:128 * (q + 1)], tps[:])

        # ---------------- ctx = mean_s(enc) on PE (bf16) ----------------
        ctx_t = pp.tile([BL, E], F32, tag="ctx")
        cps = [pacc1.tile([BL, 512], F32, tag="pacc1", name="cps")
               for _ in range(2)]
        for t in range(16):
            for h in range(2):
                nc.tensor.matmul(cps[h][:], patt8_t[:],
                                 enc_tiles[t][:, 512 * h:512 * (h + 1)],
                                 start=(t == 0), stop=(t == 15))
        for h in range(2):
            nc.scalar.activation(ctx_t[:, 512 * h:512 * (h + 1)], cps[h][:],
                                 AF.Copy, scale=1.0 / 256.0)

        # ---------------- ctxT (bf16) for span-MLP + c1 -----------------
        ctxTb = pp.tile([128, 8 * BL], BF16, tag="ctxTb")
        for c in range(8):
            tps2 = ptr.tile([128, BL], F32, tag="ptr", name="tps2")
            nc.tensor.matmul(tps2[:], ctx_t[0:BL, 128 * c:128 * (c + 1)],
                             ident_t[0:BL, 0:BL], is_transpose=True,
                             start=True, stop=True)
            nc.scalar.copy(ctxTb[:, BL * c:BL * (c + 1)], tps2[:])

        # ---------------- c1 = ctx @ rs_w1[E:]  (bf16 -> f32r) ----------
        c1n_t = pp.tile([BL, E], F32R, tag="c1n")
        for m in range(8):
            c1ps = ptr.tile([BL, 128], F32, tag="ptr", name="c1ps")
            for k in range(8):
                nc.tensor.matmul(c1ps[:], ctxTb[:, BL * k:BL * (k + 1)],
                                 wc_t[m][:, 128 * k:128 * (k + 1)],
                                 start=(k == 0), stop=(k == 7))
            nc.scalar.copy(c1n_t[:, 128 * m:128 * (m + 1)], c1ps[:])

        # ---------------- main MLP: per-m pipelined L1 -> L2 ------------
        zb_t = pp.tile([1, R], F32, tag="zb")
        rel_t = pp.tile([1, R], F32, tag="rel")
        for rb in range(NRB):
            r0 = rb * RB
            q0 = r0 // 128
            a1 = [ap.tile([128, RB], F32, tag=f"a1_{m}", name=f"a1_{m}")
                  for m in range(8)]
            ps2 = [pacc2.tile([128, RB], F32, tag="pacc2", name="ps2")
                   for _ in range(4)]
            for m in range(8):
                ps1 = pacc1.tile([128, RB], F32, tag="pacc1", name="ps1")
                for k in range(8):
                    nc.tensor.matmul(ps1[:], wm_t[m][:, 128 * k:128 * (k + 1)],
                                     a0[k][:, r0:r0 + RB],
                                     start=(k == 0), stop=False)
                nc.tensor.matmul(ps1[:], c1n_t[0:BL, 128 * m:128 * (m + 1)],
                                 onehot_t[0:BL, r0:r0 + RB],
                                 start=False, stop=True)
                nc.scalar.activation(a1[m][:], ps1[:], AF.Gelu,
                                     bias=rb1_t[:, m:m + 1])
                for m2 in range(4):
                    nc.tensor.matmul(ps2[m2][:],
                                     w2_t[m][:, 128 * m2:128 * (m2 + 1)],
                                     a1[m][:], start=(m == 0), stop=(m == 7))
            zps = psml.tile([1, RB], F32, tag="psml", name="zps")
            for m2 in range(4):
                a2m = ap.tile([128, RB], F32, tag=f"a2_{m2}", name=f"a2_{m2}")
                nc.scalar.activation(a2m[:], ps2[m2][:], AF.Gelu,
                                     bias=rb2_t[:, m2:m2 + 1])
                nc.tensor.matmul(zps[:], w3_t[:, m2:m2 + 1], a2m[:],
                                 start=(m2 == 0), stop=(m2 == 3))
            nc.vector.tensor_scalar(zb_t[:, r0:r0 + RB], zps[:],
                                    rb3_t[0:1, 0:1], None, ALU.add)
            nc.scalar.activation(rel_t[:, r0:r0 + RB], zps[:], AF.Sigmoid,
                                 bias=rb3_t[0:1, 0:1])

        # ---------------- span predictor MLP (bf16) ---------------------
        h1b = pp.tile([128, 4 * BL], BF16, tag="h1b")
        for m in range(4):
            hps = ptr.tile([128, BL], F32, tag="ptr", name="hps")
            for k in range(8):
                nc.tensor.matmul(hps[:], sw1_t[k][:, 128 * m:128 * (m + 1)],
                                 ctxTb[:, BL * k:BL * (k + 1)],
                                 start=(k == 0), stop=(k == 7))
            nc.scalar.activation(h1b[:, BL * m:BL * (m + 1)], hps[:], AF.Gelu,
                                 bias=sb1_t[:, m:m + 1])
        h2b = pp.tile([128, 2 * BL], BF16, tag="h2b")
        for m in range(2):
            hps2 = ptr.tile([128, BL], F32, tag="ptr", name="hps2")
            for k in range(4):
                nc.tensor.matmul(hps2[:], sw2_t[k][:, 128 * m:128 * (m + 1)],
                                 h1b[:, BL * k:BL * (k + 1)],
                                 start=(k == 0), stop=(k == 3))
            nc.scalar.activation(h2b[:, BL * m:BL * (m + 1)], hps2[:], AF.Gelu,
                                 bias=sb2_t[:, m:m + 1])
        ups = psml.tile([1, BL], F32, tag="psml", name="ups")
        for k in range(2):
            nc.tensor.matmul(ups[:], sw3_t[:, k:k + 1],
                             h2b[:, BL * k:BL * (k + 1)],
                             start=(k == 0), stop=(k == 1))
        sl_t = pp.tile([1, BL], F32, tag="sl")
        nc.scalar.activation(sl_t[:], ups[:], AF.Sigmoid, bias=sb3_t[0:1, 0:1])
        tspan = pp.tile([1, BL], F32, tag="tspan")
        nc.vector.tensor_scalar(tspan[:], sl_t[:], float(MAX_SPAN - MIN_SPAN),
                                float(MIN_SPAN), ALU.mult, ALU.add)
        tsh = pp.tile([1, BL], F32, tag="tsh")
        nc.vector.tensor_scalar(tsh[:], tspan[:], 0.5, None, ALU.subtract)
        spans_i = pp.tile([1, BL], I32, tag="spans_i")
        nc.vector.tensor_copy(spans_i[:], tsh[:])
        span_f = pp.tile([1, BL], F32, tag="span_f")
        nc.vector.tensor_copy(span_f[:], spans_i[:])
        nc.sync.dma_start(spans_out[:].rearrange("(o b) -> o b", o=1), spans_i[:])

        # ---------------- scores (DVE, fp32) ----------------------------
        mask_t = pp.tile([1, R], F32, tag="mask")
        nc.vector.tensor_tensor(mask_t[:], zb_t[:], cthr_t[:], ALU.is_gt)
        valid_t = pp.tile([1, R], F32, tag="valid")
        for b in range(BL):
            sl_ = slice(S_active * b, S_active * (b + 1))
            nc.vector.tensor_scalar(valid_t[:, sl_], c2_t[:, sl_],
                                    span_f[0:1, b:b + 1], None, ALU.is_lt)
        fs_t = pp.tile([1, R], F32, tag="fs")
        nc.vector.tensor_tensor(fs_t[:], rel_t[:], dec_t[:], ALU.mult)
        nc.vector.tensor_tensor(fs_t[:], fs_t[:], mask_t[:], ALU.mult)
        nc.vector.tensor_scalar(fs_t[:], fs_t[:], float(hs_mean), None, ALU.mult)
        nc.vector.tensor_tensor(fs_t[:], fs_t[:], valid_t[:], ALU.mult)
        total_t = pp.tile([1, BL], F32, tag="total")
        nc.vector.tensor_reduce(total_t[:],
                                fs_t[:].rearrange("p (b s) -> p b s", b=BL),
                                AXL.X, ALU.add)
        den_t = pp.tile([1, BL], F32, tag="den")
        nc.vector.tensor_scalar(den_t[:], total_t[:], float(regp), None, ALU.add)
        recip_t = pp.tile([1, BL], F32, tag="recip")
        nc.vector.reciprocal(recip_t[:], den_t[:])
        fsn_t = pp.tile([1, R], F32, tag="fsn")
        for b in range(BL):
            sl_ = slice(S_active * b, S_active * (b + 1))
            nc.vector.tensor_scalar(fsn_t[:, sl_], fs_t[:, sl_],
                                    recip_t[0:1, b:b + 1], None, ALU.mult)

        # ---------------- attn output -----------------------------------
        for b in range(BL):
            nc.sync.dma_start(
                attn_out[S - S_active:S, b:b + 1].rearrange("s x -> (s x)")
                .rearrange("(o f) -> o f", o=1),
                fsn_t[:, S_active * b:S_active * (b + 1)])
        if ZROWS > 0:
            zcols = ZROWS // 128
            zt = pp.tile([128, zcols], F32, tag="zt")
            nc.vector.memset(zt[:], 0.0)
            nc.sync.dma_start(
                attn_out[0:S - S_active, :].rearrange("s b -> (s b)")
                .rearrange("(p f) -> p f", p=128), zt[:])

        # ---------------- feats (block-diag packed fp32 matmul) ---------
        psf = [pacc2.tile([BL, 512], F32, tag="pacc2", name="psf")
               for _ in range(2)]
        for q in range(NQ):
            fps = ptr.tile([128, 1], F32, tag="ptr", name="fps")
            nc.tensor.matmul(fps[:], fsn_t[:, 128 * q:128 * (q + 1)],
                             ident_t[0:1, 0:1], is_transpose=True,
                             start=True, stop=True)
            fsnT = cy.tile([128, 1], F32, tag="fsnT", name="fsnT")
            nc.scalar.copy(fsnT[:], fps[:])
            bd = cy.tile([128, BL], F32, tag="bd", name="bd")
            nc.vector.tensor_scalar(bd[:], bdm_t[q][:], fsnT[:], None, ALU.mult)
            for h in range(2):
                nc.tensor.matmul(psf[h][:], bd[:],
                                 lx_t[q][:, 512 * h:512 * (h + 1)],
                                 start=(q == 0), stop=(q == NQ - 1))
        feats_t = pp.tile([BL, E], F32, tag="feats")
        for h in range(2):
            nc.scalar.copy(feats_t[:, 512 * h:512 * (h + 1)], psf[h][:])
        nc.sync.dma_start(feats_out[:], feats_t[:])

    nc.compile()
    return nc


def _host_consts(S_active, td, thr):
    """Host-side constant tensors for one core (b-major rows)."""
    R = BL * S_active
    s_lo = S - S_active
    s_idx = np.arange(S)
    decay_full = (np.float64(td) ** (S - 1 - s_idx)).astype(np.float32)
    s_of_r = s_lo + (np.arange(R) % S_active)
    dec_r = decay_full[s_of_r]
    q = np.float64(thr) / dec_r.astype(np.float64)
    with np.errstate(divide="ignore", invalid="ignore"):
        logit = np.log(q / (1.0 - q))
    cthr = np.where(q >= 1.0, 1e30, np.where(q <= 0.0, -1e30, logit))
    cthr = cthr.astype(np.float32)
    c2 = (S - 1 - s_of_r).astype(np.float32)
    b_of_r = np.arange(R) // S_active
    onehot = (b_of_r[None, :] == np.arange(BL)[:, None]).astype(np.float32)
    bdm = onehot.T.copy()
    ident = np.eye(128, dtype=np.float32)
    patt8 = (np.arange(128)[:, None] % BL == np.arange(BL)[None, :]).astype(
        np.float32)
    return dict(ident=ident, patt8=patt8, onehot=onehot, bdm=bdm,
                decay_r=dec_r, cthr_r=cthr, c2_r=c2)


def _uniform_fallback(npv, spans):
    """Exact outputs when no position can pass the threshold (total==0
    everywhere): attn = valid/span, feats accordingly."""
    lx = npv["long_x"].astype(np.float32)
    start = (S - spans).astype(np.int32)
    pos = np.arange(S)
    valid = (pos[:, None] >= start[None, :]).astype(np.float32)
    attn = valid / spans[None, :].astype(np.float32)
    feats = np.einsum("sb,sbe->be", attn, lx).astype(np.float32)
    return attn, feats, spans


def _numpy_full_reference(npv):
    """Exact float32 numpy replica of the reference (slow, safety net for
    near-dense threshold patterns that the sparse device kernel doesn't
    cover)."""
    from scipy.special import erf
    f32 = np.float32
    lx = npv["long_x"].astype(f32)
    ctx = npv["encoded_x"].astype(f32).mean(axis=0, dtype=f32)

    def gelu(x):
        return (0.5 * x * (1 + erf(x / np.sqrt(2)))).astype(f32)

    spans = _numpy_spans(npv)
    start = S - spans
    pos = np.arange(S)
    valid = (pos[:, None] >= start[None, :]).astype(f32)
    comb = np.concatenate([lx, np.broadcast_to(ctx[None], lx.shape)], axis=-1)
    r = gelu(comb.reshape(-1, 2 * E) @ npv["rs_w1"].astype(f32) +
             npv["rs_b1"].astype(f32))
    r = gelu(r @ npv["rs_w2"].astype(f32) + npv["rs_b2"].astype(f32))
    rel = 1 / (1 + np.exp(-(r @ npv["rs_w3"].astype(f32) +
                            npv["rs_b3"].astype(f32))))
    rel = rel[:, 0].reshape(S, B).astype(f32)
    td = f32(npv["temporal_decay"])
    decay = (np.float64(td) ** (S - 1 - pos)).astype(f32)
    fs = rel * decay[:, None]
    fs = fs * (fs > f32(npv["adaptive_threshold"]))
    fs = fs * f32(np.asarray(npv["head_scale"], np.float64).mean())
    fs = fs * valid
    total = fs.sum(axis=0, dtype=f32)
    normed = fs / (total[None, :] + f32(npv["attention_reg"]) + f32(1e-8))
    uniform = valid / spans[None, :].astype(f32)
    attn = np.where(total[None, :] > 0, normed, uniform).astype(f32)
    feats = np.einsum("sb,sbe->be", attn, lx).astype(f32)
    return attn, feats, spans


def _numpy_spans(npv):
    from scipy.special import erf
    ctx = npv["encoded_x"].astype(np.float64).mean(axis=0)

    def gelu(x):
        return 0.5 * x * (1 + erf(x / np.sqrt(2)))

    h = gelu(ctx @ npv["sp_w1"].astype(np.float64) + npv["sp_b1"])
    h = gelu(h @ npv["sp_w2"].astype(np.float64) + npv["sp_b2"])
    sl = 1 / (1 + np.exp(-(h @ npv["sp_w3"].astype(np.float64) + npv["sp_b3"])))
    sl = sl[:, 0].astype(np.float32)
    spans = np.minimum((sl * (MAX_SPAN - MIN_SPAN) + MIN_SPAN).astype(np.int32),
                       S)
    return spans


def kernel(**inputs):
    npv = {k: np.asarray(v) for k, v in inputs.items()}
    f32 = np.float32
    td = float(f32(npv["temporal_decay"]))
    thr = float(f32(npv["adaptive_threshold"]))
    reg = float(f32(npv["attention_reg"]))
    hs_mean = float(f32(np.asarray(npv["head_scale"], np.float64).mean()))
    regp = float(f32(reg) + f32(1e-8))

    decay_full = (np.float64(td) ** (S - 1 - np.arange(S))).astype(np.float32)
    passing = decay_full > f32(thr)
    if not passing.any():
        return _uniform_fallback(npv, _numpy_spans(npv))
    s_min = int(np.nonzero(passing)[0].min())
    S_active = min(S, int(np.ceil((S - s_min + 8) / 32.0)) * 32)
    if S_active > 128:
        return _numpy_full_reference(npv)
    s_lo = S - S_active

    nc = _build(S_active, hs_mean, regp)
    consts = _host_consts(S_active, td, thr)

    import ml_dtypes
    bf16 = ml_dtypes.bfloat16

    def c(a, dt=np.float32):
        return np.ascontiguousarray(np.asarray(a, dtype=np.float32).astype(dt))

    rw1 = np.asarray(npv["rs_w1"], np.float32)
    shared = dict(
        w1x=c(rw1[:E]), w1c=c(rw1[E:], bf16),
        rs_w2=c(npv["rs_w2"]), rs_w3=c(npv["rs_w3"]),
        rs_b1=c(npv["rs_b1"]), rs_b2=c(npv["rs_b2"]), rs_b3=c(npv["rs_b3"]),
        sw1=c(npv["sp_w1"], bf16), sw2=c(npv["sp_w2"], bf16),
        sw3=c(npv["sp_w3"], bf16),
        sp_b1=c(npv["sp_b1"]), sp_b2=c(npv["sp_b2"]), sp_b3=c(npv["sp_b3"]),
        **{k: (c(v, bf16) if k == "patt8" else c(v)) for k, v in consts.items()})
    in_maps = []
    for i in range(NCORES):
        bs = slice(i * BL, (i + 1) * BL)
        in_maps.append(dict(
            lx=c(npv["long_x"][s_lo:, bs, :].transpose(1, 0, 2)),
            encb=c(npv["encoded_x"][:, bs, :], bf16),
            **shared))

    res = run_bass_kernel_spmd(nc, in_maps, core_ids=list(range(NCORES)))
    global LAST_RESULT
    LAST_RESULT = res
    rs = res.results
    attn = np.concatenate([r["attn_out"] for r in rs], axis=1)
    feats = np.concatenate([r["feats_out"] for r in rs], axis=0)
    spans = np.concatenate([r["spans_out"] for r in rs], axis=0).astype(np.int32)
    return attn, feats, spans


if __name__ == "__main__":
    rng = np.random.default_rng(0)
    print("smoke test requires reference inputs; use test.py")


# revision 11
# speedup vs baseline: 1.5491x; 1.1681x over previous
"""AdaptiveSpanAttention TRN2 kernel: 8-way batch-parallel Bass/Tile kernel.

Structure exploited: fs = sigmoid(z)*decay_s can only exceed adaptive_threshold
where decay_s > threshold (since sigmoid < 1). For the reference scalars
(decay 0.95, thr 0.3) that is only the last 24 of 1024 positions, so the big
relevance MLP runs only on that row suffix. The threshold mask is evaluated on
the pre-sigmoid logit z against host-precomputed c_s = logit(thr/decay_s),
which is exactly monotone-equivalent and immune to LUT rounding at the
discontinuity. All matmuls run in true fp32 on the PE (4 cyc/row).

Sharding: batch dim (axis 1 of long_x/encoded_x) across the 8 NeuronCores,
weights replicated, outputs concatenated on host (pure data parallel).
"""
import sys

import numpy as np

sys.path.insert(0, "/opt/trn_rl_repo")

import concourse.tile as tile  # noqa: E402
from concourse import bacc, mybir  # noqa: E402
from concourse.bass_utils import run_bass_kernel_spmd  # noqa: E402

F32 = mybir.dt.float32
F32R = mybir.dt.float32r
BF16 = mybir.dt.bfloat16
I32 = mybir.dt.int32
AF = mybir.ActivationFunctionType
ALU = mybir.AluOpType
AXL = mybir.AxisListType

S, B, E, H = 1024, 64, 1024, 16
NCORES = 8
BL = B // NCORES  # 8 batches per core
MAX_SPAN, MIN_SPAN = 1024, 8
LAST_RESULT = None


def _build(S_active, hs_mean, regp):
    R = BL * S_active
    NQ = R // 128
    RB = min(R, 512)
    NRB = R // RB
    s_lo = S - S_active
    ZROWS = s_lo * BL

    nc = bacc.Bacc("TRN2", target_bir_lowering=False, debug=False,
                   num_devices=NCORES)

    def inp(name, shape, dt=F32):
        return nc.declare_dram_parameter(name, shape, dt, isOutput=False)

    lx = inp("lx", [BL, S_active, E])
    encb = inp("encb", [256, BL, E], BF16)
    w1x = inp("w1x", [E, E], F32R)          # rs_w1 top half (f32 bytes)
    w1c = inp("w1c", [E, E], BF16)          # rs_w1 bottom half, bf16
    rs_w2 = inp("rs_w2", [E, E // 2], F32R)
    rs_w3 = inp("rs_w3", [E // 2, 1], F32R)
    rs_b1 = inp("rs_b1", [E])
    rs_b2 = inp("rs_b2", [E // 2])
    rs_b3 = inp("rs_b3", [1])
    sw1 = inp("sw1", [E, E // 2], BF16)
    sw2 = inp("sw2", [E // 2, E // 4], BF16)
    sw3 = inp("sw3", [E // 4, 1], BF16)
    sp_b1 = inp("sp_b1", [E // 2])
    sp_b2 = inp("sp_b2", [E // 4])
    sp_b3 = inp("sp_b3", [1])
    ident = inp("ident", [128, 128])
    patt8 = inp("patt8", [128, BL], BF16)
    onehot = inp("onehot", [BL, R], F32R)
    bdm = inp("bdm", [R, BL])
    decay_r = inp("decay_r", [R])
    cthr_r = inp("cthr_r", [R])
    c2_r = inp("c2_r", [R])

    attn_out = nc.declare_dram_parameter("attn_out", [S, BL], F32, isOutput=True)
    feats_out = nc.declare_dram_parameter("feats_out", [BL, E], F32, isOutput=True)
    spans_out = nc.declare_dram_parameter("spans_out", [BL], I32, isOutput=True)

    enc_flat = encb.rearrange("s b e -> (s b) e")   # [2048, E]
    lx_flat = lx.rearrange("b s e -> (b s) e")      # [R, E]

    with tile.TileContext(nc) as tc, \
            tc.tile_pool(name="wts", bufs=1) as wp, \
            tc.tile_pool(name="per", bufs=1) as pp, \
            tc.tile_pool(name="acts", bufs=1) as ap, \
            tc.tile_pool(name="encs", bufs=16) as ep, \
            tc.tile_pool(name="cyc", bufs=3) as cy, \
            tc.tile_pool(name="pacc1", bufs=2, space="PSUM") as pacc1, \
            tc.tile_pool(name="pacc2", bufs=4, space="PSUM") as pacc2, \
            tc.tile_pool(name="ptr", bufs=1, space="PSUM") as ptr, \
            tc.tile_pool(name="psml", bufs=1, space="PSUM") as psml:

        # ---------------- DMAs (priority order) --------------------------
        ident_t = pp.tile([128, 128], F32, tag="ident")
        nc.sync.dma_start(ident_t[:], ident[:])
        patt8_t = pp.tile([128, BL], BF16, tag="patt8")
        nc.sync.dma_start(patt8_t[:], patt8[:])
        lx_t = [pp.tile([128, E], F32, tag=f"lx{q}", name=f"lx{q}")
                for q in range(NQ)]
        for q in range(NQ):
            nc.sync.dma_start(lx_t[q][:], lx_flat[128 * q:128 * (q + 1), :])
        rb1_t = pp.tile([128, 8], F32, tag="rb1")
        nc.sync.dma_start(rb1_t[:], rs_b1[:].rearrange("(m p) -> p m", p=128))
        rb2_t = pp.tile([128, 4], F32, tag="rb2")
        nc.sync.dma_start(rb2_t[:], rs_b2[:].rearrange("(m p) -> p m", p=128))
        rb3_t = pp.tile([1, 1], F32, tag="rb3")
        nc.sync.dma_start(rb3_t[:], rs_b3[:].rearrange("(p o) -> p o", p=1))
        sb1_t = pp.tile([128, 4], F32, tag="sb1")
        nc.sync.dma_start(sb1_t[:], sp_b1[:].rearrange("(m p) -> p m", p=128))
        sb2_t = pp.tile([128, 2], F32, tag="sb2")
        nc.sync.dma_start(sb2_t[:], sp_b2[:].rearrange("(m p) -> p m", p=128))
        sb3_t = pp.tile([1, 1], F32, tag="sb3")
        nc.sync.dma_start(sb3_t[:], sp_b3[:].rearrange("(p o) -> p o", p=1))

        enc_tiles = []
        for t in range(16):
            et = ep.tile([128, E], BF16, tag="enc", name="enc_t")
            nc.sync.dma_start(et[:], enc_flat[128 * t:128 * (t + 1), :])
            enc_tiles.append(et)

        sw1_t = [wp.tile([128, E // 2], BF16, tag=f"sw1_{k}", name=f"sw1_{k}")
                 for k in range(8)]
        for k in range(8):
            nc.sync.dma_start(sw1_t[k][:], sw1[128 * k:128 * (k + 1), :])
        sw2_t = [wp.tile([128, E // 4], BF16, tag=f"sw2_{k}", name=f"sw2_{k}")
                 for k in range(4)]
        for k in range(4):
            nc.sync.dma_start(sw2_t[k][:], sw2[128 * k:128 * (k + 1), :])
        sw3_t = wp.tile([128, 2], BF16, tag="sw3")
        nc.sync.dma_start(sw3_t[:], sw3[:].rearrange("(k p) o -> p (k o)", p=128))
        onehot_t = pp.tile([BL, R], F32R, tag="onehot")
        nc.sync.dma_start(onehot_t[:], onehot[:])

        wc_t = [wp.tile([128, E], BF16, tag=f"wc{m}", name=f"wc{m}")
                for m in range(8)]
        wm_t = [wp.tile([128, E], F32R, tag=f"wm{m}", name=f"wm{m}")
                for m in range(8)]
        w2_t = [wp.tile([128, E // 2], F32R, tag=f"w2_{k}", name=f"w2_{k}")
                for k in range(8)]
        for m in range(8):
            for k in range(8):
                nc.sync.dma_start(
                    wc_t[m][:, 128 * k:128 * (k + 1)],
                    w1c[128 * k:128 * (k + 1), 128 * m:128 * (m + 1)])
            for k in range(8):
                nc.sync.dma_start(
                    wm_t[m][:, 128 * k:128 * (k + 1)],
                    w1x[128 * k:128 * (k + 1), 128 * m:128 * (m + 1)])
            nc.sync.dma_start(w2_t[m][:], rs_w2[128 * m:128 * (m + 1), :])
        w3_t = wp.tile([128, 4], F32R, tag="w3")
        nc.sync.dma_start(w3_t[:], rs_w3[:].rearrange("(k p) o -> p (k o)", p=128))

        bdm_t = [pp.tile([128, BL], F32, tag=f"bdm{q}", name=f"bdm{q}")
                 for q in range(NQ)]
        for q in range(NQ):
            nc.sync.dma_start(bdm_t[q][:], bdm[128 * q:128 * (q + 1), :])
        dec_t = pp.tile([1, R], F32, tag="dec")
        nc.sync.dma_start(dec_t[:], decay_r[:].rearrange("(o r) -> o r", o=1))
        cthr_t = pp.tile([1, R], F32, tag="cthr")
        nc.sync.dma_start(cthr_t[:], cthr_r[:].rearrange("(o r) -> o r", o=1))
        c2_t = pp.tile([1, R], F32, tag="c2")
        nc.sync.dma_start(c2_t[:], c2_r[:].rearrange("(o r) -> o r", o=1))

        # attn zero rows: independent of everything -> emit early
        if ZROWS > 0:
            zcols = ZROWS // 128
            zt = pp.tile([128, zcols], F32, tag="zt")
            nc.vector.memset(zt[:], 0.0)
            nc.sync.dma_start(
                attn_out[0:S - S_active, :].rearrange("s b -> (s b)")
                .rearrange("(p f) -> p f", p=128), zt[:])

        # ---------------- PE: A0 transposes (lx^T), fp32 -> f32r --------
        a0 = [ap.tile([128, R], F32R, tag=f"a0_{c}", name=f"a0_{c}")
              for c in range(8)]
        for c in range(8):
            for q in range(NQ):
                tps = ptr.tile([128, 128], F32, tag="ptr", name="tps")
                nc.tensor.matmul(tps[:], lx_t[q][:, 128 * c:128 * (c + 1)],
                                 ident_t[:, :], is_transpose=True,
                                 start=True, stop=True)
                nc.scalar.copy(a0[c][:, 128 * q:128 * (q + 1)], tps[:])

        # ---------------- ctx = mean_s(enc) on PE (bf16) ----------------
        ctx_t = pp.tile([BL, E], F32, tag="ctx")
        cps = [pacc1.tile([BL, 512], F32, tag="pacc1", name="cps")
               for _ in range(2)]
        for t in range(16):
            for h in range(2):
                nc.tensor.matmul(cps[h][:], patt8_t[:],
                                 enc_tiles[t][:, 512 * h:512 * (h + 1)],
                                 start=(t == 0), stop=(t == 15))
        for h in range(2):
            nc.scalar.activation(ctx_t[:, 512 * h:512 * (h + 1)], cps[h][:],
                                 AF.Copy, scale=1.0 / 256.0)

        # ---------------- ctxT (bf16) ------------------------------------
        ctxTb = pp.tile([128, 8 * BL], BF16, tag="ctxTb")
        for c in range(8):
            tps2 = ptr.tile([128, BL], F32, tag="ptr", name="tps2")
            nc.tensor.matmul(tps2[:], ctx_t[0:BL, 128 * c:128 * (c + 1)],
                             ident_t[0:BL, 0:BL], is_transpose=True,
                             start=True, stop=True)
            nc.scalar.copy(ctxTb[:, BL * c:BL * (c + 1)], tps2[:])

        # ---------------- span predictor MLP (bf16), early ---------------
        h1b = pp.tile([128, 4 * BL], BF16, tag="h1b")
        for m in range(4):
            hps = ptr.tile([128, BL], F32, tag="ptr", name="hps")
            for k in range(8):
                nc.tensor.matmul(hps[:], sw1_t[k][:, 128 * m:128 * (m + 1)],
                                 ctxTb[:, BL * k:BL * (k + 1)],
                                 start=(k == 0), stop=(k == 7))
            nc.scalar.activation(h1b[:, BL * m:BL * (m + 1)], hps[:], AF.Gelu,
                                 bias=sb1_t[:, m:m + 1])
        h2b = pp.tile([128, 2 * BL], BF16, tag="h2b")
        for m in range(2):
            hps2 = ptr.tile([128, BL], F32, tag="ptr", name="hps2")
            for k in range(4):
                nc.tensor.matmul(hps2[:], sw2_t[k][:, 128 * m:128 * (m + 1)],
                                 h1b[:, BL * k:BL * (k + 1)],
                                 start=(k == 0), stop=(k == 3))
            nc.scalar.activation(h2b[:, BL * m:BL * (m + 1)], hps2[:], AF.Gelu,
                                 bias=sb2_t[:, m:m + 1])
        ups = psml.tile([1, BL], F32, tag="psml", name="ups")
        for k in range(2):
            nc.tensor.matmul(ups[:], sw3_t[:, k:k + 1],
                             h2b[:, BL * k:BL * (k + 1)],
                             start=(k == 0), stop=(k == 1))
        sl_t = pp.tile([1, BL], F32, tag="sl")
        nc.scalar.activation(sl_t[:], ups[:], AF.Sigmoid, bias=sb3_t[0:1, 0:1])
        tspan = pp.tile([1, BL], F32, tag="tspan")
        nc.vector.tensor_scalar(tspan[:], sl_t[:], float(MAX_SPAN - MIN_SPAN),
                                float(MIN_SPAN), ALU.mult, ALU.add)
        tsh = pp.tile([1, BL], F32, tag="tsh")
        nc.vector.tensor_scalar(tsh[:], tspan[:], 0.5, None, ALU.subtract)
        spans_i = pp.tile([1, BL], I32, tag="spans_i")
        nc.vector.tensor_copy(spans_i[:], tsh[:])
        span_f = pp.tile([1, BL], F32, tag="span_f")
        nc.vector.tensor_copy(span_f[:], spans_i[:])
        nc.sync.dma_start(spans_out[:].rearrange("(o b) -> o b", o=1), spans_i[:])

        # ---------------- c1 = ctx @ rs_w1[E:]  (bf16 -> f32r) ----------
        c1n_t = pp.tile([BL, E], F32R, tag="c1n")
        for m in range(8):
            c1ps = ptr.tile([BL, 128], F32, tag="ptr", name="c1ps")
            for k in range(8):
                nc.tensor.matmul(c1ps[:], ctxTb[:, BL * k:BL * (k + 1)],
                                 wc_t[m][:, 128 * k:128 * (k + 1)],
                                 start=(k == 0), stop=(k == 7))
            nc.scalar.copy(c1n_t[:, 128 * m:128 * (m + 1)], c1ps[:])

        # ---------------- main MLP: per-m pipelined L1 -> L2 (f32r) -----
        zb_t = pp.tile([1, R], F32, tag="zb")
        rel_t = pp.tile([1, R], F32, tag="rel")
        for rb in range(NRB):
            r0 = rb * RB
            a1 = [ap.tile([128, RB], F32R, tag=f"a1_{m}", name=f"a1_{m}")
                  for m in range(8)]
            ps2 = [pacc2.tile([128, RB], F32, tag="pacc2", name="ps2")
                   for _ in range(4)]

            def emit_l2(mm):
                for m2 in range(4):
                    nc.tensor.matmul(ps2[m2][:],
                                     w2_t[mm][:, 128 * m2:128 * (m2 + 1)],
                                     a1[mm][:], start=(mm == 0), stop=(mm == 7))

            for m in range(8):
                ps1 = pacc1.tile([128, RB], F32, tag="pacc1", name="ps1")
                for k in range(8):
                    nc.tensor.matmul(ps1[:], wm_t[m][:, 128 * k:128 * (k + 1)],
                                     a0[k][:, r0:r0 + RB],
                                     start=(k == 0), stop=False)
                nc.tensor.matmul(ps1[:], c1n_t[0:BL, 128 * m:128 * (m + 1)],
                                 onehot_t[0:BL, r0:r0 + RB],
                                 start=False, stop=True)
                nc.scalar.activation(a1[m][:], ps1[:], AF.Gelu,
                                     bias=rb1_t[:, m:m + 1])
                if m >= 1:
                    emit_l2(m - 1)
            emit_l2(7)

            a2 = [ap.tile([128, RB], F32R, tag=f"a2_{m2}", name=f"a2_{m2}")
                  for m2 in range(4)]
            for m2 in range(4):
                nc.scalar.activation(a2[m2][:], ps2[m2][:], AF.Gelu,
                                     bias=rb2_t[:, m2:m2 + 1])
            zps = psml.tile([1, RB], F32, tag="psml", name="zps")
            for m2 in range(4):
                nc.tensor.matmul(zps[:], w3_t[:, m2:m2 + 1], a2[m2][:],
                                 start=(m2 == 0), stop=(m2 == 3))
            nc.vector.tensor_scalar(zb_t[:, r0:r0 + RB], zps[:],
                                    rb3_t[0:1, 0:1], None, ALU.add)
            nc.scalar.activation(rel_t[:, r0:r0 + RB], zps[:], AF.Sigmoid,
                                 bias=rb3_t[0:1, 0:1])

        # ---------------- scores (DVE, fp32) ----------------------------
        mask_t = pp.tile([1, R], F32, tag="mask")
        nc.vector.tensor_tensor(mask_t[:], zb_t[:], cthr_t[:], ALU.is_gt)
        valid_t = pp.tile([1, R], F32, tag="valid")
        for b in range(BL):
            sl_ = slice(S_active * b, S_active * (b + 1))
            nc.vector.tensor_scalar(valid_t[:, sl_], c2_t[:, sl_],
                                    span_f[0:1, b:b + 1], None, ALU.is_lt)
        fs_t = pp.tile([1, R], F32, tag="fs")
        nc.vector.tensor_tensor(fs_t[:], rel_t[:], dec_t[:], ALU.mult)
        nc.vector.tensor_tensor(fs_t[:], fs_t[:], mask_t[:], ALU.mult)
        nc.vector.tensor_scalar(fs_t[:], fs_t[:], float(hs_mean), None, ALU.mult)
        nc.vector.tensor_tensor(fs_t[:], fs_t[:], valid_t[:], ALU.mult)
        total_t = pp.tile([1, BL], F32, tag="total")
        nc.vector.tensor_reduce(total_t[:],
                                fs_t[:].rearrange("p (b s) -> p b s", b=BL),
                                AXL.X, ALU.add)
        den_t = pp.tile([1, BL], F32, tag="den")
        nc.vector.tensor_scalar(den_t[:], total_t[:], float(regp), None, ALU.add)
        recip_t = pp.tile([1, BL], F32, tag="recip")
        nc.vector.reciprocal(recip_t[:], den_t[:])
        fsn_t = pp.tile([1, R], F32, tag="fsn")
        for b in range(BL):
            sl_ = slice(S_active * b, S_active * (b + 1))
            nc.vector.tensor_scalar(fsn_t[:, sl_], fs_t[:, sl_],
                                    recip_t[0:1, b:b + 1], None, ALU.mult)

        # ---------------- attn active rows -------------------------------
        for b in range(BL):
            nc.sync.dma_start(
                attn_out[S - S_active:S, b:b + 1].rearrange("s x -> (s x)")
                .rearrange("(o f) -> o f", o=1),
                fsn_t[:, S_active * b:S_active * (b + 1)])

        # ---------------- feats (block-diag packed fp32 matmul) ---------
        psf = [pacc2.tile([BL, 512], F32, tag="pacc2", name="psf")
               for _ in range(2)]
        for q in range(NQ):
            fps = ptr.tile([128, 1], F32, tag="ptr", name="fps")
            nc.tensor.matmul(fps[:], fsn_t[:, 128 * q:128 * (q + 1)],
                             ident_t[0:1, 0:1], is_transpose=True,
                             start=True, stop=True)
            fsnT = cy.tile([128, 1], F32, tag="fsnT", name="fsnT")
            nc.scalar.copy(fsnT[:], fps[:])
            bd = cy.tile([128, BL], F32, tag="bd", name="bd")
            nc.vector.tensor_scalar(bd[:], bdm_t[q][:], fsnT[:], None, ALU.mult)
            for h in range(2):
                nc.tensor.matmul(psf[h][:], bd[:],
                                 lx_t[q][:, 512 * h:512 * (h + 1)],
                                 start=(q == 0), stop=(q == NQ - 1))
        feats_t = pp.tile([BL, E], F32, tag="feats")
        for h in range(2):
            nc.scalar.copy(feats_t[:, 512 * h:512 * (h + 1)], psf[h][:])
        nc.sync.dma_start(feats_out[:], feats_t[:])

    nc.compile()
    return nc


def _host_consts(S_active, td, thr):
    """Host-side constant tensors for one core (b-major rows)."""
    R = BL * S_active
    s_lo = S - S_active
    s_idx = np.arange(S)
    decay_full = (np.float64(td) ** (S - 1 - s_idx)).astype(np.float32)
    s_of_r = s_lo + (np.arange(R) % S_active)
    dec_r = decay_full[s_of_r]
    q = np.float64(thr) / dec_r.astype(np.float64)
    with np.errstate(divide="ignore", invalid="ignore"):
        logit = np.log(q / (1.0 - q))
    cthr = np.where(q >= 1.0, 1e30, np.where(q <= 0.0, -1e30, logit))
    cthr = cthr.astype(np.float32)
    c2 = (S - 1 - s_of_r).astype(np.float32)
    b_of_r = np.arange(R) // S_active
    onehot = (b_of_r[None, :] == np.arange(BL)[:, None]).astype(np.float32)
    bdm = onehot.T.copy()
    ident = np.eye(128, dtype=np.float32)
    patt8 = (np.arange(128)[:, None] % BL == np.arange(BL)[None, :]).astype(
        np.float32)
    return dict(ident=ident, patt8=patt8, onehot=onehot, bdm=bdm,
                decay_r=dec_r, cthr_r=cthr, c2_r=c2)


def _uniform_fallback(npv, spans):
    """Exact outputs when no position can pass the threshold (total==0
    everywhere): attn = valid/span, feats accordingly."""
    lx = npv["long_x"].astype(np.float32)
    start = (S - spans).astype(np.int32)
    pos = np.arange(S)
    valid = (pos[:, None] >= start[None, :]).astype(np.float32)
    attn = valid / spans[None, :].astype(np.float32)
    feats = np.einsum("sb,sbe->be", attn, lx).astype(np.float32)
    return attn, feats, spans


def _numpy_full_reference(npv):
    """Exact float32 numpy replica of the reference (slow, safety net for
    near-dense threshold patterns that the sparse device kernel doesn't
    cover)."""
    from scipy.special import erf
    f32 = np.float32
    lx = npv["long_x"].astype(f32)
    ctx = npv["encoded_x"].astype(f32).mean(axis=0, dtype=f32)

    def gelu(x):
        return (0.5 * x * (1 + erf(x / np.sqrt(2)))).astype(f32)

    spans = _numpy_spans(npv)
    start = S - spans
    pos = np.arange(S)
    valid = (pos[:, None] >= start[None, :]).astype(f32)
    comb = np.concatenate([lx, np.broadcast_to(ctx[None], lx.shape)], axis=-1)
    r = gelu(comb.reshape(-1, 2 * E) @ npv["rs_w1"].astype(f32) +
             npv["rs_b1"].astype(f32))
    r = gelu(r @ npv["rs_w2"].astype(f32) + npv["rs_b2"].astype(f32))
    rel = 1 / (1 + np.exp(-(r @ npv["rs_w3"].astype(f32) +
                            npv["rs_b3"].astype(f32))))
    rel = rel[:, 0].reshape(S, B).astype(f32)
    td = f32(npv["temporal_decay"])
    decay = (np.float64(td) ** (S - 1 - pos)).astype(f32)
    fs = rel * decay[:, None]
    fs = fs * (fs > f32(npv["adaptive_threshold"]))
    fs = fs * f32(np.asarray(npv["head_scale"], np.float64).mean())
    fs = fs * valid
    total = fs.sum(axis=0, dtype=f32)
    normed = fs / (total[None, :] + f32(npv["attention_reg"]) + f32(1e-8))
    uniform = valid / spans[None, :].astype(f32)
    attn = np.where(total[None, :] > 0, normed, uniform).astype(f32)
    feats = np.einsum("sb,sbe->be", attn, lx).astype(f32)
    return attn, feats, spans


def _numpy_spans(npv):
    from scipy.special import erf
    ctx = npv["encoded_x"].astype(np.float64).mean(axis=0)

    def gelu(x):
        return 0.5 * x * (1 + erf(x / np.sqrt(2)))

    h = gelu(ctx @ npv["sp_w1"].astype(np.float64) + npv["sp_b1"])
    h = gelu(h @ npv["sp_w2"].astype(np.float64) + npv["sp_b2"])
    sl = 1 / (1 + np.exp(-(h @ npv["sp_w3"].astype(np.float64) + npv["sp_b3"])))
    sl = sl[:, 0].astype(np.float32)
    spans = np.minimum((sl * (MAX_SPAN - MIN_SPAN) + MIN_SPAN).astype(np.int32),
                       S)
    return spans


def kernel(**inputs):
    npv = {k: np.asarray(v) for k, v in inputs.items()}
    f32 = np.float32
    td = float(f32(npv["temporal_decay"]))
    thr = float(f32(npv["adaptive_threshold"]))
    reg = float(f32(npv["attention_reg"]))
    hs_mean = float(f32(np.asarray(npv["head_scale"], np.float64).mean()))
    regp = float(f32(reg) + f32(1e-8))

    decay_full = (np.float64(td) ** (S - 1 - np.arange(S))).astype(np.float32)
    passing = decay_full > f32(thr)
    if not passing.any():
        return _uniform_fallback(npv, _numpy_spans(npv))
    s_min = int(np.nonzero(passing)[0].min())
    S_active = min(S, int(np.ceil((S - s_min + 8) / 32.0)) * 32)
    if S_active > 128:
        return _numpy_full_reference(npv)
    s_lo = S - S_active

    nc = _build(S_active, hs_mean, regp)
    consts = _host_consts(S_active, td, thr)

    import ml_dtypes
    bf16 = ml_dtypes.bfloat16

    def c(a, dt=np.float32):
        return np.ascontiguousarray(np.asarray(a, dtype=np.float32).astype(dt))

    rw1 = np.asarray(npv["rs_w1"], np.float32)
    shared = dict(
        w1x=c(rw1[:E]), w1c=c(rw1[E:], bf16),
        rs_w2=c(npv["rs_w2"]), rs_w3=c(npv["rs_w3"]),
        rs_b1=c(npv["rs_b1"]), rs_b2=c(npv["rs_b2"]), rs_b3=c(npv["rs_b3"]),
        sw1=c(npv["sp_w1"], bf16), sw2=c(npv["sp_w2"], bf16),
        sw3=c(npv["sp_w3"], bf16),
        sp_b1=c(npv["sp_b1"]), sp_b2=c(npv["sp_b2"]), sp_b3=c(npv["sp_b3"]),
        **{k: (c(v, bf16) if k == "patt8" else c(v)) for k, v in consts.items()})
    in_maps = []
    for i in range(NCORES):
        bs = slice(i * BL, (i + 1) * BL)
        in_maps.append(dict(
            lx=c(npv["long_x"][s_lo:, bs, :].transpose(1, 0, 2)),
            encb=c(npv["encoded_x"][:, bs, :], bf16),
            **shared))

    res = run_bass_kernel_spmd(nc, in_maps, core_ids=list(range(NCORES)))
    global LAST_RESULT
    LAST_RESULT = res
    rs = res.results
    attn = np.concatenate([r["attn_out"] for r in rs], axis=1)
    feats = np.concatenate([r["feats_out"] for r in rs], axis=0)
    spans = np.concatenate([r["spans_out"] for r in rs], axis=0).astype(np.int32)
    return attn, feats, spans


if __name__ == "__main__":
    rng = np.random.default_rng(0)
    print("smoke test requires reference inputs; use test.py")


# revision 12
# speedup vs baseline: 2.3291x; 1.5035x over previous
"""AdaptiveSpanAttention TRN2 kernel: 8-way batch-parallel Bass/Tile kernel.

Structure exploited: fs = sigmoid(z)*decay_s can only exceed adaptive_threshold
where decay_s > threshold (since sigmoid < 1). For the reference scalars
(decay 0.95, thr 0.3) that is only the last 24 of 1024 positions, so the big
relevance MLP runs only on that row suffix. The threshold mask is evaluated on
the pre-sigmoid logit z against host-precomputed c_s = logit(thr/decay_s),
which is exactly monotone-equivalent and immune to LUT rounding at the
discontinuity. All matmuls run in true fp32 on the PE (4 cyc/row).

Sharding: batch dim (axis 1 of long_x/encoded_x) across the 8 NeuronCores,
weights replicated, outputs concatenated on host (pure data parallel).
"""
import sys

import numpy as np

sys.path.insert(0, "/opt/trn_rl_repo")

import concourse.tile as tile  # noqa: E402
from concourse import bacc, mybir  # noqa: E402
from concourse.bass_utils import run_bass_kernel_spmd  # noqa: E402

F32 = mybir.dt.float32
F32R = mybir.dt.float32r
BF16 = mybir.dt.bfloat16
I32 = mybir.dt.int32
AF = mybir.ActivationFunctionType
ALU = mybir.AluOpType
AXL = mybir.AxisListType

S, B, E, H = 1024, 64, 1024, 16
NCORES = 8
BL = B // NCORES  # 8 batches per core
MAX_SPAN, MIN_SPAN = 1024, 8
LAST_RESULT = None


def _build(S_active, hs_mean, regp):
    R = BL * S_active
    NQ = R // 128
    RB = min(R, 512)
    NRB = R // RB
    s_lo = S - S_active
    ZROWS = s_lo * BL

    nc = bacc.Bacc("TRN2", target_bir_lowering=False, debug=False,
                   num_devices=NCORES)

    def inp(name, shape, dt=F32):
        return nc.declare_dram_parameter(name, shape, dt, isOutput=False)

    lx = inp("lx", [BL, S_active, E])
    encb = inp("encb", [256, BL, E], BF16)
    w1x = inp("w1x", [E, E], F32R)          # rs_w1 top half (f32 bytes)
    w1c = inp("w1c", [E, E], BF16)          # rs_w1 bottom half, bf16
    rs_w2 = inp("rs_w2", [E, E // 2], F32R)
    rs_w3 = inp("rs_w3", [E // 2, 1], F32R)
    rs_b1 = inp("rs_b1", [E])
    rs_b2 = inp("rs_b2", [E // 2])
    rs_b3 = inp("rs_b3", [1])
    sw1 = inp("sw1", [E, E // 2], BF16)
    sw2 = inp("sw2", [E // 2, E // 4], BF16)
    sw3 = inp("sw3", [E // 4, 1], BF16)
    sp_b1 = inp("sp_b1", [E // 2])
    sp_b2 = inp("sp_b2", [E // 4])
    sp_b3 = inp("sp_b3", [1])
    ident = inp("ident", [128, 128])
    patt8 = inp("patt8", [128, BL])
    onehot = inp("onehot", [BL, R], F32R)
    bdm = inp("bdm", [R, BL])
    decay_r = inp("decay_r", [R])
    cthr_r = inp("cthr_r", [R])
    c2_r = inp("c2_r", [R])

    attn_out = nc.declare_dram_parameter("attn_out", [S, BL], F32, isOutput=True)
    feats_out = nc.declare_dram_parameter("feats_out", [BL, E], F32, isOutput=True)
    spans_out = nc.declare_dram_parameter("spans_out", [BL], I32, isOutput=True)

    enc_flat = encb.rearrange("s b e -> (s b) e")   # [2048, E]
    lx_flat = lx.rearrange("b s e -> (b s) e")      # [R, E]

    with tile.TileContext(nc) as tc, \
            tc.tile_pool(name="wts", bufs=1) as wp, \
            tc.tile_pool(name="per", bufs=1) as pp, \
            tc.tile_pool(name="acts", bufs=1) as ap, \
            tc.tile_pool(name="encs", bufs=16) as ep, \
            tc.tile_pool(name="cyc", bufs=3) as cy, \
            tc.tile_pool(name="pacc1", bufs=2, space="PSUM") as pacc1, \
            tc.tile_pool(name="pacc2", bufs=4, space="PSUM") as pacc2, \
            tc.tile_pool(name="ptr", bufs=1, space="PSUM") as ptr, \
            tc.tile_pool(name="psml", bufs=1, space="PSUM") as psml:

        # ---------------- DMAs (priority order) --------------------------
        ident_t = pp.tile([128, 128], F32, tag="ident")
        nc.sync.dma_start(ident_t[:], ident[:])
        patt8_t = pp.tile([128, BL], F32, tag="patt8")
        nc.sync.dma_start(patt8_t[:], patt8[:])
        lx_t = [pp.tile([128, E], F32, tag=f"lx{q}", name=f"lx{q}")
                for q in range(NQ)]
        for q in range(NQ):
            nc.sync.dma_start(lx_t[q][:], lx_flat[128 * q:128 * (q + 1), :])
        rb1_t = pp.tile([128, 8], F32, tag="rb1")
        nc.sync.dma_start(rb1_t[:], rs_b1[:].rearrange("(m p) -> p m", p=128))
        rb2_t = pp.tile([128, 4], F32, tag="rb2")
        nc.sync.dma_start(rb2_t[:], rs_b2[:].rearrange("(m p) -> p m", p=128))
        rb3_t = pp.tile([1, 1], F32, tag="rb3")
        nc.sync.dma_start(rb3_t[:], rs_b3[:].rearrange("(p o) -> p o", p=1))
        sb1_t = pp.tile([128, 4], F32, tag="sb1")
        nc.sync.dma_start(sb1_t[:], sp_b1[:].rearrange("(m p) -> p m", p=128))
        sb2_t = pp.tile([128, 2], F32, tag="sb2")
        nc.sync.dma_start(sb2_t[:], sp_b2[:].rearrange("(m p) -> p m", p=128))
        sb3_t = pp.tile([1, 1], F32, tag="sb3")
        nc.sync.dma_start(sb3_t[:], sp_b3[:].rearrange("(p o) -> p o", p=1))

        enc_tiles = []
        for t in range(16):
            et = ep.tile([128, E], BF16, tag="enc", name="enc_t")
            nc.sync.dma_start(et[:], enc_flat[128 * t:128 * (t + 1), :])
            enc_tiles.append(et)

        sw1_t = [wp.tile([128, E // 2], BF16, tag=f"sw1_{k}", name=f"sw1_{k}")
                 for k in range(8)]
        for k in range(8):
            nc.sync.dma_start(sw1_t[k][:], sw1[128 * k:128 * (k + 1), :])
        sw2_t = [wp.tile([128, E // 4], BF16, tag=f"sw2_{k}", name=f"sw2_{k}")
                 for k in range(4)]
        for k in range(4):
            nc.sync.dma_start(sw2_t[k][:], sw2[128 * k:128 * (k + 1), :])
        sw3_t = wp.tile([128, 2], BF16, tag="sw3")
        nc.sync.dma_start(sw3_t[:], sw3[:].rearrange("(k p) o -> p (k o)", p=128))
        onehot_t = pp.tile([BL, R], F32R, tag="onehot")
        nc.sync.dma_start(onehot_t[:], onehot[:])

        wc_t = [wp.tile([128, E], BF16, tag=f"wc{k}", name=f"wc{k}")
                for k in range(8)]
        wm_t = [wp.tile([128, E], F32R, tag=f"wm{k}", name=f"wm{k}")
                for k in range(8)]
        w2_t = [wp.tile([128, E // 2], F32R, tag=f"w2_{k}", name=f"w2_{k}")
                for k in range(8)]
        for k in range(8):
            nc.sync.dma_start(wc_t[k][:], w1c[128 * k:128 * (k + 1), :])
        for k in range(8):
            nc.sync.dma_start(wm_t[k][:], w1x[128 * k:128 * (k + 1), :])
        for k in range(8):
            nc.sync.dma_start(w2_t[k][:], rs_w2[128 * k:128 * (k + 1), :])
        w3_t = wp.tile([128, 4], F32R, tag="w3")
        nc.sync.dma_start(w3_t[:], rs_w3[:].rearrange("(k p) o -> p (k o)", p=128))

        bdm_t = [pp.tile([128, BL], F32, tag=f"bdm{q}", name=f"bdm{q}")
                 for q in range(NQ)]
        for q in range(NQ):
            nc.sync.dma_start(bdm_t[q][:], bdm[128 * q:128 * (q + 1), :])
        dec_t = pp.tile([1, R], F32, tag="dec")
        nc.sync.dma_start(dec_t[:], decay_r[:].rearrange("(o r) -> o r", o=1))
        cthr_t = pp.tile([1, R], F32, tag="cthr")
        nc.sync.dma_start(cthr_t[:], cthr_r[:].rearrange("(o r) -> o r", o=1))
        c2_t = pp.tile([1, R], F32, tag="c2")
        nc.sync.dma_start(c2_t[:], c2_r[:].rearrange("(o r) -> o r", o=1))

        # attn zero rows: independent of everything -> emit early
        if ZROWS > 0:
            zcols = ZROWS // 128
            zt = pp.tile([128, zcols], F32, tag="zt")
            nc.vector.memset(zt[:], 0.0)
            nc.sync.dma_start(
                attn_out[0:S - S_active, :].rearrange("s b -> (s b)")
                .rearrange("(p f) -> p f", p=128), zt[:])

        # ---------------- PE: A0 transposes (lx^T), fp32 -> f32r --------
        a0 = [ap.tile([128, R], F32R, tag=f"a0_{c}", name=f"a0_{c}")
              for c in range(8)]
        for c in range(8):
            for q in range(NQ):
                tps = ptr.tile([128, 128], F32, tag="ptr", name="tps")
                nc.tensor.matmul(tps[:], lx_t[q][:, 128 * c:128 * (c + 1)],
                                 ident_t[:, :], is_transpose=True,
                                 start=True, stop=True)
                nc.scalar.copy(a0[c][:, 128 * q:128 * (q + 1)], tps[:])

        # ---------------- ctx = mean_s(enc): DVE accumulate + PE fold ----
        acc = pp.tile([128, E], F32, tag="acc")
        nc.vector.tensor_tensor(acc[:], enc_tiles[0][:], enc_tiles[1][:],
                                ALU.add)
        for t in range(2, 16):
            nc.vector.tensor_tensor(acc[:], acc[:], enc_tiles[t][:], ALU.add)
        # fold 128 partitions (16 (s,b) groups of 8) -> [BL, E] via patt8 matmul
        ctx_t = pp.tile([BL, E], F32, tag="ctx")
        cps = [pacc1.tile([BL, 512], F32, tag="pacc1", name="cps")
               for _ in range(2)]
        for h in range(2):
            nc.tensor.matmul(cps[h][:], patt8_t[:],
                             acc[:, 512 * h:512 * (h + 1)],
                             start=True, stop=True)
            nc.scalar.activation(ctx_t[:, 512 * h:512 * (h + 1)], cps[h][:],
                                 AF.Copy, scale=1.0 / 256.0)

        # ---------------- ctxT (bf16) ------------------------------------
        ctxTb = pp.tile([128, 8 * BL], BF16, tag="ctxTb")
        for c in range(8):
            tps2 = ptr.tile([128, BL], F32, tag="ptr", name="tps2")
            nc.tensor.matmul(tps2[:], ctx_t[0:BL, 128 * c:128 * (c + 1)],
                             ident_t[0:BL, 0:BL], is_transpose=True,
                             start=True, stop=True)
            nc.scalar.copy(ctxTb[:, BL * c:BL * (c + 1)], tps2[:])

        # ---------------- span predictor MLP (bf16), early ---------------
        h1b = pp.tile([128, 4 * BL], BF16, tag="h1b")
        for m in range(4):
            hps = ptr.tile([128, BL], F32, tag="ptr", name="hps")
            for k in range(8):
                nc.tensor.matmul(hps[:], sw1_t[k][:, 128 * m:128 * (m + 1)],
                                 ctxTb[:, BL * k:BL * (k + 1)],
                                 start=(k == 0), stop=(k == 7))
            nc.scalar.activation(h1b[:, BL * m:BL * (m + 1)], hps[:], AF.Gelu,
                                 bias=sb1_t[:, m:m + 1])
        h2b = pp.tile([128, 2 * BL], BF16, tag="h2b")
        for m in range(2):
            hps2 = ptr.tile([128, BL], F32, tag="ptr", name="hps2")
            for k in range(4):
                nc.tensor.matmul(hps2[:], sw2_t[k][:, 128 * m:128 * (m + 1)],
                                 h1b[:, BL * k:BL * (k + 1)],
                                 start=(k == 0), stop=(k == 3))
            nc.scalar.activation(h2b[:, BL * m:BL * (m + 1)], hps2[:], AF.Gelu,
                                 bias=sb2_t[:, m:m + 1])
        ups = psml.tile([1, BL], F32, tag="psml", name="ups")
        for k in range(2):
            nc.tensor.matmul(ups[:], sw3_t[:, k:k + 1],
                             h2b[:, BL * k:BL * (k + 1)],
                             start=(k == 0), stop=(k == 1))
        sl_t = pp.tile([1, BL], F32, tag="sl")
        nc.scalar.activation(sl_t[:], ups[:], AF.Sigmoid, bias=sb3_t[0:1, 0:1])
        tspan = pp.tile([1, BL], F32, tag="tspan")
        nc.vector.tensor_scalar(tspan[:], sl_t[:], float(MAX_SPAN - MIN_SPAN),
                                float(MIN_SPAN), ALU.mult, ALU.add)
        tsh = pp.tile([1, BL], F32, tag="tsh")
        nc.vector.tensor_scalar(tsh[:], tspan[:], 0.5, None, ALU.subtract)
        spans_i = pp.tile([1, BL], I32, tag="spans_i")
        nc.vector.tensor_copy(spans_i[:], tsh[:])
        span_f = pp.tile([1, BL], F32, tag="span_f")
        nc.vector.tensor_copy(span_f[:], spans_i[:])
        nc.sync.dma_start(spans_out[:].rearrange("(o b) -> o b", o=1), spans_i[:])

        # ---------------- c1 = ctx @ rs_w1[E:]  (bf16 -> f32r) ----------
        c1n_t = pp.tile([BL, E], F32R, tag="c1n")
        for m in range(8):
            c1ps = ptr.tile([BL, 128], F32, tag="ptr", name="c1ps")
            for k in range(8):
                nc.tensor.matmul(c1ps[:], ctxTb[:, BL * k:BL * (k + 1)],
                                 wc_t[k][:, 128 * m:128 * (m + 1)],
                                 start=(k == 0), stop=(k == 7))
            nc.scalar.copy(c1n_t[:, 128 * m:128 * (m + 1)], c1ps[:])

        # ---------------- main MLP: per-m pipelined L1 -> L2 (f32r) -----
        zb_t = pp.tile([1, R], F32, tag="zb")
        rel_t = pp.tile([1, R], F32, tag="rel")
        for rb in range(NRB):
            r0 = rb * RB
            a1 = [ap.tile([128, RB], F32R, tag=f"a1_{m}", name=f"a1_{m}")
                  for m in range(8)]
            ps2 = [pacc2.tile([128, RB], F32, tag="pacc2", name="ps2")
                   for _ in range(4)]

            def emit_l2(mm):
                for m2 in range(4):
                    nc.tensor.matmul(ps2[m2][:],
                                     w2_t[mm][:, 128 * m2:128 * (m2 + 1)],
                                     a1[mm][:], start=(mm == 0), stop=(mm == 7))

            for m in range(8):
                ps1 = pacc1.tile([128, RB], F32, tag="pacc1", name="ps1")
                for k in range(8):
                    nc.tensor.matmul(ps1[:], wm_t[k][:, 128 * m:128 * (m + 1)],
                                     a0[k][:, r0:r0 + RB],
                                     start=(k == 0), stop=False)
                nc.tensor.matmul(ps1[:], c1n_t[0:BL, 128 * m:128 * (m + 1)],
                                 onehot_t[0:BL, r0:r0 + RB],
                                 start=False, stop=True)
                nc.scalar.activation(a1[m][:], ps1[:], AF.Gelu,
                                     bias=rb1_t[:, m:m + 1])
                if m >= 1:
                    emit_l2(m - 1)
            emit_l2(7)

            a2 = [ap.tile([128, RB], F32R, tag=f"a2_{m2}", name=f"a2_{m2}")
                  for m2 in range(4)]
            for m2 in range(4):
                nc.scalar.activation(a2[m2][:], ps2[m2][:], AF.Gelu,
                                     bias=rb2_t[:, m2:m2 + 1])
            zps = psml.tile([1, RB], F32, tag="psml", name="zps")
            for m2 in range(4):
                nc.tensor.matmul(zps[:], w3_t[:, m2:m2 + 1], a2[m2][:],
                                 start=(m2 == 0), stop=(m2 == 3))
            nc.vector.tensor_scalar(zb_t[:, r0:r0 + RB], zps[:],
                                    rb3_t[0:1, 0:1], None, ALU.add)
            nc.scalar.activation(rel_t[:, r0:r0 + RB], zps[:], AF.Sigmoid,
                                 bias=rb3_t[0:1, 0:1])

        # ---------------- scores (DVE, fp32) ----------------------------
        mask_t = pp.tile([1, R], F32, tag="mask")
        nc.vector.tensor_tensor(mask_t[:], zb_t[:], cthr_t[:], ALU.is_gt)
        valid_t = pp.tile([1, R], F32, tag="valid")
        for b in range(BL):
            sl_ = slice(S_active * b, S_active * (b + 1))
            nc.vector.tensor_scalar(valid_t[:, sl_], c2_t[:, sl_],
                                    span_f[0:1, b:b + 1], None, ALU.is_lt)
        fs_t = pp.tile([1, R], F32, tag="fs")
        nc.vector.tensor_tensor(fs_t[:], rel_t[:], dec_t[:], ALU.mult)
        nc.vector.tensor_tensor(fs_t[:], fs_t[:], mask_t[:], ALU.mult)
        nc.vector.tensor_scalar(fs_t[:], fs_t[:], float(hs_mean), None, ALU.mult)
        nc.vector.tensor_tensor(fs_t[:], fs_t[:], valid_t[:], ALU.mult)
        total_t = pp.tile([1, BL], F32, tag="total")
        nc.vector.tensor_reduce(total_t[:],
                                fs_t[:].rearrange("p (b s) -> p b s", b=BL),
                                AXL.X, ALU.add)
        den_t = pp.tile([1, BL], F32, tag="den")
        nc.vector.tensor_scalar(den_t[:], total_t[:], float(regp), None, ALU.add)
        recip_t = pp.tile([1, BL], F32, tag="recip")
        nc.vector.reciprocal(recip_t[:], den_t[:])
        fsn_t = pp.tile([1, R], F32, tag="fsn")
        for b in range(BL):
            sl_ = slice(S_active * b, S_active * (b + 1))
            nc.vector.tensor_scalar(fsn_t[:, sl_], fs_t[:, sl_],
                                    recip_t[0:1, b:b + 1], None, ALU.mult)

        # ---------------- attn active rows -------------------------------
        for b in range(BL):
            nc.sync.dma_start(
                attn_out[S - S_active:S, b:b + 1].rearrange("s x -> (s x)")
                .rearrange("(o f) -> o f", o=1),
                fsn_t[:, S_active * b:S_active * (b + 1)])

        # ---------------- feats (block-diag packed fp32 matmul) ---------
        psf = [pacc2.tile([BL, 512], F32, tag="pacc2", name="psf")
               for _ in range(2)]
        for q in range(NQ):
            fps = ptr.tile([128, 1], F32, tag="ptr", name="fps")
            nc.tensor.matmul(fps[:], fsn_t[:, 128 * q:128 * (q + 1)],
                             ident_t[0:1, 0:1], is_transpose=True,
                             start=True, stop=True)
            fsnT = cy.tile([128, 1], F32, tag="fsnT", name="fsnT")
            nc.scalar.copy(fsnT[:], fps[:])
            bd = cy.tile([128, BL], F32, tag="bd", name="bd")
            nc.vector.tensor_scalar(bd[:], bdm_t[q][:], fsnT[:], None, ALU.mult)
            for h in range(2):
                nc.tensor.matmul(psf[h][:], bd[:],
                                 lx_t[q][:, 512 * h:512 * (h + 1)],
                                 start=(q == 0), stop=(q == NQ - 1))
        feats_t = pp.tile([BL, E], F32, tag="feats")
        for h in range(2):
            nc.scalar.copy(feats_t[:, 512 * h:512 * (h + 1)], psf[h][:])
        nc.sync.dma_start(feats_out[:], feats_t[:])

    nc.compile()
    return nc


def _host_consts(S_active, td, thr):
    """Host-side constant tensors for one core (b-major rows)."""
    R = BL * S_active
    s_lo = S - S_active
    s_idx = np.arange(S)
    decay_full = (np.float64(td) ** (S - 1 - s_idx)).astype(np.float32)
    s_of_r = s_lo + (np.arange(R) % S_active)
    dec_r = decay_full[s_of_r]
    q = np.float64(thr) / dec_r.astype(np.float64)
    with np.errstate(divide="ignore", invalid="ignore"):
        logit = np.log(q / (1.0 - q))
    cthr = np.where(q >= 1.0, 1e30, np.where(q <= 0.0, -1e30, logit))
    cthr = cthr.astype(np.float32)
    c2 = (S - 1 - s_of_r).astype(np.float32)
    b_of_r = np.arange(R) // S_active
    onehot = (b_of_r[None, :] == np.arange(BL)[:, None]).astype(np.float32)
    bdm = onehot.T.copy()
    ident = np.eye(128, dtype=np.float32)
    patt8 = (np.arange(128)[:, None] % BL == np.arange(BL)[None, :]).astype(
        np.float32)
    return dict(ident=ident, patt8=patt8, onehot=onehot, bdm=bdm,
                decay_r=dec_r, cthr_r=cthr, c2_r=c2)


def _uniform_fallback(npv, spans):
    """Exact outputs when no position can pass the threshold (total==0
    everywhere): attn = valid/span, feats accordingly."""
    lx = npv["long_x"].astype(np.float32)
    start = (S - spans).astype(np.int32)
    pos = np.arange(S)
    valid = (pos[:, None] >= start[None, :]).astype(np.float32)
    attn = valid / spans[None, :].astype(np.float32)
    feats = np.einsum("sb,sbe->be", attn, lx).astype(np.float32)
    return attn, feats, spans


def _numpy_full_reference(npv):
    """Exact float32 numpy replica of the reference (slow, safety net for
    near-dense threshold patterns that the sparse device kernel doesn't
    cover)."""
    from scipy.special import erf
    f32 = np.float32
    lx = npv["long_x"].astype(f32)
    ctx = npv["encoded_x"].astype(f32).mean(axis=0, dtype=f32)

    def gelu(x):
        return (0.5 * x * (1 + erf(x / np.sqrt(2)))).astype(f32)

    spans = _numpy_spans(npv)
    start = S - spans
    pos = np.arange(S)
    valid = (pos[:, None] >= start[None, :]).astype(f32)
    comb = np.concatenate([lx, np.broadcast_to(ctx[None], lx.shape)], axis=-1)
    r = gelu(comb.reshape(-1, 2 * E) @ npv["rs_w1"].astype(f32) +
             npv["rs_b1"].astype(f32))
    r = gelu(r @ npv["rs_w2"].astype(f32) + npv["rs_b2"].astype(f32))
    rel = 1 / (1 + np.exp(-(r @ npv["rs_w3"].astype(f32) +
                            npv["rs_b3"].astype(f32))))
    rel = rel[:, 0].reshape(S, B).astype(f32)
    td = f32(npv["temporal_decay"])
    decay = (np.float64(td) ** (S - 1 - pos)).astype(f32)
    fs = rel * decay[:, None]
    fs = fs * (fs > f32(npv["adaptive_threshold"]))
    fs = fs * f32(np.asarray(npv["head_scale"], np.float64).mean())
    fs = fs * valid
    total = fs.sum(axis=0, dtype=f32)
    normed = fs / (total[None, :] + f32(npv["attention_reg"]) + f32(1e-8))
    uniform = valid / spans[None, :].astype(f32)
    attn = np.where(total[None, :] > 0, normed, uniform).astype(f32)
    feats = np.einsum("sb,sbe->be", attn, lx).astype(f32)
    return attn, feats, spans


def _numpy_spans(npv):
    from scipy.special import erf
    ctx = npv["encoded_x"].astype(np.float64).mean(axis=0)

    def gelu(x):
        return 0.5 * x * (1 + erf(x / np.sqrt(2)))

    h = gelu(ctx @ npv["sp_w1"].astype(np.float64) + npv["sp_b1"])
    h = gelu(h @ npv["sp_w2"].astype(np.float64) + npv["sp_b2"])
    sl = 1 / (1 + np.exp(-(h @ npv["sp_w3"].astype(np.float64) + npv["sp_b3"])))
    sl = sl[:, 0].astype(np.float32)
    spans = np.minimum((sl * (MAX_SPAN - MIN_SPAN) + MIN_SPAN).astype(np.int32),
                       S)
    return spans


def kernel(**inputs):
    npv = {k: np.asarray(v) for k, v in inputs.items()}
    f32 = np.float32
    td = float(f32(npv["temporal_decay"]))
    thr = float(f32(npv["adaptive_threshold"]))
    reg = float(f32(npv["attention_reg"]))
    hs_mean = float(f32(np.asarray(npv["head_scale"], np.float64).mean()))
    regp = float(f32(reg) + f32(1e-8))

    decay_full = (np.float64(td) ** (S - 1 - np.arange(S))).astype(np.float32)
    passing = decay_full > f32(thr)
    if not passing.any():
        return _uniform_fallback(npv, _numpy_spans(npv))
    s_min = int(np.nonzero(passing)[0].min())
    S_active = min(S, int(np.ceil((S - s_min + 8) / 32.0)) * 32)
    if S_active > 128:
        return _numpy_full_reference(npv)
    s_lo = S - S_active

    nc = _build(S_active, hs_mean, regp)
    consts = _host_consts(S_active, td, thr)

    import ml_dtypes
    bf16 = ml_dtypes.bfloat16

    def c(a, dt=np.float32):
        return np.ascontiguousarray(np.asarray(a, dtype=np.float32).astype(dt))

    rw1 = np.asarray(npv["rs_w1"], np.float32)
    shared = dict(
        w1x=c(rw1[:E]), w1c=c(rw1[E:], bf16),
        rs_w2=c(npv["rs_w2"]), rs_w3=c(npv["rs_w3"]),
        rs_b1=c(npv["rs_b1"]), rs_b2=c(npv["rs_b2"]), rs_b3=c(npv["rs_b3"]),
        sw1=c(npv["sp_w1"], bf16), sw2=c(npv["sp_w2"], bf16),
        sw3=c(npv["sp_w3"], bf16),
        sp_b1=c(npv["sp_b1"]), sp_b2=c(npv["sp_b2"]), sp_b3=c(npv["sp_b3"]),
        **{k: c(v) for k, v in consts.items()})
    in_maps = []
    for i in range(NCORES):
        bs = slice(i * BL, (i + 1) * BL)
        in_maps.append(dict(
            lx=c(npv["long_x"][s_lo:, bs, :].transpose(1, 0, 2)),
            encb=c(npv["encoded_x"][:, bs, :], bf16),
            **shared))

    res = run_bass_kernel_spmd(nc, in_maps, core_ids=list(range(NCORES)))
    global LAST_RESULT
    LAST_RESULT = res
    rs = res.results
    attn = np.concatenate([r["attn_out"] for r in rs], axis=1)
    feats = np.concatenate([r["feats_out"] for r in rs], axis=0)
    spans = np.concatenate([r["spans_out"] for r in rs], axis=0).astype(np.int32)
    return attn, feats, spans


if __name__ == "__main__":
    rng = np.random.default_rng(0)
    print("smoke test requires reference inputs; use test.py")


# revision 15
# speedup vs baseline: 2.5377x; 1.0896x over previous
"""AdaptiveSpanAttention TRN2 kernel: 8-way batch-parallel Bass/Tile kernel.

Structure exploited: fs = sigmoid(z)*decay_s can only exceed adaptive_threshold
where decay_s > threshold (since sigmoid < 1). For the reference scalars
(decay 0.95, thr 0.3) that is only the last 24 of 1024 positions, so the big
relevance MLP runs only on that row suffix. The threshold mask is evaluated on
the pre-sigmoid logit z against host-precomputed c_s = logit(thr/decay_s),
which is exactly monotone-equivalent and immune to LUT rounding at the
discontinuity. All matmuls run in true fp32 on the PE (4 cyc/row).

Sharding: batch dim (axis 1 of long_x/encoded_x) across the 8 NeuronCores,
weights replicated, outputs concatenated on host (pure data parallel).
"""
import sys

import numpy as np

sys.path.insert(0, "/opt/trn_rl_repo")

import concourse.tile as tile  # noqa: E402
from concourse import bacc, mybir  # noqa: E402
from concourse.bass_utils import run_bass_kernel_spmd  # noqa: E402

F32 = mybir.dt.float32
F32R = mybir.dt.float32r
BF16 = mybir.dt.bfloat16
I32 = mybir.dt.int32
AF = mybir.ActivationFunctionType
ALU = mybir.AluOpType
AXL = mybir.AxisListType

S, B, E, H = 1024, 64, 1024, 16
NCORES = 8
BL = B // NCORES  # 8 batches per core
MAX_SPAN, MIN_SPAN = 1024, 8
LAST_RESULT = None


def _build(S_active, hs_mean, regp):
    R = BL * S_active
    NQ = R // 128
    RB = min(R, 512)
    NRB = R // RB
    s_lo = S - S_active
    ZROWS = s_lo * BL

    nc = bacc.Bacc("TRN2", target_bir_lowering=False, debug=False,
                   num_devices=NCORES)

    def inp(name, shape, dt=F32):
        return nc.declare_dram_parameter(name, shape, dt, isOutput=False)

    lx = inp("lx", [BL, S_active, E])
    encb = inp("encb", [256, BL, E], BF16)
    w1x = inp("w1x", [E, E], F32R)          # rs_w1 top half (f32 bytes)
    w1c = inp("w1c", [E, E], BF16)          # rs_w1 bottom half, bf16
    rs_w2 = inp("rs_w2", [E, E // 2], F32R)
    rs_w3 = inp("rs_w3", [E // 2, 1], F32R)
    rs_b1 = inp("rs_b1", [E])
    rs_b2 = inp("rs_b2", [E // 2])
    rs_b3 = inp("rs_b3", [1])
    sw1 = inp("sw1", [E, E // 2], BF16)
    sw2 = inp("sw2", [E // 2, E // 4], BF16)
    sw3 = inp("sw3", [E // 4, 1], BF16)
    sp_b1 = inp("sp_b1", [E // 2])
    sp_b2 = inp("sp_b2", [E // 4])
    sp_b3 = inp("sp_b3", [1])
    ident = inp("ident", [128, 128])
    patt8 = inp("patt8", [128, BL], BF16)
    onehot = inp("onehot", [BL, R], F32R)
    bdm = inp("bdm", [R, BL])
    decay_r = inp("decay_r", [R])
    cthr_r = inp("cthr_r", [R])
    c2_r = inp("c2_r", [R])

    attn_out = nc.declare_dram_parameter("attn_out", [S, BL], F32, isOutput=True)
    feats_out = nc.declare_dram_parameter("feats_out", [BL, E], F32, isOutput=True)
    spans_out = nc.declare_dram_parameter("spans_out", [BL], I32, isOutput=True)

    enc_flat = encb.rearrange("s b e -> (s b) e")   # [2048, E]
    lx_flat = lx.rearrange("b s e -> (b s) e")      # [R, E]

    with tile.TileContext(nc) as tc, \
            tc.tile_pool(name="wts", bufs=1) as wp, \
            tc.tile_pool(name="per", bufs=1) as pp, \
            tc.tile_pool(name="acts", bufs=1) as ap, \
            tc.tile_pool(name="encs", bufs=16) as ep, \
            tc.tile_pool(name="cyc", bufs=3) as cy, \
            tc.tile_pool(name="pacc1", bufs=2, space="PSUM") as pacc1, \
            tc.tile_pool(name="pacc2", bufs=4, space="PSUM") as pacc2, \
            tc.tile_pool(name="ptr", bufs=1, space="PSUM") as ptr, \
            tc.tile_pool(name="psml", bufs=1, space="PSUM") as psml:

        # ---------------- DMAs (priority order) --------------------------
        ident_t = pp.tile([128, 128], F32, tag="ident")
        nc.sync.dma_start(ident_t[:], ident[:])
        patt8_t = pp.tile([128, BL], BF16, tag="patt8")
        nc.sync.dma_start(patt8_t[:], patt8[:])
        lx_t = [pp.tile([128, E], F32, tag=f"lx{q}", name=f"lx{q}")
                for q in range(NQ)]
        for q in range(NQ):
            nc.sync.dma_start(lx_t[q][:], lx_flat[128 * q:128 * (q + 1), :])
        rb1_t = pp.tile([128, 8], F32, tag="rb1")
        nc.sync.dma_start(rb1_t[:], rs_b1[:].rearrange("(m p) -> p m", p=128))
        rb2_t = pp.tile([128, 4], F32, tag="rb2")
        nc.sync.dma_start(rb2_t[:], rs_b2[:].rearrange("(m p) -> p m", p=128))
        rb3_t = pp.tile([1, 1], F32, tag="rb3")
        nc.sync.dma_start(rb3_t[:], rs_b3[:].rearrange("(p o) -> p o", p=1))
        sb1_t = pp.tile([128, 4], F32, tag="sb1")
        nc.sync.dma_start(sb1_t[:], sp_b1[:].rearrange("(m p) -> p m", p=128))
        sb2_t = pp.tile([128, 2], F32, tag="sb2")
        nc.sync.dma_start(sb2_t[:], sp_b2[:].rearrange("(m p) -> p m", p=128))
        sb3_t = pp.tile([1, 1], F32, tag="sb3")
        nc.sync.dma_start(sb3_t[:], sp_b3[:].rearrange("(p o) -> p o", p=1))

        enc_tiles = []
        for t in range(16):
            et = ep.tile([128, E], BF16, tag="enc", name="enc_t")
            nc.sync.dma_start(et[:], enc_flat[128 * t:128 * (t + 1), :])
            enc_tiles.append(et)

        wc_t = [wp.tile([128, E], BF16, tag=f"wc{k}", name=f"wc{k}")
                for k in range(8)]
        for k in range(8):
            nc.sync.dma_start(wc_t[k][:], w1c[128 * k:128 * (k + 1), :])
        onehot_t = pp.tile([BL, R], F32R, tag="onehot")
        nc.sync.dma_start(onehot_t[:], onehot[:])
        sw1_t = [wp.tile([128, E // 2], BF16, tag=f"sw1_{k}", name=f"sw1_{k}")
                 for k in range(8)]
        for k in range(8):
            nc.sync.dma_start(sw1_t[k][:], sw1[128 * k:128 * (k + 1), :])
        sw2_t = [wp.tile([128, E // 4], BF16, tag=f"sw2_{k}", name=f"sw2_{k}")
                 for k in range(4)]
        for k in range(4):
            nc.sync.dma_start(sw2_t[k][:], sw2[128 * k:128 * (k + 1), :])
        sw3_t = wp.tile([128, 2], BF16, tag="sw3")
        nc.sync.dma_start(sw3_t[:], sw3[:].rearrange("(k p) o -> p (k o)", p=128))

        wm_t = [wp.tile([128, E], F32R, tag=f"wm{k}", name=f"wm{k}")
                for k in range(8)]
        w2_t = [wp.tile([128, E // 2], F32R, tag=f"w2_{k}", name=f"w2_{k}")
                for k in range(8)]
        for k in range(8):
            nc.sync.dma_start(wm_t[k][:], w1x[128 * k:128 * (k + 1), :])
        for k in range(8):
            nc.sync.dma_start(w2_t[k][:], rs_w2[128 * k:128 * (k + 1), :])
        w3_t = wp.tile([128, 4], F32R, tag="w3")
        nc.sync.dma_start(w3_t[:], rs_w3[:].rearrange("(k p) o -> p (k o)", p=128))

        bdm_t = [pp.tile([128, BL], F32, tag=f"bdm{q}", name=f"bdm{q}")
                 for q in range(NQ)]
        for q in range(NQ):
            nc.sync.dma_start(bdm_t[q][:], bdm[128 * q:128 * (q + 1), :])
        dec_t = pp.tile([1, R], F32, tag="dec")
        nc.sync.dma_start(dec_t[:], decay_r[:].rearrange("(o r) -> o r", o=1))
        cthr_t = pp.tile([1, R], F32, tag="cthr")
        nc.sync.dma_start(cthr_t[:], cthr_r[:].rearrange("(o r) -> o r", o=1))
        c2_t = pp.tile([1, R], F32, tag="c2")
        nc.sync.dma_start(c2_t[:], c2_r[:].rearrange("(o r) -> o r", o=1))

        # attn zero rows: independent of everything -> emit early
        if ZROWS > 0:
            zcols = ZROWS // 128
            zt = pp.tile([128, zcols], F32, tag="zt")
            nc.vector.memset(zt[:], 0.0)
            nc.sync.dma_start(
                attn_out[0:S - S_active, :].rearrange("s b -> (s b)")
                .rearrange("(p f) -> p f", p=128), zt[:])

        # ---------------- PE: A0 transposes (lx^T), fp32 -> f32r --------
        a0 = [ap.tile([128, R], F32R, tag=f"a0_{c}", name=f"a0_{c}")
              for c in range(8)]
        for c in range(8):
            for q in range(NQ):
                tps = ptr.tile([128, 128], F32, tag="ptr", name="tps")
                nc.tensor.matmul(tps[:], lx_t[q][:, 128 * c:128 * (c + 1)],
                                 ident_t[:, :], is_transpose=True,
                                 start=True, stop=True)
                nc.scalar.copy(a0[c][:, 128 * q:128 * (q + 1)], tps[:])

        # ---------------- ctx = mean_s(enc) on PE (bf16, chases enc DMA) -
        ctx_t = pp.tile([BL, E], F32, tag="ctx")
        cps = [pacc1.tile([BL, 512], F32, tag="pacc1", name="cps")
               for _ in range(2)]
        for t in range(16):
            for h in range(2):
                nc.tensor.matmul(cps[h][:], patt8_t[:],
                                 enc_tiles[t][:, 512 * h:512 * (h + 1)],
                                 start=(t == 0), stop=(t == 15))
        for h in range(2):
            nc.scalar.activation(ctx_t[:, 512 * h:512 * (h + 1)], cps[h][:],
                                 AF.Copy, scale=1.0 / 256.0)

        # ---------------- ctxT (bf16) ------------------------------------
        ctxTb = pp.tile([128, 8 * BL], BF16, tag="ctxTb")
        for c in range(8):
            tps2 = ptr.tile([128, BL], F32, tag="ptr", name="tps2")
            nc.tensor.matmul(tps2[:], ctx_t[0:BL, 128 * c:128 * (c + 1)],
                             ident_t[0:BL, 0:BL], is_transpose=True,
                             start=True, stop=True)
            nc.scalar.copy(ctxTb[:, BL * c:BL * (c + 1)], tps2[:])

        # ---------------- c1 = ctx @ rs_w1[E:]  (bf16 -> f32r) ----------
        c1n_t = pp.tile([BL, E], F32R, tag="c1n")
        for m in range(8):
            c1ps = ptr.tile([BL, 128], F32, tag="ptr", name="c1ps")
            for k in range(8):
                nc.tensor.matmul(c1ps[:], ctxTb[:, BL * k:BL * (k + 1)],
                                 wc_t[k][:, 128 * m:128 * (m + 1)],
                                 start=(k == 0), stop=(k == 7))
            nc.scalar.copy(c1n_t[:, 128 * m:128 * (m + 1)], c1ps[:])

        # ---------------- span predictor MLP (bf16), early ---------------
        h1b = pp.tile([128, 4 * BL], BF16, tag="h1b")
        for m in range(4):
            hps = ptr.tile([128, BL], F32, tag="ptr", name="hps")
            for k in range(8):
                nc.tensor.matmul(hps[:], sw1_t[k][:, 128 * m:128 * (m + 1)],
                                 ctxTb[:, BL * k:BL * (k + 1)],
                                 start=(k == 0), stop=(k == 7))
            nc.scalar.activation(h1b[:, BL * m:BL * (m + 1)], hps[:], AF.Gelu,
                                 bias=sb1_t[:, m:m + 1])
        h2b = pp.tile([128, 2 * BL], BF16, tag="h2b")
        for m in range(2):
            hps2 = ptr.tile([128, BL], F32, tag="ptr", name="hps2")
            for k in range(4):
                nc.tensor.matmul(hps2[:], sw2_t[k][:, 128 * m:128 * (m + 1)],
                                 h1b[:, BL * k:BL * (k + 1)],
                                 start=(k == 0), stop=(k == 3))
            nc.scalar.activation(h2b[:, BL * m:BL * (m + 1)], hps2[:], AF.Gelu,
                                 bias=sb2_t[:, m:m + 1])
        ups = psml.tile([1, BL], F32, tag="psml", name="ups")
        for k in range(2):
            nc.tensor.matmul(ups[:], sw3_t[:, k:k + 1],
                             h2b[:, BL * k:BL * (k + 1)],
                             start=(k == 0), stop=(k == 1))
        sl_t = pp.tile([1, BL], F32, tag="sl")
        nc.scalar.activation(sl_t[:], ups[:], AF.Sigmoid, bias=sb3_t[0:1, 0:1])
        tspan = pp.tile([1, BL], F32, tag="tspan")
        nc.vector.tensor_scalar(tspan[:], sl_t[:], float(MAX_SPAN - MIN_SPAN),
                                float(MIN_SPAN), ALU.mult, ALU.add)
        tsh = pp.tile([1, BL], F32, tag="tsh")
        nc.vector.tensor_scalar(tsh[:], tspan[:], 0.5, None, ALU.subtract)
        spans_i = pp.tile([1, BL], I32, tag="spans_i")
        nc.vector.tensor_copy(spans_i[:], tsh[:])
        span_f = pp.tile([1, BL], F32, tag="span_f")
        nc.vector.tensor_copy(span_f[:], spans_i[:])
        nc.sync.dma_start(spans_out[:].rearrange("(o b) -> o b", o=1), spans_i[:])

        # ---------------- main MLP: k-outer m-pairs, pipelined L2 (f32r) -
        mask_t = pp.tile([1, R], F32, tag="mask")
        rel_t = pp.tile([1, R], F32, tag="rel")
        for rb in range(NRB):
            r0 = rb * RB
            a1 = [ap.tile([128, RB], F32R, tag=f"a1_{m}", name=f"a1_{m}")
                  for m in range(8)]
            ps2 = [pacc2.tile([128, RB], F32, tag="pacc2", name="ps2")
                   for _ in range(4)]

            def emit_l2(mm):
                for m2 in range(4):
                    nc.tensor.matmul(ps2[m2][:],
                                     w2_t[mm][:, 128 * m2:128 * (m2 + 1)],
                                     a1[mm][:], start=(mm == 0), stop=(mm == 7))

            for p in range(4):
                mA, mB = 2 * p, 2 * p + 1
                psA = pacc1.tile([128, RB], F32, tag="pacc1", name="ps1a")
                psB = pacc1.tile([128, RB], F32, tag="pacc1", name="ps1b")
                for k in range(8):
                    nc.tensor.matmul(psA[:], wm_t[k][:, 128 * mA:128 * (mA + 1)],
                                     a0[k][:, r0:r0 + RB],
                                     start=(k == 0), stop=False)
                    nc.tensor.matmul(psB[:], wm_t[k][:, 128 * mB:128 * (mB + 1)],
                                     a0[k][:, r0:r0 + RB],
                                     start=(k == 0), stop=False)
                nc.tensor.matmul(psA[:], c1n_t[0:BL, 128 * mA:128 * (mA + 1)],
                                 onehot_t[0:BL, r0:r0 + RB],
                                 start=False, stop=True)
                nc.scalar.activation(a1[mA][:], psA[:], AF.Gelu,
                                     bias=rb1_t[:, mA:mA + 1])
                nc.tensor.matmul(psB[:], c1n_t[0:BL, 128 * mB:128 * (mB + 1)],
                                 onehot_t[0:BL, r0:r0 + RB],
                                 start=False, stop=True)
                nc.scalar.activation(a1[mB][:], psB[:], AF.Gelu,
                                     bias=rb1_t[:, mB:mB + 1])
                if p >= 1:
                    emit_l2(2 * p - 2)
                    emit_l2(2 * p - 1)
            emit_l2(6)
            emit_l2(7)

            a2 = [ap.tile([128, RB], F32R, tag=f"a2_{m2}", name=f"a2_{m2}")
                  for m2 in range(4)]
            for m2 in range(4):
                nc.scalar.activation(a2[m2][:], ps2[m2][:], AF.Gelu,
                                     bias=rb2_t[:, m2:m2 + 1])
            zps = psml.tile([1, RB], F32, tag="psml", name="zps")
            for m2 in range(4):
                nc.tensor.matmul(zps[:], w3_t[:, m2:m2 + 1], a2[m2][:],
                                 start=(m2 == 0), stop=(m2 == 3))
            # mask on pre-sigmoid logit (cthr_r has rs_b3 folded in host-side)
            nc.vector.tensor_tensor(mask_t[:, r0:r0 + RB], zps[:],
                                    cthr_t[:, r0:r0 + RB], ALU.is_gt)
            nc.scalar.activation(rel_t[:, r0:r0 + RB], zps[:], AF.Sigmoid,
                                 bias=rb3_t[0:1, 0:1])

        # ---------------- scores (DVE + tiny PE broadcasts) --------------
        stp = psml.tile([BL, 1], F32, tag="psml", name="stp")
        nc.tensor.matmul(stp[:], span_f[0:1, 0:BL], ident_t[0:1, 0:1],
                         is_transpose=True, start=True, stop=True)
        spanTr = pp.tile([BL, 1], F32R, tag="spanTr")
        nc.scalar.copy(spanTr[:], stp[:])
        valid_t = pp.tile([1, R], F32, tag="valid")
        for n0 in range(0, R, 512):
            nn = min(512, R - n0)
            bps = psml.tile([1, 512], F32, tag="psml", name="bps")
            nc.tensor.matmul(bps[:, 0:nn], spanTr[:], onehot_t[:, n0:n0 + nn],
                             start=True, stop=True)
            nc.vector.tensor_tensor(valid_t[:, n0:n0 + nn], bps[:, 0:nn],
                                    c2_t[:, n0:n0 + nn], ALU.is_gt)
        fs_t = pp.tile([1, R], F32, tag="fs")
        nc.vector.tensor_tensor(fs_t[:], rel_t[:], dec_t[:], ALU.mult)
        nc.vector.tensor_tensor(fs_t[:], fs_t[:], mask_t[:], ALU.mult)
        nc.vector.tensor_scalar(fs_t[:], fs_t[:], float(hs_mean), None, ALU.mult)
        nc.vector.tensor_tensor(fs_t[:], fs_t[:], valid_t[:], ALU.mult)
        total_t = pp.tile([1, BL], F32, tag="total")
        nc.vector.tensor_reduce(total_t[:],
                                fs_t[:].rearrange("p (b s) -> p b s", b=BL),
                                AXL.X, ALU.add)
        den_t = pp.tile([1, BL], F32, tag="den")
        nc.vector.tensor_scalar(den_t[:], total_t[:], float(regp), None, ALU.add)
        recip_t = pp.tile([1, BL], F32, tag="recip")
        nc.vector.reciprocal(recip_t[:], den_t[:])
        fsn_t = pp.tile([1, R], F32, tag="fsn")
        for b in range(BL):
            sl_ = slice(S_active * b, S_active * (b + 1))
            nc.vector.tensor_scalar(fsn_t[:, sl_], fs_t[:, sl_],
                                    recip_t[0:1, b:b + 1], None, ALU.mult)

        # ---------------- attn active rows -------------------------------
        for b in range(BL):
            nc.sync.dma_start(
                attn_out[S - S_active:S, b:b + 1].rearrange("s x -> (s x)")
                .rearrange("(o f) -> o f", o=1),
                fsn_t[:, S_active * b:S_active * (b + 1)])

        # ---------------- feats (block-diag packed fp32 matmul) ---------
        psf = [pacc2.tile([BL, 512], F32, tag="pacc2", name="psf")
               for _ in range(2)]
        for q in range(NQ):
            fps = ptr.tile([128, 1], F32, tag="ptr", name="fps")
            nc.tensor.matmul(fps[:], fsn_t[:, 128 * q:128 * (q + 1)],
                             ident_t[0:1, 0:1], is_transpose=True,
                             start=True, stop=True)
            fsnT = cy.tile([128, 1], F32, tag="fsnT", name="fsnT")
            nc.scalar.copy(fsnT[:], fps[:])
            bd = cy.tile([128, BL], F32, tag="bd", name="bd")
            nc.vector.tensor_scalar(bd[:], bdm_t[q][:], fsnT[:], None, ALU.mult)
            for h in range(2):
                nc.tensor.matmul(psf[h][:], bd[:],
                                 lx_t[q][:, 512 * h:512 * (h + 1)],
                                 start=(q == 0), stop=(q == NQ - 1))
        feats_t = pp.tile([BL, E], F32, tag="feats")
        for h in range(2):
            nc.scalar.copy(feats_t[:, 512 * h:512 * (h + 1)], psf[h][:])
        nc.sync.dma_start(feats_out[:], feats_t[:])

    nc.compile()
    return nc


def _host_consts(S_active, td, thr, b3=0.0):
    """Host-side constant tensors for one core (b-major rows). The mask is
    evaluated on the pre-bias logit z (PSUM), so rs_b3 is folded into cthr."""
    R = BL * S_active
    s_lo = S - S_active
    s_idx = np.arange(S)
    decay_full = (np.float64(td) ** (S - 1 - s_idx)).astype(np.float32)
    s_of_r = s_lo + (np.arange(R) % S_active)
    dec_r = decay_full[s_of_r]
    q = np.float64(thr) / dec_r.astype(np.float64)
    with np.errstate(divide="ignore", invalid="ignore"):
        logit = np.log(q / (1.0 - q))
    cthr = np.where(q >= 1.0, 1e30, np.where(q <= 0.0, -1e30,
                                              logit - np.float64(b3)))
    cthr = cthr.astype(np.float32)
    c2 = (S - 1 - s_of_r).astype(np.float32)
    b_of_r = np.arange(R) // S_active
    onehot = (b_of_r[None, :] == np.arange(BL)[:, None]).astype(np.float32)
    bdm = onehot.T.copy()
    ident = np.eye(128, dtype=np.float32)
    patt8 = (np.arange(128)[:, None] % BL == np.arange(BL)[None, :]).astype(
        np.float32)
    return dict(ident=ident, patt8=patt8, onehot=onehot, bdm=bdm,
                decay_r=dec_r, cthr_r=cthr, c2_r=c2)


def _uniform_fallback(npv, spans):
    """Exact outputs when no position can pass the threshold (total==0
    everywhere): attn = valid/span, feats accordingly."""
    lx = npv["long_x"].astype(np.float32)
    start = (S - spans).astype(np.int32)
    pos = np.arange(S)
    valid = (pos[:, None] >= start[None, :]).astype(np.float32)
    attn = valid / spans[None, :].astype(np.float32)
    feats = np.einsum("sb,sbe->be", attn, lx).astype(np.float32)
    return attn, feats, spans


def _numpy_full_reference(npv):
    """Exact float32 numpy replica of the reference (slow, safety net for
    near-dense threshold patterns that the sparse device kernel doesn't
    cover)."""
    from scipy.special import erf
    f32 = np.float32
    lx = npv["long_x"].astype(f32)
    ctx = npv["encoded_x"].astype(f32).mean(axis=0, dtype=f32)

    def gelu(x):
        return (0.5 * x * (1 + erf(x / np.sqrt(2)))).astype(f32)

    spans = _numpy_spans(npv)
    start = S - spans
    pos = np.arange(S)
    valid = (pos[:, None] >= start[None, :]).astype(f32)
    comb = np.concatenate([lx, np.broadcast_to(ctx[None], lx.shape)], axis=-1)
    r = gelu(comb.reshape(-1, 2 * E) @ npv["rs_w1"].astype(f32) +
             npv["rs_b1"].astype(f32))
    r = gelu(r @ npv["rs_w2"].astype(f32) + npv["rs_b2"].astype(f32))
    rel = 1 / (1 + np.exp(-(r @ npv["rs_w3"].astype(f32) +
                            npv["rs_b3"].astype(f32))))
    rel = rel[:, 0].reshape(S, B).astype(f32)
    td = f32(npv["temporal_decay"])
    decay = (np.float64(td) ** (S - 1 - pos)).astype(f32)
    fs = rel * decay[:, None]
    fs = fs * (fs > f32(npv["adaptive_threshold"]))
    fs = fs * f32(np.asarray(npv["head_scale"], np.float64).mean())
    fs = fs * valid
    total = fs.sum(axis=0, dtype=f32)
    normed = fs / (total[None, :] + f32(npv["attention_reg"]) + f32(1e-8))
    uniform = valid / spans[None, :].astype(f32)
    attn = np.where(total[None, :] > 0, normed, uniform).astype(f32)
    feats = np.einsum("sb,sbe->be", attn, lx).astype(f32)
    return attn, feats, spans


def _numpy_spans(npv):
    from scipy.special import erf
    ctx = npv["encoded_x"].astype(np.float64).mean(axis=0)

    def gelu(x):
        return 0.5 * x * (1 + erf(x / np.sqrt(2)))

    h = gelu(ctx @ npv["sp_w1"].astype(np.float64) + npv["sp_b1"])
    h = gelu(h @ npv["sp_w2"].astype(np.float64) + npv["sp_b2"])
    sl = 1 / (1 + np.exp(-(h @ npv["sp_w3"].astype(np.float64) + npv["sp_b3"])))
    sl = sl[:, 0].astype(np.float32)
    spans = np.minimum((sl * (MAX_SPAN - MIN_SPAN) + MIN_SPAN).astype(np.int32),
                       S)
    return spans


def kernel(**inputs):
    npv = {k: np.asarray(v) for k, v in inputs.items()}
    f32 = np.float32
    td = float(f32(npv["temporal_decay"]))
    thr = float(f32(npv["adaptive_threshold"]))
    reg = float(f32(npv["attention_reg"]))
    hs_mean = float(f32(np.asarray(npv["head_scale"], np.float64).mean()))
    regp = float(f32(reg) + f32(1e-8))

    decay_full = (np.float64(td) ** (S - 1 - np.arange(S))).astype(np.float32)
    passing = decay_full > f32(thr)
    if not passing.any():
        return _uniform_fallback(npv, _numpy_spans(npv))
    s_min = int(np.nonzero(passing)[0].min())
    S_active = min(S, int(np.ceil((S - s_min + 8) / 32.0)) * 32)
    if S_active > 128:
        return _numpy_full_reference(npv)
    s_lo = S - S_active

    nc = _build(S_active, hs_mean, regp)
    consts = _host_consts(S_active, td, thr,
                          float(np.float32(npv['rs_b3'].reshape(-1)[0])))

    import ml_dtypes
    bf16 = ml_dtypes.bfloat16

    def c(a, dt=np.float32):
        return np.ascontiguousarray(np.asarray(a, dtype=np.float32).astype(dt))

    rw1 = np.asarray(npv["rs_w1"], np.float32)
    shared = dict(
        w1x=c(rw1[:E]), w1c=c(rw1[E:], bf16),
        rs_w2=c(npv["rs_w2"]), rs_w3=c(npv["rs_w3"]),
        rs_b1=c(npv["rs_b1"]), rs_b2=c(npv["rs_b2"]), rs_b3=c(npv["rs_b3"]),
        sw1=c(npv["sp_w1"], bf16), sw2=c(npv["sp_w2"], bf16),
        sw3=c(npv["sp_w3"], bf16),
        sp_b1=c(npv["sp_b1"]), sp_b2=c(npv["sp_b2"]), sp_b3=c(npv["sp_b3"]),
        **{k: (c(v, bf16) if k == "patt8" else c(v)) for k, v in consts.items()})
    in_maps = []
    for i in range(NCORES):
        bs = slice(i * BL, (i + 1) * BL)
        in_maps.append(dict(
            lx=c(npv["long_x"][s_lo:, bs, :].transpose(1, 0, 2)),
            encb=c(npv["encoded_x"][:, bs, :], bf16),
            **shared))

    res = run_bass_kernel_spmd(nc, in_maps, core_ids=list(range(NCORES)))
    global LAST_RESULT
    LAST_RESULT = res
    rs = res.results
    attn = np.concatenate([r["attn_out"] for r in rs], axis=1)
    feats = np.concatenate([r["feats_out"] for r in rs], axis=0)
    spans = np.concatenate([r["spans_out"] for r in rs], axis=0).astype(np.int32)
    return attn, feats, spans


if __name__ == "__main__":
    rng = np.random.default_rng(0)
    print("smoke test requires reference inputs; use test.py")


# revision 16
# speedup vs baseline: 2.5668x; 1.0114x over previous
"""AdaptiveSpanAttention TRN2 kernel: 8-way batch-parallel Bass/Tile kernel.

Structure exploited: fs = sigmoid(z)*decay_s can only exceed adaptive_threshold
where decay_s > threshold (since sigmoid < 1). For the reference scalars
(decay 0.95, thr 0.3) that is only the last 24 of 1024 positions, so the big
relevance MLP runs only on that row suffix. The threshold mask is evaluated on
the pre-sigmoid logit z against host-precomputed c_s = logit(thr/decay_s),
which is exactly monotone-equivalent and immune to LUT rounding at the
discontinuity. All matmuls run in true fp32 on the PE (4 cyc/row).

Sharding: batch dim (axis 1 of long_x/encoded_x) across the 8 NeuronCores,
weights replicated, outputs concatenated on host (pure data parallel).
"""
import sys

import numpy as np

sys.path.insert(0, "/opt/trn_rl_repo")

import concourse.tile as tile  # noqa: E402
from concourse import bacc, mybir  # noqa: E402
from concourse.bass_utils import run_bass_kernel_spmd  # noqa: E402

F32 = mybir.dt.float32
F32R = mybir.dt.float32r
BF16 = mybir.dt.bfloat16
I32 = mybir.dt.int32
AF = mybir.ActivationFunctionType
ALU = mybir.AluOpType
AXL = mybir.AxisListType

S, B, E, H = 1024, 64, 1024, 16
NCORES = 8
BL = B // NCORES  # 8 batches per core
MAX_SPAN, MIN_SPAN = 1024, 8
LAST_RESULT = None


def _build(S_active, hs_mean, regp):
    R = BL * S_active
    NQ = R // 128
    RB = min(R, 512)
    NRB = R // RB
    s_lo = S - S_active
    ZROWS = s_lo * BL

    nc = bacc.Bacc("TRN2", target_bir_lowering=False, debug=False,
                   num_devices=NCORES)

    def inp(name, shape, dt=F32):
        return nc.declare_dram_parameter(name, shape, dt, isOutput=False)

    lx = inp("lx", [BL, S_active, E])
    encb = inp("encb", [256, BL, E], BF16)
    w1x = inp("w1x", [E, E], F32R)          # rs_w1 top half (f32 bytes)
    w1c = inp("w1c", [E, E], BF16)          # rs_w1 bottom half, bf16
    rs_w2 = inp("rs_w2", [E, E // 2], F32R)
    rs_w3 = inp("rs_w3", [E // 2, 1], F32R)
    rs_b1 = inp("rs_b1", [E])
    rs_b2 = inp("rs_b2", [E // 2])
    rs_b3 = inp("rs_b3", [1])
    sw1 = inp("sw1", [E, E // 2], BF16)
    sw2 = inp("sw2", [E // 2, E // 4], BF16)
    sw3 = inp("sw3", [E // 4, 1], BF16)
    sp_b1 = inp("sp_b1", [E // 2])
    sp_b2 = inp("sp_b2", [E // 4])
    sp_b3 = inp("sp_b3", [1])
    ident = inp("ident", [128, 128])
    patt8 = inp("patt8", [128, BL], BF16)
    onehot = inp("onehot", [BL, R], F32R)
    bdm = inp("bdm", [R, BL])
    decay_r = inp("decay_r", [R])
    cthr_r = inp("cthr_r", [R])
    c2_r = inp("c2_r", [R])

    attn_out = nc.declare_dram_parameter("attn_out", [S, BL], F32, isOutput=True)
    feats_out = nc.declare_dram_parameter("feats_out", [BL, E], F32, isOutput=True)
    spans_out = nc.declare_dram_parameter("spans_out", [BL], I32, isOutput=True)

    enc_flat = encb.rearrange("s b e -> (s b) e")   # [2048, E]
    lx_flat = lx.rearrange("b s e -> (b s) e")      # [R, E]

    with tile.TileContext(nc) as tc, \
            tc.tile_pool(name="wts", bufs=1) as wp, \
            tc.tile_pool(name="per", bufs=1) as pp, \
            tc.tile_pool(name="acts", bufs=1) as ap, \
            tc.tile_pool(name="encs", bufs=16) as ep, \
            tc.tile_pool(name="cyc", bufs=3) as cy, \
            tc.tile_pool(name="pacc1", bufs=2, space="PSUM") as pacc1, \
            tc.tile_pool(name="pacc2", bufs=4, space="PSUM") as pacc2, \
            tc.tile_pool(name="ptr", bufs=1, space="PSUM") as ptr, \
            tc.tile_pool(name="psml", bufs=1, space="PSUM") as psml:

        # ---------------- DMAs (priority order) --------------------------
        ident_t = pp.tile([128, 128], F32, tag="ident")
        nc.sync.dma_start(ident_t[:], ident[:])
        patt8_t = pp.tile([128, BL], BF16, tag="patt8")
        nc.sync.dma_start(patt8_t[:], patt8[:])
        lx_t = [pp.tile([128, E], F32, tag=f"lx{q}", name=f"lx{q}")
                for q in range(NQ)]
        for q in range(NQ):
            nc.sync.dma_start(lx_t[q][:], lx_flat[128 * q:128 * (q + 1), :])
        rb1_t = pp.tile([128, 8], F32, tag="rb1")
        nc.sync.dma_start(rb1_t[:], rs_b1[:].rearrange("(m p) -> p m", p=128))
        rb2_t = pp.tile([128, 4], F32, tag="rb2")
        nc.sync.dma_start(rb2_t[:], rs_b2[:].rearrange("(m p) -> p m", p=128))
        rb3_t = pp.tile([1, 1], F32, tag="rb3")
        nc.sync.dma_start(rb3_t[:], rs_b3[:].rearrange("(p o) -> p o", p=1))
        sb1_t = pp.tile([128, 4], F32, tag="sb1")
        nc.sync.dma_start(sb1_t[:], sp_b1[:].rearrange("(m p) -> p m", p=128))
        sb2_t = pp.tile([128, 2], F32, tag="sb2")
        nc.sync.dma_start(sb2_t[:], sp_b2[:].rearrange("(m p) -> p m", p=128))
        sb3_t = pp.tile([1, 1], F32, tag="sb3")
        nc.sync.dma_start(sb3_t[:], sp_b3[:].rearrange("(p o) -> p o", p=1))

        enc_tiles = []
        for t in range(16):
            et = ep.tile([128, E], BF16, tag="enc", name="enc_t")
            nc.sync.dma_start(et[:], enc_flat[128 * t:128 * (t + 1), :])
            enc_tiles.append(et)

        wc_t = [wp.tile([128, E], BF16, tag=f"wc{k}", name=f"wc{k}")
                for k in range(8)]
        for k in range(8):
            nc.sync.dma_start(wc_t[k][:], w1c[128 * k:128 * (k + 1), :])
        onehot_t = pp.tile([BL, R], F32R, tag="onehot")
        nc.sync.dma_start(onehot_t[:], onehot[:])
        wm_t = [wp.tile([128, E], F32R, tag=f"wm{k}", name=f"wm{k}")
                for k in range(8)]
        w2_t = [wp.tile([128, E // 2], F32R, tag=f"w2_{k}", name=f"w2_{k}")
                for k in range(8)]
        for k in range(8):
            nc.sync.dma_start(wm_t[k][:], w1x[128 * k:128 * (k + 1), :])
        for k in range(8):
            nc.sync.dma_start(w2_t[k][:], rs_w2[128 * k:128 * (k + 1), :])
        w3_t = wp.tile([128, 4], F32R, tag="w3")
        nc.sync.dma_start(w3_t[:], rs_w3[:].rearrange("(k p) o -> p (k o)", p=128))

        bdm_t = [pp.tile([128, BL], F32, tag=f"bdm{q}", name=f"bdm{q}")
                 for q in range(NQ)]
        for q in range(NQ):
            nc.sync.dma_start(bdm_t[q][:], bdm[128 * q:128 * (q + 1), :])
        dec_t = pp.tile([1, R], F32, tag="dec")
        nc.sync.dma_start(dec_t[:], decay_r[:].rearrange("(o r) -> o r", o=1))
        cthr_t = pp.tile([1, R], F32, tag="cthr")
        nc.sync.dma_start(cthr_t[:], cthr_r[:].rearrange("(o r) -> o r", o=1))
        c2_t = pp.tile([1, R], F32, tag="c2")
        nc.sync.dma_start(c2_t[:], c2_r[:].rearrange("(o r) -> o r", o=1))

        sw1_t = [wp.tile([128, E // 2], BF16, tag=f"sw1_{k}", name=f"sw1_{k}")
                 for k in range(8)]
        for k in range(8):
            nc.sync.dma_start(sw1_t[k][:], sw1[128 * k:128 * (k + 1), :])
        sw2_t = [wp.tile([128, E // 4], BF16, tag=f"sw2_{k}", name=f"sw2_{k}")
                 for k in range(4)]
        for k in range(4):
            nc.sync.dma_start(sw2_t[k][:], sw2[128 * k:128 * (k + 1), :])
        sw3_t = wp.tile([128, 2], BF16, tag="sw3")
        nc.sync.dma_start(sw3_t[:], sw3[:].rearrange("(k p) o -> p (k o)", p=128))

        # attn zero rows: independent of everything -> emit early
        if ZROWS > 0:
            zcols = ZROWS // 128
            zt = pp.tile([128, zcols], F32, tag="zt")
            nc.vector.memset(zt[:], 0.0)
            nc.sync.dma_start(
                attn_out[0:S - S_active, :].rearrange("s b -> (s b)")
                .rearrange("(p f) -> p f", p=128), zt[:])

        # ---------------- PE: A0 transposes (lx^T), fp32 -> f32r --------
        a0 = [ap.tile([128, R], F32R, tag=f"a0_{c}", name=f"a0_{c}")
              for c in range(8)]
        for c in range(8):
            for q in range(NQ):
                tps = ptr.tile([128, 128], F32, tag="ptr", name="tps")
                nc.tensor.matmul(tps[:], lx_t[q][:, 128 * c:128 * (c + 1)],
                                 ident_t[:, :], is_transpose=True,
                                 start=True, stop=True)
                nc.scalar.copy(a0[c][:, 128 * q:128 * (q + 1)], tps[:])

        # ---------------- ctx = mean_s(enc) on PE (bf16, chases enc DMA) -
        ctx_t = pp.tile([BL, E], F32, tag="ctx")
        cps = [pacc1.tile([BL, 512], F32, tag="pacc1", name="cps")
               for _ in range(2)]
        for t in range(16):
            for h in range(2):
                nc.tensor.matmul(cps[h][:], patt8_t[:],
                                 enc_tiles[t][:, 512 * h:512 * (h + 1)],
                                 start=(t == 0), stop=(t == 15))
        for h in range(2):
            nc.scalar.activation(ctx_t[:, 512 * h:512 * (h + 1)], cps[h][:],
                                 AF.Copy, scale=1.0 / 256.0)

        # ---------------- ctxT (bf16) ------------------------------------
        ctxTb = pp.tile([128, 8 * BL], BF16, tag="ctxTb")
        for c in range(8):
            tps2 = ptr.tile([128, BL], F32, tag="ptr", name="tps2")
            nc.tensor.matmul(tps2[:], ctx_t[0:BL, 128 * c:128 * (c + 1)],
                             ident_t[0:BL, 0:BL], is_transpose=True,
                             start=True, stop=True)
            nc.scalar.copy(ctxTb[:, BL * c:BL * (c + 1)], tps2[:])

        # ---------------- c1 = ctx @ rs_w1[E:]  (bf16 -> f32r) ----------
        c1n_t = pp.tile([BL, E], F32R, tag="c1n")
        for m in range(8):
            c1ps = ptr.tile([BL, 128], F32, tag="ptr", name="c1ps")
            for k in range(8):
                nc.tensor.matmul(c1ps[:], ctxTb[:, BL * k:BL * (k + 1)],
                                 wc_t[k][:, 128 * m:128 * (m + 1)],
                                 start=(k == 0), stop=(k == 7))
            nc.scalar.copy(c1n_t[:, 128 * m:128 * (m + 1)], c1ps[:])

        # ---------------- main MLP: k-outer m-pairs, pipelined L2 (f32r) -
        mask_t = pp.tile([1, R], F32, tag="mask")
        rel_t = pp.tile([1, R], F32, tag="rel")
        for rb in range(NRB):
            r0 = rb * RB
            a1 = [ap.tile([128, RB], F32R, tag=f"a1_{m}", name=f"a1_{m}")
                  for m in range(8)]
            ps2 = [pacc2.tile([128, RB], F32, tag="pacc2", name="ps2")
                   for _ in range(4)]

            def emit_l2(mm):
                for m2 in range(4):
                    nc.tensor.matmul(ps2[m2][:],
                                     w2_t[mm][:, 128 * m2:128 * (m2 + 1)],
                                     a1[mm][:], start=(mm == 0), stop=(mm == 7))

            for p in range(4):
                mA, mB = 2 * p, 2 * p + 1
                psA = pacc1.tile([128, RB], F32, tag="pacc1", name="ps1a")
                psB = pacc1.tile([128, RB], F32, tag="pacc1", name="ps1b")
                for k in range(8):
                    nc.tensor.matmul(psA[:], wm_t[k][:, 128 * mA:128 * (mA + 1)],
                                     a0[k][:, r0:r0 + RB],
                                     start=(k == 0), stop=False)
                    nc.tensor.matmul(psB[:], wm_t[k][:, 128 * mB:128 * (mB + 1)],
                                     a0[k][:, r0:r0 + RB],
                                     start=(k == 0), stop=False)
                nc.tensor.matmul(psA[:], c1n_t[0:BL, 128 * mA:128 * (mA + 1)],
                                 onehot_t[0:BL, r0:r0 + RB],
                                 start=False, stop=True)
                nc.scalar.activation(a1[mA][:], psA[:], AF.Gelu,
                                     bias=rb1_t[:, mA:mA + 1])
                nc.tensor.matmul(psB[:], c1n_t[0:BL, 128 * mB:128 * (mB + 1)],
                                 onehot_t[0:BL, r0:r0 + RB],
                                 start=False, stop=True)
                nc.scalar.activation(a1[mB][:], psB[:], AF.Gelu,
                                     bias=rb1_t[:, mB:mB + 1])
                if p >= 1:
                    emit_l2(2 * p - 2)
                    emit_l2(2 * p - 1)
            emit_l2(6)
            emit_l2(7)

            a2 = [ap.tile([128, RB], F32R, tag=f"a2_{m2}", name=f"a2_{m2}")
                  for m2 in range(4)]
            for m2 in range(4):
                nc.scalar.activation(a2[m2][:], ps2[m2][:], AF.Gelu,
                                     bias=rb2_t[:, m2:m2 + 1])
            zps = psml.tile([1, RB], F32, tag="psml", name="zps")
            for m2 in range(4):
                nc.tensor.matmul(zps[:], w3_t[:, m2:m2 + 1], a2[m2][:],
                                 start=(m2 == 0), stop=(m2 == 3))
            # mask on pre-sigmoid logit (cthr_r has rs_b3 folded in host-side)
            nc.vector.tensor_tensor(mask_t[:, r0:r0 + RB], zps[:],
                                    cthr_t[:, r0:r0 + RB], ALU.is_gt)
            nc.scalar.activation(rel_t[:, r0:r0 + RB], zps[:], AF.Sigmoid,
                                 bias=rb3_t[0:1, 0:1])

        # ---------------- span predictor MLP (bf16), early ---------------
        h1b = pp.tile([128, 4 * BL], BF16, tag="h1b")
        for m in range(4):
            hps = ptr.tile([128, BL], F32, tag="ptr", name="hps")
            for k in range(8):
                nc.tensor.matmul(hps[:], sw1_t[k][:, 128 * m:128 * (m + 1)],
                                 ctxTb[:, BL * k:BL * (k + 1)],
                                 start=(k == 0), stop=(k == 7))
            nc.scalar.activation(h1b[:, BL * m:BL * (m + 1)], hps[:], AF.Gelu,
                                 bias=sb1_t[:, m:m + 1])
        h2b = pp.tile([128, 2 * BL], BF16, tag="h2b")
        for m in range(2):
            hps2 = ptr.tile([128, BL], F32, tag="ptr", name="hps2")
            for k in range(4):
                nc.tensor.matmul(hps2[:], sw2_t[k][:, 128 * m:128 * (m + 1)],
                                 h1b[:, BL * k:BL * (k + 1)],
                                 start=(k == 0), stop=(k == 3))
            nc.scalar.activation(h2b[:, BL * m:BL * (m + 1)], hps2[:], AF.Gelu,
                                 bias=sb2_t[:, m:m + 1])
        ups = psml.tile([1, BL], F32, tag="psml", name="ups")
        for k in range(2):
            nc.tensor.matmul(ups[:], sw3_t[:, k:k + 1],
                             h2b[:, BL * k:BL * (k + 1)],
                             start=(k == 0), stop=(k == 1))
        sl_t = pp.tile([1, BL], F32, tag="sl")
        nc.scalar.activation(sl_t[:], ups[:], AF.Sigmoid, bias=sb3_t[0:1, 0:1])
        tspan = pp.tile([1, BL], F32, tag="tspan")
        nc.vector.tensor_scalar(tspan[:], sl_t[:], float(MAX_SPAN - MIN_SPAN),
                                float(MIN_SPAN), ALU.mult, ALU.add)
        tsh = pp.tile([1, BL], F32, tag="tsh")
        nc.vector.tensor_scalar(tsh[:], tspan[:], 0.5, None, ALU.subtract)
        spans_i = pp.tile([1, BL], I32, tag="spans_i")
        nc.vector.tensor_copy(spans_i[:], tsh[:])
        span_f = pp.tile([1, BL], F32, tag="span_f")
        nc.vector.tensor_copy(span_f[:], spans_i[:])
        nc.sync.dma_start(spans_out[:].rearrange("(o b) -> o b", o=1), spans_i[:])

        # ---------------- scores (DVE + tiny PE broadcasts) --------------
        stp = psml.tile([BL, 1], F32, tag="psml", name="stp")
        nc.tensor.matmul(stp[:], span_f[0:1, 0:BL], ident_t[0:1, 0:1],
                         is_transpose=True, start=True, stop=True)
        spanTr = pp.tile([BL, 1], F32R, tag="spanTr")
        nc.scalar.copy(spanTr[:], stp[:])
        valid_t = pp.tile([1, R], F32, tag="valid")
        for n0 in range(0, R, 512):
            nn = min(512, R - n0)
            bps = psml.tile([1, 512], F32, tag="psml", name="bps")
            nc.tensor.matmul(bps[:, 0:nn], spanTr[:], onehot_t[:, n0:n0 + nn],
                             start=True, stop=True)
            nc.vector.tensor_tensor(valid_t[:, n0:n0 + nn], bps[:, 0:nn],
                                    c2_t[:, n0:n0 + nn], ALU.is_gt)
        fs_t = pp.tile([1, R], F32, tag="fs")
        nc.vector.tensor_tensor(fs_t[:], rel_t[:], dec_t[:], ALU.mult)
        nc.vector.tensor_tensor(fs_t[:], fs_t[:], mask_t[:], ALU.mult)
        nc.vector.tensor_scalar(fs_t[:], fs_t[:], float(hs_mean), None, ALU.mult)
        nc.vector.tensor_tensor(fs_t[:], fs_t[:], valid_t[:], ALU.mult)
        total_t = pp.tile([1, BL], F32, tag="total")
        nc.vector.tensor_reduce(total_t[:],
                                fs_t[:].rearrange("p (b s) -> p b s", b=BL),
                                AXL.X, ALU.add)
        den_t = pp.tile([1, BL], F32, tag="den")
        nc.vector.tensor_scalar(den_t[:], total_t[:], float(regp), None, ALU.add)
        recip_t = pp.tile([1, BL], F32, tag="recip")
        nc.vector.reciprocal(recip_t[:], den_t[:])
        fsn_t = pp.tile([1, R], F32, tag="fsn")
        for b in range(BL):
            sl_ = slice(S_active * b, S_active * (b + 1))
            nc.vector.tensor_scalar(fsn_t[:, sl_], fs_t[:, sl_],
                                    recip_t[0:1, b:b + 1], None, ALU.mult)

        # ---------------- attn active rows -------------------------------
        for b in range(BL):
            nc.sync.dma_start(
                attn_out[S - S_active:S, b:b + 1].rearrange("s x -> (s x)")
                .rearrange("(o f) -> o f", o=1),
                fsn_t[:, S_active * b:S_active * (b + 1)])

        # ---------------- feats (block-diag packed fp32 matmul) ---------
        rtp = psml.tile([BL, 1], F32, tag="psml", name="rtp")
        nc.tensor.matmul(rtp[:], recip_t[0:1, 0:BL], ident_t[0:1, 0:1],
                         is_transpose=True, start=True, stop=True)
        recipT = pp.tile([BL, 1], F32, tag="recipT")
        nc.scalar.copy(recipT[:], rtp[:])
        psf = [pacc2.tile([BL, 512], F32, tag="pacc2", name="psf")
               for _ in range(2)]
        for q in range(NQ):
            fps = ptr.tile([128, 1], F32, tag="ptr", name="fps")
            nc.tensor.matmul(fps[:], fs_t[:, 128 * q:128 * (q + 1)],
                             ident_t[0:1, 0:1], is_transpose=True,
                             start=True, stop=True)
            fsT = cy.tile([128, 1], F32, tag="fsT", name="fsT")
            nc.scalar.copy(fsT[:], fps[:])
            bd = cy.tile([128, BL], F32, tag="bd", name="bd")
            nc.vector.tensor_scalar(bd[:], bdm_t[q][:], fsT[:], None, ALU.mult)
            for h in range(2):
                nc.tensor.matmul(psf[h][:], bd[:],
                                 lx_t[q][:, 512 * h:512 * (h + 1)],
                                 start=(q == 0), stop=(q == NQ - 1))
        feats_t = pp.tile([BL, E], F32, tag="feats")
        for h in range(2):
            nc.scalar.activation(feats_t[:, 512 * h:512 * (h + 1)], psf[h][:],
                                 AF.Copy, scale=recipT[0:BL, 0:1])
        nc.sync.dma_start(feats_out[:], feats_t[:])

    nc.compile()
    return nc


def _host_consts(S_active, td, thr, b3=0.0):
    """Host-side constant tensors for one core (b-major rows). The mask is
    evaluated on the pre-bias logit z (PSUM), so rs_b3 is folded into cthr."""
    R = BL * S_active
    s_lo = S - S_active
    s_idx = np.arange(S)
    decay_full = (np.float64(td) ** (S - 1 - s_idx)).astype(np.float32)
    s_of_r = s_lo + (np.arange(R) % S_active)
    dec_r = decay_full[s_of_r]
    q = np.float64(thr) / dec_r.astype(np.float64)
    with np.errstate(divide="ignore", invalid="ignore"):
        logit = np.log(q / (1.0 - q))
    cthr = np.where(q >= 1.0, 1e30, np.where(q <= 0.0, -1e30,
                                              logit - np.float64(b3)))
    cthr = cthr.astype(np.float32)
    c2 = (S - 1 - s_of_r).astype(np.float32)
    b_of_r = np.arange(R) // S_active
    onehot = (b_of_r[None, :] == np.arange(BL)[:, None]).astype(np.float32)
    bdm = onehot.T.copy()
    ident = np.eye(128, dtype=np.float32)
    patt8 = (np.arange(128)[:, None] % BL == np.arange(BL)[None, :]).astype(
        np.float32)
    return dict(ident=ident, patt8=patt8, onehot=onehot, bdm=bdm,
                decay_r=dec_r, cthr_r=cthr, c2_r=c2)


def _uniform_fallback(npv, spans):
    """Exact outputs when no position can pass the threshold (total==0
    everywhere): attn = valid/span, feats accordingly."""
    lx = npv["long_x"].astype(np.float32)
    start = (S - spans).astype(np.int32)
    pos = np.arange(S)
    valid = (pos[:, None] >= start[None, :]).astype(np.float32)
    attn = valid / spans[None, :].astype(np.float32)
    feats = np.einsum("sb,sbe->be", attn, lx).astype(np.float32)
    return attn, feats, spans


def _numpy_full_reference(npv):
    """Exact float32 numpy replica of the reference (slow, safety net for
    near-dense threshold patterns that the sparse device kernel doesn't
    cover)."""
    from scipy.special import erf
    f32 = np.float32
    lx = npv["long_x"].astype(f32)
    ctx = npv["encoded_x"].astype(f32).mean(axis=0, dtype=f32)

    def gelu(x):
        return (0.5 * x * (1 + erf(x / np.sqrt(2)))).astype(f32)

    spans = _numpy_spans(npv)
    start = S - spans
    pos = np.arange(S)
    valid = (pos[:, None] >= start[None, :]).astype(f32)
    comb = np.concatenate([lx, np.broadcast_to(ctx[None], lx.shape)], axis=-1)
    r = gelu(comb.reshape(-1, 2 * E) @ npv["rs_w1"].astype(f32) +
             npv["rs_b1"].astype(f32))
    r = gelu(r @ npv["rs_w2"].astype(f32) + npv["rs_b2"].astype(f32))
    rel = 1 / (1 + np.exp(-(r @ npv["rs_w3"].astype(f32) +
                            npv["rs_b3"].astype(f32))))
    rel = rel[:, 0].reshape(S, B).astype(f32)
    td = f32(npv["temporal_decay"])
    decay = (np.float64(td) ** (S - 1 - pos)).astype(f32)
    fs = rel * decay[:, None]
    fs = fs * (fs > f32(npv["adaptive_threshold"]))
    fs = fs * f32(np.asarray(npv["head_scale"], np.float64).mean())
    fs = fs * valid
    total = fs.sum(axis=0, dtype=f32)
    normed = fs / (total[None, :] + f32(npv["attention_reg"]) + f32(1e-8))
    uniform = valid / spans[None, :].astype(f32)
    attn = np.where(total[None, :] > 0, normed, uniform).astype(f32)
    feats = np.einsum("sb,sbe->be", attn, lx).astype(f32)
    return attn, feats, spans


def _numpy_spans(npv):
    from scipy.special import erf
    ctx = npv["encoded_x"].astype(np.float64).mean(axis=0)

    def gelu(x):
        return 0.5 * x * (1 + erf(x / np.sqrt(2)))

    h = gelu(ctx @ npv["sp_w1"].astype(np.float64) + npv["sp_b1"])
    h = gelu(h @ npv["sp_w2"].astype(np.float64) + npv["sp_b2"])
    sl = 1 / (1 + np.exp(-(h @ npv["sp_w3"].astype(np.float64) + npv["sp_b3"])))
    sl = sl[:, 0].astype(np.float32)
    spans = np.minimum((sl * (MAX_SPAN - MIN_SPAN) + MIN_SPAN).astype(np.int32),
                       S)
    return spans


def kernel(**inputs):
    npv = {k: np.asarray(v) for k, v in inputs.items()}
    f32 = np.float32
    td = float(f32(npv["temporal_decay"]))
    thr = float(f32(npv["adaptive_threshold"]))
    reg = float(f32(npv["attention_reg"]))
    hs_mean = float(f32(np.asarray(npv["head_scale"], np.float64).mean()))
    regp = float(f32(reg) + f32(1e-8))

    decay_full = (np.float64(td) ** (S - 1 - np.arange(S))).astype(np.float32)
    passing = decay_full > f32(thr)
    if not passing.any():
        return _uniform_fallback(npv, _numpy_spans(npv))
    s_min = int(np.nonzero(passing)[0].min())
    S_active = min(S, int(np.ceil((S - s_min + 8) / 32.0)) * 32)
    if S_active > 128:
        return _numpy_full_reference(npv)
    s_lo = S - S_active

    nc = _build(S_active, hs_mean, regp)
    consts = _host_consts(S_active, td, thr,
                          float(np.float32(npv['rs_b3'].reshape(-1)[0])))

    import ml_dtypes
    bf16 = ml_dtypes.bfloat16

    def c(a, dt=np.float32):
        return np.ascontiguousarray(np.asarray(a, dtype=np.float32).astype(dt))

    rw1 = np.asarray(npv["rs_w1"], np.float32)
    shared = dict(
        w1x=c(rw1[:E]), w1c=c(rw1[E:], bf16),
        rs_w2=c(npv["rs_w2"]), rs_w3=c(npv["rs_w3"]),
        rs_b1=c(npv["rs_b1"]), rs_b2=c(npv["rs_b2"]), rs_b3=c(npv["rs_b3"]),
        sw1=c(npv["sp_w1"], bf16), sw2=c(npv["sp_w2"], bf16),
        sw3=c(npv["sp_w3"], bf16),
        sp_b1=c(npv["sp_b1"]), sp_b2=c(npv["sp_b2"]), sp_b3=c(npv["sp_b3"]),
        **{k: (c(v, bf16) if k == "patt8" else c(v)) for k, v in consts.items()})
    in_maps = []
    for i in range(NCORES):
        bs = slice(i * BL, (i + 1) * BL)
        in_maps.append(dict(
            lx=c(npv["long_x"][s_lo:, bs, :].transpose(1, 0, 2)),
            encb=c(npv["encoded_x"][:, bs, :], bf16),
            **shared))

    res = run_bass_kernel_spmd(nc, in_maps, core_ids=list(range(NCORES)))
    global LAST_RESULT
    LAST_RESULT = res
    rs = res.results
    attn = np.concatenate([r["attn_out"] for r in rs], axis=1)
    feats = np.concatenate([r["feats_out"] for r in rs], axis=0)
    spans = np.concatenate([r["spans_out"] for r in rs], axis=0).astype(np.int32)
    return attn, feats, spans


if __name__ == "__main__":
    rng = np.random.default_rng(0)
    print("smoke test requires reference inputs; use test.py")


# revision 17
# speedup vs baseline: 2.5806x; 1.0054x over previous
"""AdaptiveSpanAttention TRN2 kernel: 8-way batch-parallel Bass/Tile kernel.

Structure exploited: fs = sigmoid(z)*decay_s can only exceed adaptive_threshold
where decay_s > threshold (since sigmoid < 1). For the reference scalars
(decay 0.95, thr 0.3) that is only the last 24 of 1024 positions, so the big
relevance MLP runs only on that row suffix. The threshold mask is evaluated on
the pre-sigmoid logit z against host-precomputed c_s = logit(thr/decay_s),
which is exactly monotone-equivalent and immune to LUT rounding at the
discontinuity. All matmuls run in true fp32 on the PE (4 cyc/row).

Sharding: batch dim (axis 1 of long_x/encoded_x) across the 8 NeuronCores,
weights replicated, outputs concatenated on host (pure data parallel).
"""
import sys

import numpy as np

sys.path.insert(0, "/opt/trn_rl_repo")

import concourse.tile as tile  # noqa: E402
from concourse import bacc, mybir  # noqa: E402
from concourse.bass_utils import run_bass_kernel_spmd  # noqa: E402

F32 = mybir.dt.float32
F32R = mybir.dt.float32r
BF16 = mybir.dt.bfloat16
I32 = mybir.dt.int32
AF = mybir.ActivationFunctionType
ALU = mybir.AluOpType
AXL = mybir.AxisListType

S, B, E, H = 1024, 64, 1024, 16
NCORES = 8
BL = B // NCORES  # 8 batches per core
MAX_SPAN, MIN_SPAN = 1024, 8
LAST_RESULT = None


def _build(S_active, hs_mean, regp):
    R = BL * S_active
    NQ = R // 128
    RB = min(R, 512)
    NRB = R // RB
    s_lo = S - S_active
    ZROWS = s_lo * BL

    nc = bacc.Bacc("TRN2", target_bir_lowering=False, debug=False,
                   num_devices=NCORES)

    def inp(name, shape, dt=F32):
        return nc.declare_dram_parameter(name, shape, dt, isOutput=False)

    lx = inp("lx", [BL, S_active, E])
    encb = inp("encb", [256, BL, E], BF16)
    w1x = inp("w1x", [E, E], F32R)          # rs_w1 top half (f32 bytes)
    w1c = inp("w1c", [E, E], BF16)          # rs_w1 bottom half, bf16
    rs_w2 = inp("rs_w2", [E, E // 2], F32R)
    rs_w3 = inp("rs_w3", [E // 2, 1], F32R)
    rs_b1 = inp("rs_b1", [E])
    rs_b2 = inp("rs_b2", [E // 2])
    rs_b3 = inp("rs_b3", [1])
    sw1 = inp("sw1", [E, E // 2], BF16)
    sw2 = inp("sw2", [E // 2, E // 4], BF16)
    sw3 = inp("sw3", [E // 4, 1], BF16)
    sp_b1 = inp("sp_b1", [E // 2])
    sp_b2 = inp("sp_b2", [E // 4])
    sp_b3 = inp("sp_b3", [1])
    ident = inp("ident", [128, 128])
    patt8 = inp("patt8", [128, BL], BF16)
    onehot = inp("onehot", [BL, R], F32R)
    bdm = inp("bdm", [R, BL])
    decay_r = inp("decay_r", [R])
    cthr_r = inp("cthr_r", [R])
    c2_r = inp("c2_r", [R])

    attn_out = nc.declare_dram_parameter("attn_out", [S, BL], F32, isOutput=True)
    feats_out = nc.declare_dram_parameter("feats_out", [BL, E], F32, isOutput=True)
    spans_out = nc.declare_dram_parameter("spans_out", [BL], I32, isOutput=True)

    enc_flat = encb.rearrange("s b e -> (s b) e")   # [2048, E]
    lx_flat = lx.rearrange("b s e -> (b s) e")      # [R, E]

    with tile.TileContext(nc) as tc, \
            tc.tile_pool(name="wts", bufs=1) as wp, \
            tc.tile_pool(name="per", bufs=1) as pp, \
            tc.tile_pool(name="acts", bufs=1) as ap, \
            tc.tile_pool(name="encs", bufs=16) as ep, \
            tc.tile_pool(name="cyc", bufs=3) as cy, \
            tc.tile_pool(name="pacc1", bufs=2, space="PSUM") as pacc1, \
            tc.tile_pool(name="pacc2", bufs=4, space="PSUM") as pacc2, \
            tc.tile_pool(name="ptr", bufs=1, space="PSUM") as ptr, \
            tc.tile_pool(name="psml", bufs=1, space="PSUM") as psml:

        # ---------------- DMAs (priority order) --------------------------
        ident_t = pp.tile([128, 128], F32, tag="ident")
        nc.sync.dma_start(ident_t[:], ident[:])
        patt8_t = pp.tile([128, BL], BF16, tag="patt8")
        nc.sync.dma_start(patt8_t[:], patt8[:])
        lx_t = [pp.tile([128, E], F32, tag=f"lx{q}", name=f"lx{q}")
                for q in range(NQ)]
        for q in range(NQ):
            nc.sync.dma_start(lx_t[q][:], lx_flat[128 * q:128 * (q + 1), :])
        rb1_t = pp.tile([128, 8], F32, tag="rb1")
        nc.sync.dma_start(rb1_t[:], rs_b1[:].rearrange("(m p) -> p m", p=128))
        rb2_t = pp.tile([128, 4], F32, tag="rb2")
        nc.sync.dma_start(rb2_t[:], rs_b2[:].rearrange("(m p) -> p m", p=128))
        rb3_t = pp.tile([1, 1], F32, tag="rb3")
        nc.sync.dma_start(rb3_t[:], rs_b3[:].rearrange("(p o) -> p o", p=1))
        sb1_t = pp.tile([128, 4], F32, tag="sb1")
        nc.sync.dma_start(sb1_t[:], sp_b1[:].rearrange("(m p) -> p m", p=128))
        sb2_t = pp.tile([128, 2], F32, tag="sb2")
        nc.sync.dma_start(sb2_t[:], sp_b2[:].rearrange("(m p) -> p m", p=128))
        sb3_t = pp.tile([1, 1], F32, tag="sb3")
        nc.sync.dma_start(sb3_t[:], sp_b3[:].rearrange("(p o) -> p o", p=1))

        enc_tiles = []
        for t in range(16):
            et = ep.tile([128, E], BF16, tag="enc", name="enc_t")
            nc.sync.dma_start(et[:], enc_flat[128 * t:128 * (t + 1), :])
            enc_tiles.append(et)

        wc_t = [wp.tile([128, E], BF16, tag=f"wc{k}", name=f"wc{k}")
                for k in range(8)]
        for k in range(8):
            nc.sync.dma_start(wc_t[k][:], w1c[128 * k:128 * (k + 1), :])
        onehot_t = pp.tile([BL, R], F32R, tag="onehot")
        nc.sync.dma_start(onehot_t[:], onehot[:])
        sw1_t = [wp.tile([128, E // 2], BF16, tag=f"sw1_{k}", name=f"sw1_{k}")
                 for k in range(8)]
        for k in range(8):
            nc.sync.dma_start(sw1_t[k][:], sw1[128 * k:128 * (k + 1), :])
        sw2_t = [wp.tile([128, E // 4], BF16, tag=f"sw2_{k}", name=f"sw2_{k}")
                 for k in range(4)]
        for k in range(4):
            nc.sync.dma_start(sw2_t[k][:], sw2[128 * k:128 * (k + 1), :])
        sw3_t = wp.tile([128, 2], BF16, tag="sw3")
        nc.sync.dma_start(sw3_t[:], sw3[:].rearrange("(k p) o -> p (k o)", p=128))

        wm_t = [wp.tile([128, E], F32R, tag=f"wm{k}", name=f"wm{k}")
                for k in range(8)]
        w2_t = [wp.tile([128, E // 2], F32R, tag=f"w2_{k}", name=f"w2_{k}")
                for k in range(8)]
        for k in range(8):
            nc.sync.dma_start(wm_t[k][:], w1x[128 * k:128 * (k + 1), :])
        for k in range(8):
            nc.sync.dma_start(w2_t[k][:], rs_w2[128 * k:128 * (k + 1), :])
        w3_t = wp.tile([128, 4], F32R, tag="w3")
        nc.sync.dma_start(w3_t[:], rs_w3[:].rearrange("(k p) o -> p (k o)", p=128))

        bdm_t = [pp.tile([128, BL], F32, tag=f"bdm{q}", name=f"bdm{q}")
                 for q in range(NQ)]
        for q in range(NQ):
            nc.sync.dma_start(bdm_t[q][:], bdm[128 * q:128 * (q + 1), :])
        dec_t = pp.tile([1, R], F32, tag="dec")
        nc.sync.dma_start(dec_t[:], decay_r[:].rearrange("(o r) -> o r", o=1))
        cthr_t = pp.tile([1, R], F32, tag="cthr")
        nc.sync.dma_start(cthr_t[:], cthr_r[:].rearrange("(o r) -> o r", o=1))
        c2_t = pp.tile([1, R], F32, tag="c2")
        nc.sync.dma_start(c2_t[:], c2_r[:].rearrange("(o r) -> o r", o=1))

        # attn zero rows: independent of everything -> emit early
        if ZROWS > 0:
            zcols = ZROWS // 128
            zt = pp.tile([128, zcols], F32, tag="zt")
            nc.vector.memset(zt[:], 0.0)
            nc.sync.dma_start(
                attn_out[0:S - S_active, :].rearrange("s b -> (s b)")
                .rearrange("(p f) -> p f", p=128), zt[:])

        # ---------------- PE: A0 transposes (lx^T), fp32 -> f32r --------
        a0 = [ap.tile([128, R], F32R, tag=f"a0_{c}", name=f"a0_{c}")
              for c in range(8)]
        for c in range(8):
            for q in range(NQ):
                tps = ptr.tile([128, 128], F32, tag="ptr", name="tps")
                nc.tensor.matmul(tps[:], lx_t[q][:, 128 * c:128 * (c + 1)],
                                 ident_t[:, :], is_transpose=True,
                                 start=True, stop=True)
                nc.scalar.copy(a0[c][:, 128 * q:128 * (q + 1)], tps[:])

        # ---------------- ctx = mean_s(enc) on PE (bf16, chases enc DMA) -
        ctx_t = pp.tile([BL, E], F32, tag="ctx")
        cps = [pacc1.tile([BL, 512], F32, tag="pacc1", name="cps")
               for _ in range(2)]
        for t in range(16):
            for h in range(2):
                nc.tensor.matmul(cps[h][:], patt8_t[:],
                                 enc_tiles[t][:, 512 * h:512 * (h + 1)],
                                 start=(t == 0), stop=(t == 15))
        for h in range(2):
            nc.scalar.activation(ctx_t[:, 512 * h:512 * (h + 1)], cps[h][:],
                                 AF.Copy, scale=1.0 / 256.0)

        # ---------------- ctxT (bf16) ------------------------------------
        ctxTb = pp.tile([128, 8 * BL], BF16, tag="ctxTb")
        for c in range(8):
            tps2 = ptr.tile([128, BL], F32, tag="ptr", name="tps2")
            nc.tensor.matmul(tps2[:], ctx_t[0:BL, 128 * c:128 * (c + 1)],
                             ident_t[0:BL, 0:BL], is_transpose=True,
                             start=True, stop=True)
            nc.scalar.copy(ctxTb[:, BL * c:BL * (c + 1)], tps2[:])

        # ---------------- c1 = ctx @ rs_w1[E:]  (bf16 -> f32r) ----------
        c1n_t = pp.tile([BL, E], F32R, tag="c1n")
        for m in range(8):
            c1ps = ptr.tile([BL, 128], F32, tag="ptr", name="c1ps")
            for k in range(8):
                nc.tensor.matmul(c1ps[:], ctxTb[:, BL * k:BL * (k + 1)],
                                 wc_t[k][:, 128 * m:128 * (m + 1)],
                                 start=(k == 0), stop=(k == 7))
            nc.scalar.copy(c1n_t[:, 128 * m:128 * (m + 1)], c1ps[:])

        # ---------------- span predictor MLP (bf16), early ---------------
        h1b = pp.tile([128, 4 * BL], BF16, tag="h1b")
        for m in range(4):
            hps = ptr.tile([128, BL], F32, tag="ptr", name="hps")
            for k in range(8):
                nc.tensor.matmul(hps[:], sw1_t[k][:, 128 * m:128 * (m + 1)],
                                 ctxTb[:, BL * k:BL * (k + 1)],
                                 start=(k == 0), stop=(k == 7))
            nc.scalar.activation(h1b[:, BL * m:BL * (m + 1)], hps[:], AF.Gelu,
                                 bias=sb1_t[:, m:m + 1])
        h2b = pp.tile([128, 2 * BL], BF16, tag="h2b")
        for m in range(2):
            hps2 = ptr.tile([128, BL], F32, tag="ptr", name="hps2")
            for k in range(4):
                nc.tensor.matmul(hps2[:], sw2_t[k][:, 128 * m:128 * (m + 1)],
                                 h1b[:, BL * k:BL * (k + 1)],
                                 start=(k == 0), stop=(k == 3))
            nc.scalar.activation(h2b[:, BL * m:BL * (m + 1)], hps2[:], AF.Gelu,
                                 bias=sb2_t[:, m:m + 1])
        ups = psml.tile([1, BL], F32, tag="psml", name="ups")
        for k in range(2):
            nc.tensor.matmul(ups[:], sw3_t[:, k:k + 1],
                             h2b[:, BL * k:BL * (k + 1)],
                             start=(k == 0), stop=(k == 1))
        u_sb = pp.tile([1, BL], F32, tag="u_sb")
        nc.scalar.copy(u_sb[:], ups[:])

        # ---------------- main MLP: k-outer m-pairs, pipelined L2 (f32r) -
        mask_t = pp.tile([1, R], F32, tag="mask")
        rel_t = pp.tile([1, R], F32, tag="rel")
        for rb in range(NRB):
            r0 = rb * RB
            a1 = [ap.tile([128, RB], F32R, tag=f"a1_{m}", name=f"a1_{m}")
                  for m in range(8)]
            ps2 = [pacc2.tile([128, RB], F32, tag="pacc2", name="ps2")
                   for _ in range(4)]

            def emit_l2(mm):
                for m2 in range(4):
                    nc.tensor.matmul(ps2[m2][:],
                                     w2_t[mm][:, 128 * m2:128 * (m2 + 1)],
                                     a1[mm][:], start=(mm == 0), stop=(mm == 7))

            for p in range(4):
                mA, mB = 2 * p, 2 * p + 1
                psA = pacc1.tile([128, RB], F32, tag="pacc1", name="ps1a")
                psB = pacc1.tile([128, RB], F32, tag="pacc1", name="ps1b")
                for k in range(8):
                    nc.tensor.matmul(psA[:], wm_t[k][:, 128 * mA:128 * (mA + 1)],
                                     a0[k][:, r0:r0 + RB],
                                     start=(k == 0), stop=False)
                    nc.tensor.matmul(psB[:], wm_t[k][:, 128 * mB:128 * (mB + 1)],
                                     a0[k][:, r0:r0 + RB],
                                     start=(k == 0), stop=False)
                nc.tensor.matmul(psA[:], c1n_t[0:BL, 128 * mA:128 * (mA + 1)],
                                 onehot_t[0:BL, r0:r0 + RB],
                                 start=False, stop=True)
                nc.scalar.activation(a1[mA][:], psA[:], AF.Gelu,
                                     bias=rb1_t[:, mA:mA + 1])
                nc.tensor.matmul(psB[:], c1n_t[0:BL, 128 * mB:128 * (mB + 1)],
                                 onehot_t[0:BL, r0:r0 + RB],
                                 start=False, stop=True)
                nc.scalar.activation(a1[mB][:], psB[:], AF.Gelu,
                                     bias=rb1_t[:, mB:mB + 1])
                if p >= 1:
                    emit_l2(2 * p - 2)
                    emit_l2(2 * p - 1)
            emit_l2(6)
            emit_l2(7)

            a2 = [ap.tile([128, RB], F32R, tag=f"a2_{m2}", name=f"a2_{m2}")
                  for m2 in range(4)]
            for m2 in range(4):
                nc.scalar.activation(a2[m2][:], ps2[m2][:], AF.Gelu,
                                     bias=rb2_t[:, m2:m2 + 1])
            zps = psml.tile([1, RB], F32, tag="psml", name="zps")
            for m2 in range(4):
                nc.tensor.matmul(zps[:], w3_t[:, m2:m2 + 1], a2[m2][:],
                                 start=(m2 == 0), stop=(m2 == 3))
            # mask on pre-sigmoid logit (cthr_r has rs_b3 folded in host-side)
            nc.vector.tensor_tensor(mask_t[:, r0:r0 + RB], zps[:],
                                    cthr_t[:, r0:r0 + RB], ALU.is_gt)
            nc.scalar.activation(rel_t[:, r0:r0 + RB], zps[:], AF.Sigmoid,
                                 bias=rb3_t[0:1, 0:1])

        # ---------------- spans (sigmoid deferred to share table set) ----
        sl_t = pp.tile([1, BL], F32, tag="sl")
        nc.scalar.activation(sl_t[:], u_sb[:], AF.Sigmoid, bias=sb3_t[0:1, 0:1])
        tspan = pp.tile([1, BL], F32, tag="tspan")
        nc.vector.tensor_scalar(tspan[:], sl_t[:], float(MAX_SPAN - MIN_SPAN),
                                float(MIN_SPAN), ALU.mult, ALU.add)
        tsh = pp.tile([1, BL], F32, tag="tsh")
        nc.vector.tensor_scalar(tsh[:], tspan[:], 0.5, None, ALU.subtract)
        spans_i = pp.tile([1, BL], I32, tag="spans_i")
        nc.vector.tensor_copy(spans_i[:], tsh[:])
        span_f = pp.tile([1, BL], F32, tag="span_f")
        nc.vector.tensor_copy(span_f[:], spans_i[:])
        nc.sync.dma_start(spans_out[:].rearrange("(o b) -> o b", o=1), spans_i[:])

        # ---------------- scores (DVE + tiny PE broadcasts) --------------
        stp = psml.tile([BL, 1], F32, tag="psml", name="stp")
        nc.tensor.matmul(stp[:], span_f[0:1, 0:BL], ident_t[0:1, 0:1],
                         is_transpose=True, start=True, stop=True)
        spanTr = pp.tile([BL, 1], F32R, tag="spanTr")
        nc.scalar.copy(spanTr[:], stp[:])
        valid_t = pp.tile([1, R], F32, tag="valid")
        for n0 in range(0, R, 512):
            nn = min(512, R - n0)
            bps = psml.tile([1, 512], F32, tag="psml", name="bps")
            nc.tensor.matmul(bps[:, 0:nn], spanTr[:], onehot_t[:, n0:n0 + nn],
                             start=True, stop=True)
            nc.vector.tensor_tensor(valid_t[:, n0:n0 + nn], bps[:, 0:nn],
                                    c2_t[:, n0:n0 + nn], ALU.is_gt)
        fs_t = pp.tile([1, R], F32, tag="fs")
        nc.vector.tensor_tensor(fs_t[:], rel_t[:], dec_t[:], ALU.mult)
        nc.vector.tensor_tensor(fs_t[:], fs_t[:], mask_t[:], ALU.mult)
        nc.vector.tensor_scalar(fs_t[:], fs_t[:], float(hs_mean), None, ALU.mult)
        nc.vector.tensor_tensor(fs_t[:], fs_t[:], valid_t[:], ALU.mult)
        total_t = pp.tile([1, BL], F32, tag="total")
        nc.vector.tensor_reduce(total_t[:],
                                fs_t[:].rearrange("p (b s) -> p b s", b=BL),
                                AXL.X, ALU.add)
        den_t = pp.tile([1, BL], F32, tag="den")
        nc.vector.tensor_scalar(den_t[:], total_t[:], float(regp), None, ALU.add)
        recip_t = pp.tile([1, BL], F32, tag="recip")
        nc.vector.reciprocal(recip_t[:], den_t[:])
        fsn_t = pp.tile([1, R], F32, tag="fsn")
        for b in range(BL):
            sl_ = slice(S_active * b, S_active * (b + 1))
            nc.vector.tensor_scalar(fsn_t[:, sl_], fs_t[:, sl_],
                                    recip_t[0:1, b:b + 1], None, ALU.mult)

        # ---------------- attn active rows -------------------------------
        for b in range(BL):
            nc.sync.dma_start(
                attn_out[S - S_active:S, b:b + 1].rearrange("s x -> (s x)")
                .rearrange("(o f) -> o f", o=1),
                fsn_t[:, S_active * b:S_active * (b + 1)])

        # ---------------- feats (block-diag packed fp32 matmul) ---------
        rtp = psml.tile([BL, 1], F32, tag="psml", name="rtp")
        nc.tensor.matmul(rtp[:], recip_t[0:1, 0:BL], ident_t[0:1, 0:1],
                         is_transpose=True, start=True, stop=True)
        recipT = pp.tile([BL, 1], F32, tag="recipT")
        nc.scalar.copy(recipT[:], rtp[:])
        psf = [pacc2.tile([BL, 512], F32, tag="pacc2", name="psf")
               for _ in range(2)]
        for q in range(NQ):
            fps = ptr.tile([128, 1], F32, tag="ptr", name="fps")
            nc.tensor.matmul(fps[:], fs_t[:, 128 * q:128 * (q + 1)],
                             ident_t[0:1, 0:1], is_transpose=True,
                             start=True, stop=True)
            fsT = cy.tile([128, 1], F32, tag="fsT", name="fsT")
            nc.scalar.copy(fsT[:], fps[:])
            bd = cy.tile([128, BL], F32, tag="bd", name="bd")
            nc.vector.tensor_scalar(bd[:], bdm_t[q][:], fsT[:], None, ALU.mult)
            for h in range(2):
                nc.tensor.matmul(psf[h][:], bd[:],
                                 lx_t[q][:, 512 * h:512 * (h + 1)],
                                 start=(q == 0), stop=(q == NQ - 1))
        feats_t = pp.tile([BL, E], F32, tag="feats")
        for h in range(2):
            nc.scalar.activation(feats_t[:, 512 * h:512 * (h + 1)], psf[h][:],
                                 AF.Copy, scale=recipT[0:BL, 0:1])
        nc.sync.dma_start(feats_out[:], feats_t[:])

    nc.compile()
    return nc


def _host_consts(S_active, td, thr, b3=0.0):
    """Host-side constant tensors for one core (b-major rows). The mask is
    evaluated on the pre-bias logit z (PSUM), so rs_b3 is folded into cthr."""
    R = BL * S_active
    s_lo = S - S_active
    s_idx = np.arange(S)
    decay_full = (np.float64(td) ** (S - 1 - s_idx)).astype(np.float32)
    s_of_r = s_lo + (np.arange(R) % S_active)
    dec_r = decay_full[s_of_r]
    q = np.float64(thr) / dec_r.astype(np.float64)
    with np.errstate(divide="ignore", invalid="ignore"):
        logit = np.log(q / (1.0 - q))
    cthr = np.where(q >= 1.0, 1e30, np.where(q <= 0.0, -1e30,
                                              logit - np.float64(b3)))
    cthr = cthr.astype(np.float32)
    c2 = (S - 1 - s_of_r).astype(np.float32)
    b_of_r = np.arange(R) // S_active
    onehot = (b_of_r[None, :] == np.arange(BL)[:, None]).astype(np.float32)
    bdm = onehot.T.copy()
    ident = np.eye(128, dtype=np.float32)
    patt8 = (np.arange(128)[:, None] % BL == np.arange(BL)[None, :]).astype(
        np.float32)
    return dict(ident=ident, patt8=patt8, onehot=onehot, bdm=bdm,
                decay_r=dec_r, cthr_r=cthr, c2_r=c2)


def _uniform_fallback(npv, spans):
    """Exact outputs when no position can pass the threshold (total==0
    everywhere): attn = valid/span, feats accordingly."""
    lx = npv["long_x"].astype(np.float32)
    start = (S - spans).astype(np.int32)
    pos = np.arange(S)
    valid = (pos[:, None] >= start[None, :]).astype(np.float32)
    attn = valid / spans[None, :].astype(np.float32)
    feats = np.einsum("sb,sbe->be", attn, lx).astype(np.float32)
    return attn, feats, spans


def _numpy_full_reference(npv):
    """Exact float32 numpy replica of the reference (slow, safety net for
    near-dense threshold patterns that the sparse device kernel doesn't
    cover)."""
    from scipy.special import erf
    f32 = np.float32
    lx = npv["long_x"].astype(f32)
    ctx = npv["encoded_x"].astype(f32).mean(axis=0, dtype=f32)

    def gelu(x):
        return (0.5 * x * (1 + erf(x / np.sqrt(2)))).astype(f32)

    spans = _numpy_spans(npv)
    start = S - spans
    pos = np.arange(S)
    valid = (pos[:, None] >= start[None, :]).astype(f32)
    comb = np.concatenate([lx, np.broadcast_to(ctx[None], lx.shape)], axis=-1)
    r = gelu(comb.reshape(-1, 2 * E) @ npv["rs_w1"].astype(f32) +
             npv["rs_b1"].astype(f32))
    r = gelu(r @ npv["rs_w2"].astype(f32) + npv["rs_b2"].astype(f32))
    rel = 1 / (1 + np.exp(-(r @ npv["rs_w3"].astype(f32) +
                            npv["rs_b3"].astype(f32))))
    rel = rel[:, 0].reshape(S, B).astype(f32)
    td = f32(npv["temporal_decay"])
    decay = (np.float64(td) ** (S - 1 - pos)).astype(f32)
    fs = rel * decay[:, None]
    fs = fs * (fs > f32(npv["adaptive_threshold"]))
    fs = fs * f32(np.asarray(npv["head_scale"], np.float64).mean())
    fs = fs * valid
    total = fs.sum(axis=0, dtype=f32)
    normed = fs / (total[None, :] + f32(npv["attention_reg"]) + f32(1e-8))
    uniform = valid / spans[None, :].astype(f32)
    attn = np.where(total[None, :] > 0, normed, uniform).astype(f32)
    feats = np.einsum("sb,sbe->be", attn, lx).astype(f32)
    return attn, feats, spans


def _numpy_spans(npv):
    from scipy.special import erf
    ctx = npv["encoded_x"].astype(np.float64).mean(axis=0)

    def gelu(x):
        return 0.5 * x * (1 + erf(x / np.sqrt(2)))

    h = gelu(ctx @ npv["sp_w1"].astype(np.float64) + npv["sp_b1"])
    h = gelu(h @ npv["sp_w2"].astype(np.float64) + npv["sp_b2"])
    sl = 1 / (1 + np.exp(-(h @ npv["sp_w3"].astype(np.float64) + npv["sp_b3"])))
    sl = sl[:, 0].astype(np.float32)
    spans = np.minimum((sl * (MAX_SPAN - MIN_SPAN) + MIN_SPAN).astype(np.int32),
                       S)
    return spans


def kernel(**inputs):
    npv = {k: np.asarray(v) for k, v in inputs.items()}
    f32 = np.float32
    td = float(f32(npv["temporal_decay"]))
    thr = float(f32(npv["adaptive_threshold"]))
    reg = float(f32(npv["attention_reg"]))
    hs_mean = float(f32(np.asarray(npv["head_scale"], np.float64).mean()))
    regp = float(f32(reg) + f32(1e-8))

    decay_full = (np.float64(td) ** (S - 1 - np.arange(S))).astype(np.float32)
    passing = decay_full > f32(thr)
    if not passing.any():
        return _uniform_fallback(npv, _numpy_spans(npv))
    s_min = int(np.nonzero(passing)[0].min())
    S_active = min(S, int(np.ceil((S - s_min + 8) / 32.0)) * 32)
    if S_active > 128:
        return _numpy_full_reference(npv)
    s_lo = S - S_active

    nc = _build(S_active, hs_mean, regp)
    consts = _host_consts(S_active, td, thr,
                          float(np.float32(npv['rs_b3'].reshape(-1)[0])))

    import ml_dtypes
    bf16 = ml_dtypes.bfloat16

    def c(a, dt=np.float32):
        return np.ascontiguousarray(np.asarray(a, dtype=np.float32).astype(dt))

    rw1 = np.asarray(npv["rs_w1"], np.float32)
    shared = dict(
        w1x=c(rw1[:E]), w1c=c(rw1[E:], bf16),
        rs_w2=c(npv["rs_w2"]), rs_w3=c(npv["rs_w3"]),
        rs_b1=c(npv["rs_b1"]), rs_b2=c(npv["rs_b2"]), rs_b3=c(npv["rs_b3"]),
        sw1=c(npv["sp_w1"], bf16), sw2=c(npv["sp_w2"], bf16),
        sw3=c(npv["sp_w3"], bf16),
        sp_b1=c(npv["sp_b1"]), sp_b2=c(npv["sp_b2"]), sp_b3=c(npv["sp_b3"]),
        **{k: (c(v, bf16) if k == "patt8" else c(v)) for k, v in consts.items()})
    in_maps = []
    for i in range(NCORES):
        bs = slice(i * BL, (i + 1) * BL)
        in_maps.append(dict(
            lx=c(npv["long_x"][s_lo:, bs, :].transpose(1, 0, 2)),
            encb=c(npv["encoded_x"][:, bs, :], bf16),
            **shared))

    res = run_bass_kernel_spmd(nc, in_maps, core_ids=list(range(NCORES)))
    global LAST_RESULT
    LAST_RESULT = res
    rs = res.results
    attn = np.concatenate([r["attn_out"] for r in rs], axis=1)
    feats = np.concatenate([r["feats_out"] for r in rs], axis=0)
    spans = np.concatenate([r["spans_out"] for r in rs], axis=0).astype(np.int32)
    return attn, feats, spans


if __name__ == "__main__":
    rng = np.random.default_rng(0)
    print("smoke test requires reference inputs; use test.py")


# revision 19
# speedup vs baseline: 2.7674x; 1.0724x over previous
"""AdaptiveSpanAttention TRN2 kernel: 8-way batch-parallel Bass/Tile kernel.

Structure exploited: fs = sigmoid(z)*decay_s can only exceed adaptive_threshold
where decay_s > threshold (since sigmoid < 1). For the reference scalars
(decay 0.95, thr 0.3) that is only the last 24 of 1024 positions, so the big
relevance MLP runs only on that row suffix. The threshold mask is evaluated on
the pre-sigmoid logit z against host-precomputed c_s = logit(thr/decay_s),
which is exactly monotone-equivalent and immune to LUT rounding at the
discontinuity. All matmuls run in true fp32 on the PE (4 cyc/row).

Sharding: batch dim (axis 1 of long_x/encoded_x) across the 8 NeuronCores,
weights replicated, outputs concatenated on host (pure data parallel).
"""
import sys

import numpy as np

sys.path.insert(0, "/opt/trn_rl_repo")

import concourse.tile as tile  # noqa: E402
from concourse import bacc, mybir  # noqa: E402
from concourse.bass_utils import run_bass_kernel_spmd  # noqa: E402

F32 = mybir.dt.float32
F32R = mybir.dt.float32r
BF16 = mybir.dt.bfloat16
I32 = mybir.dt.int32
AF = mybir.ActivationFunctionType
ALU = mybir.AluOpType
AXL = mybir.AxisListType

S, B, E, H = 1024, 64, 1024, 16
NCORES = 8
BL = B // NCORES  # 8 batches per core
MAX_SPAN, MIN_SPAN = 1024, 8
LAST_RESULT = None


def _build(S_active, hs_mean, regp):
    R = BL * S_active
    NQ = R // 128
    RB = min(R, 512)
    NRB = R // RB
    s_lo = S - S_active
    ZROWS = s_lo * BL

    nc = bacc.Bacc("TRN2", target_bir_lowering=False, debug=False,
                   num_devices=NCORES)

    def inp(name, shape, dt=F32):
        return nc.declare_dram_parameter(name, shape, dt, isOutput=False)

    lx = inp("lx", [BL, S_active, E])
    encb = inp("encb", [256, BL, E], BF16)
    w1x = inp("w1x", [E, E], F32R)          # rs_w1 top half (f32 bytes)
    w1c = inp("w1c", [E, E], BF16)          # rs_w1 bottom half, bf16
    rs_w2 = inp("rs_w2", [E, E // 2], F32R)
    rs_w3 = inp("rs_w3", [E // 2, 1], F32R)
    rs_b1 = inp("rs_b1", [E])
    rs_b2 = inp("rs_b2", [E // 2])
    rs_b3 = inp("rs_b3", [1])
    sw1 = inp("sw1", [E, E // 2], BF16)
    sw2 = inp("sw2", [E // 2, E // 4], BF16)
    sw3 = inp("sw3", [E // 4, 1], BF16)
    sp_b1 = inp("sp_b1", [E // 2])
    sp_b2 = inp("sp_b2", [E // 4])
    sp_b3 = inp("sp_b3", [1])
    ident = inp("ident", [128, 128])
    patt8 = inp("patt8", [128, BL], BF16)
    onehot = inp("onehot", [BL, R], F32R)
    bdm = inp("bdm", [R, BL])
    decay_r = inp("decay_r", [R])
    cthr_r = inp("cthr_r", [R])
    c2_r = inp("c2_r", [R])

    attn_out = nc.declare_dram_parameter("attn_out", [S, BL], F32, isOutput=True)
    feats_out = nc.declare_dram_parameter("feats_out", [BL, E], F32, isOutput=True)
    spans_out = nc.declare_dram_parameter("spans_out", [BL], I32, isOutput=True)

    enc_flat = encb.rearrange("s b e -> (s b) e")   # [2048, E]
    lx_flat = lx.rearrange("b s e -> (b s) e")      # [R, E]

    with tile.TileContext(nc) as tc, \
            tc.tile_pool(name="wts", bufs=1) as wp, \
            tc.tile_pool(name="per", bufs=1) as pp, \
            tc.tile_pool(name="acts", bufs=1) as ap, \
            tc.tile_pool(name="encs", bufs=16) as ep, \
            tc.tile_pool(name="cyc", bufs=3) as cy, \
            tc.tile_pool(name="pacc1", bufs=2, space="PSUM") as pacc1, \
            tc.tile_pool(name="pacc2", bufs=4, space="PSUM") as pacc2, \
            tc.tile_pool(name="ptr", bufs=1, space="PSUM") as ptr, \
            tc.tile_pool(name="psml", bufs=1, space="PSUM") as psml:

        # ---------------- DMAs (priority order) --------------------------
        ident_t = pp.tile([128, 128], F32, tag="ident")
        nc.sync.dma_start(ident_t[:], ident[:])
        patt8_t = pp.tile([128, BL], BF16, tag="patt8")
        nc.sync.dma_start(patt8_t[:], patt8[:])
        lx_t = [pp.tile([128, E], F32, tag=f"lx{q}", name=f"lx{q}")
                for q in range(NQ)]
        for q in range(NQ):
            nc.sync.dma_start(lx_t[q][:], lx_flat[128 * q:128 * (q + 1), :])
        rb1_t = pp.tile([128, 8], F32, tag="rb1")
        nc.sync.dma_start(rb1_t[:], rs_b1[:].rearrange("(m p) -> p m", p=128))
        rb2_t = pp.tile([128, 4], F32, tag="rb2")
        nc.sync.dma_start(rb2_t[:], rs_b2[:].rearrange("(m p) -> p m", p=128))
        rb3_t = pp.tile([1, 1], F32, tag="rb3")
        nc.sync.dma_start(rb3_t[:], rs_b3[:].rearrange("(p o) -> p o", p=1))
        sb1_t = pp.tile([128, 4], F32, tag="sb1")
        nc.sync.dma_start(sb1_t[:], sp_b1[:].rearrange("(m p) -> p m", p=128))
        sb2_t = pp.tile([128, 2], F32, tag="sb2")
        nc.sync.dma_start(sb2_t[:], sp_b2[:].rearrange("(m p) -> p m", p=128))
        sb3_t = pp.tile([1, 1], F32, tag="sb3")
        nc.sync.dma_start(sb3_t[:], sp_b3[:].rearrange("(p o) -> p o", p=1))

        enc_tiles = []
        for t in range(16):
            et = ep.tile([128, E], BF16, tag="enc", name="enc_t")
            nc.sync.dma_start(et[:], enc_flat[128 * t:128 * (t + 1), :])
            enc_tiles.append(et)

        wc_t = [wp.tile([128, E], BF16, tag=f"wc{k}", name=f"wc{k}")
                for k in range(8)]
        for k in range(8):
            nc.sync.dma_start(wc_t[k][:], w1c[128 * k:128 * (k + 1), :])
        onehot_t = pp.tile([BL, R], F32R, tag="onehot")
        nc.sync.dma_start(onehot_t[:], onehot[:])
        wm_t = [wp.tile([128, E], F32R, tag=f"wm{k}", name=f"wm{k}")
                for k in range(8)]
        w2_t = [wp.tile([128, E // 2], F32R, tag=f"w2_{k}", name=f"w2_{k}")
                for k in range(8)]
        for k in range(8):
            nc.sync.dma_start(wm_t[k][:], w1x[128 * k:128 * (k + 1), :])
        for k in range(8):
            nc.sync.dma_start(w2_t[k][:], rs_w2[128 * k:128 * (k + 1), :])
        w3_t = wp.tile([128, 4], F32R, tag="w3")
        nc.sync.dma_start(w3_t[:], rs_w3[:].rearrange("(k p) o -> p (k o)", p=128))
        sw1_t = [wp.tile([128, E // 2], BF16, tag=f"sw1_{k}", name=f"sw1_{k}")
                 for k in range(8)]
        for k in range(8):
            nc.sync.dma_start(sw1_t[k][:], sw1[128 * k:128 * (k + 1), :])
        sw2_t = [wp.tile([128, E // 4], BF16, tag=f"sw2_{k}", name=f"sw2_{k}")
                 for k in range(4)]
        for k in range(4):
            nc.sync.dma_start(sw2_t[k][:], sw2[128 * k:128 * (k + 1), :])
        sw3_t = wp.tile([128, 2], BF16, tag="sw3")
        nc.sync.dma_start(sw3_t[:], sw3[:].rearrange("(k p) o -> p (k o)", p=128))


        bdm_t = [pp.tile([128, BL], F32, tag=f"bdm{q}", name=f"bdm{q}")
                 for q in range(NQ)]
        for q in range(NQ):
            nc.sync.dma_start(bdm_t[q][:], bdm[128 * q:128 * (q + 1), :])
        dec_t = pp.tile([1, R], F32, tag="dec")
        nc.sync.dma_start(dec_t[:], decay_r[:].rearrange("(o r) -> o r", o=1))
        cthr_t = pp.tile([1, R], F32, tag="cthr")
        nc.sync.dma_start(cthr_t[:], cthr_r[:].rearrange("(o r) -> o r", o=1))
        c2_t = pp.tile([1, R], F32, tag="c2")
        nc.sync.dma_start(c2_t[:], c2_r[:].rearrange("(o r) -> o r", o=1))

        # attn zero rows: independent of everything -> emit early
        if ZROWS > 0:
            zcols = ZROWS // 128
            zt = pp.tile([128, zcols], F32, tag="zt")
            nc.vector.memset(zt[:], 0.0)
            nc.sync.dma_start(
                attn_out[0:S - S_active, :].rearrange("s b -> (s b)")
                .rearrange("(p f) -> p f", p=128), zt[:])

        # ---------------- PE: A0 transposes (lx^T), fp32 -> f32r --------
        a0 = [ap.tile([128, R], F32R, tag=f"a0_{c}", name=f"a0_{c}")
              for c in range(8)]
        for c in range(8):
            for q in range(NQ):
                tps = ptr.tile([128, 128], F32, tag="ptr", name="tps")
                nc.tensor.matmul(tps[:], lx_t[q][:, 128 * c:128 * (c + 1)],
                                 ident_t[:, :], is_transpose=True,
                                 start=True, stop=True)
                nc.scalar.copy(a0[c][:, 128 * q:128 * (q + 1)], tps[:])

        # ---------------- ctx = mean_s(enc) on PE (bf16, chases enc DMA) -
        ctx_t = pp.tile([BL, E], F32, tag="ctx")
        cps = [pacc1.tile([BL, 512], F32, tag="pacc1", name="cps")
               for _ in range(2)]
        for t in range(16):
            for h in range(2):
                nc.tensor.matmul(cps[h][:], patt8_t[:],
                                 enc_tiles[t][:, 512 * h:512 * (h + 1)],
                                 start=(t == 0), stop=(t == 15))
        for h in range(2):
            nc.scalar.activation(ctx_t[:, 512 * h:512 * (h + 1)], cps[h][:],
                                 AF.Copy, scale=1.0 / 256.0)

        # ---------------- ctxT (bf16) ------------------------------------
        ctxTb = pp.tile([128, 8 * BL], BF16, tag="ctxTb")
        for c in range(8):
            tps2 = ptr.tile([128, BL], F32, tag="ptr", name="tps2")
            nc.tensor.matmul(tps2[:], ctx_t[0:BL, 128 * c:128 * (c + 1)],
                             ident_t[0:BL, 0:BL], is_transpose=True,
                             start=True, stop=True)
            nc.scalar.copy(ctxTb[:, BL * c:BL * (c + 1)], tps2[:])

        # ---------------- c1 = ctx @ rs_w1[E:]  (bf16 -> f32r) ----------
        c1n_t = pp.tile([BL, E], F32R, tag="c1n")
        for m in range(8):
            c1ps = ptr.tile([BL, 128], F32, tag="ptr", name="c1ps")
            for k in range(8):
                nc.tensor.matmul(c1ps[:], ctxTb[:, BL * k:BL * (k + 1)],
                                 wc_t[k][:, 128 * m:128 * (m + 1)],
                                 start=(k == 0), stop=(k == 7))
            nc.scalar.copy(c1n_t[:, 128 * m:128 * (m + 1)], c1ps[:])

        # ---------------- main MLP: L1 k-outer (6+2 psums), then L2 ------
        mask_t = pp.tile([1, R], F32, tag="mask")
        rel_t = pp.tile([1, R], F32, tag="rel")
        for rb in range(NRB):
            r0 = rb * RB
            a1 = [ap.tile([128, RB], F32R, tag=f"a1_{m}", name=f"a1_{m}")
                  for m in range(8)]
            # group A: m=0..5 share six psum banks, k-outer chasing wm DMA
            psA = [pacc1.tile([128, RB], F32, tag="pacc1", name="psA")
                   for _ in range(2)]
            psA += [pacc2.tile([128, RB], F32, tag="pacc2", name="psA2")
                    for _ in range(4)]
            for k in range(8):
                for m in range(6):
                    nc.tensor.matmul(psA[m][:],
                                     wm_t[k][:, 128 * m:128 * (m + 1)],
                                     a0[k][:, r0:r0 + RB],
                                     start=(k == 0), stop=False)
            for m in range(6):
                nc.tensor.matmul(psA[m][:], c1n_t[0:BL, 128 * m:128 * (m + 1)],
                                 onehot_t[0:BL, r0:r0 + RB],
                                 start=False, stop=True)
                nc.scalar.activation(a1[m][:], psA[m][:], AF.Gelu,
                                     bias=rb1_t[:, m:m + 1])
            # group B: m=6,7
            psB = [pacc1.tile([128, RB], F32, tag="pacc1", name="psB")
                   for _ in range(2)]
            for k in range(8):
                for j in range(2):
                    m = 6 + j
                    nc.tensor.matmul(psB[j][:],
                                     wm_t[k][:, 128 * m:128 * (m + 1)],
                                     a0[k][:, r0:r0 + RB],
                                     start=(k == 0), stop=False)
            for j in range(2):
                m = 6 + j
                nc.tensor.matmul(psB[j][:], c1n_t[0:BL, 128 * m:128 * (m + 1)],
                                 onehot_t[0:BL, r0:r0 + RB],
                                 start=False, stop=True)
                nc.scalar.activation(a1[m][:], psB[j][:], AF.Gelu,
                                     bias=rb1_t[:, m:m + 1])
            # deferred dense L2 pass (chases rs_w2 DMA)
            ps2 = [pacc2.tile([128, RB], F32, tag="pacc2", name="ps2")
                   for _ in range(4)]
            for mm in range(8):
                for m2 in range(4):
                    nc.tensor.matmul(ps2[m2][:],
                                     w2_t[mm][:, 128 * m2:128 * (m2 + 1)],
                                     a1[mm][:], start=(mm == 0), stop=(mm == 7))
            a2 = [ap.tile([128, RB], F32R, tag=f"a2_{m2}", name=f"a2_{m2}")
                  for m2 in range(4)]
            for m2 in range(4):
                nc.scalar.activation(a2[m2][:], ps2[m2][:], AF.Gelu,
                                     bias=rb2_t[:, m2:m2 + 1])
            zps = psml.tile([1, RB], F32, tag="psml", name="zps")
            for m2 in range(4):
                nc.tensor.matmul(zps[:], w3_t[:, m2:m2 + 1], a2[m2][:],
                                 start=(m2 == 0), stop=(m2 == 3))
            # mask on pre-sigmoid logit (cthr_r has rs_b3 folded in host-side)
            nc.vector.tensor_tensor(mask_t[:, r0:r0 + RB], zps[:],
                                    cthr_t[:, r0:r0 + RB], ALU.is_gt)
            nc.scalar.activation(rel_t[:, r0:r0 + RB], zps[:], AF.Sigmoid,
                                 bias=rb3_t[0:1, 0:1])

        # ---------------- span predictor MLP (bf16), early ---------------
        h1b = pp.tile([128, 4 * BL], BF16, tag="h1b")
        for m in range(4):
            hps = ptr.tile([128, BL], F32, tag="ptr", name="hps")
            for k in range(8):
                nc.tensor.matmul(hps[:], sw1_t[k][:, 128 * m:128 * (m + 1)],
                                 ctxTb[:, BL * k:BL * (k + 1)],
                                 start=(k == 0), stop=(k == 7))
            nc.scalar.activation(h1b[:, BL * m:BL * (m + 1)], hps[:], AF.Gelu,
                                 bias=sb1_t[:, m:m + 1])
        h2b = pp.tile([128, 2 * BL], BF16, tag="h2b")
        for m in range(2):
            hps2 = ptr.tile([128, BL], F32, tag="ptr", name="hps2")
            for k in range(4):
                nc.tensor.matmul(hps2[:], sw2_t[k][:, 128 * m:128 * (m + 1)],
                                 h1b[:, BL * k:BL * (k + 1)],
                                 start=(k == 0), stop=(k == 3))
            nc.scalar.activation(h2b[:, BL * m:BL * (m + 1)], hps2[:], AF.Gelu,
                                 bias=sb2_t[:, m:m + 1])
        ups = psml.tile([1, BL], F32, tag="psml", name="ups")
        for k in range(2):
            nc.tensor.matmul(ups[:], sw3_t[:, k:k + 1],
                             h2b[:, BL * k:BL * (k + 1)],
                             start=(k == 0), stop=(k == 1))
        u_sb = pp.tile([1, BL], F32, tag="u_sb")
        nc.scalar.copy(u_sb[:], ups[:])

        # ---------------- spans (sigmoid deferred to share table set) ----
        sl_t = pp.tile([1, BL], F32, tag="sl")
        nc.scalar.activation(sl_t[:], u_sb[:], AF.Sigmoid, bias=sb3_t[0:1, 0:1])
        tspan = pp.tile([1, BL], F32, tag="tspan")
        nc.vector.tensor_scalar(tspan[:], sl_t[:], float(MAX_SPAN - MIN_SPAN),
                                float(MIN_SPAN), ALU.mult, ALU.add)
        tsh = pp.tile([1, BL], F32, tag="tsh")
        nc.vector.tensor_scalar(tsh[:], tspan[:], 0.5, None, ALU.subtract)
        spans_i = pp.tile([1, BL], I32, tag="spans_i")
        nc.vector.tensor_copy(spans_i[:], tsh[:])
        span_f = pp.tile([1, BL], F32, tag="span_f")
        nc.vector.tensor_copy(span_f[:], spans_i[:])
        nc.sync.dma_start(spans_out[:].rearrange("(o b) -> o b", o=1), spans_i[:])

        # ---------------- scores (DVE + tiny PE broadcasts) --------------
        stp = psml.tile([BL, 1], F32, tag="psml", name="stp")
        nc.tensor.matmul(stp[:], span_f[0:1, 0:BL], ident_t[0:1, 0:1],
                         is_transpose=True, start=True, stop=True)
        spanTr = pp.tile([BL, 1], F32R, tag="spanTr")
        nc.scalar.copy(spanTr[:], stp[:])
        valid_t = pp.tile([1, R], F32, tag="valid")
        for n0 in range(0, R, 512):
            nn = min(512, R - n0)
            bps = psml.tile([1, 512], F32, tag="psml", name="bps")
            nc.tensor.matmul(bps[:, 0:nn], spanTr[:], onehot_t[:, n0:n0 + nn],
                             start=True, stop=True)
            nc.vector.tensor_tensor(valid_t[:, n0:n0 + nn], bps[:, 0:nn],
                                    c2_t[:, n0:n0 + nn], ALU.is_gt)
        fs_t = pp.tile([1, R], F32, tag="fs")
        nc.vector.tensor_tensor(fs_t[:], rel_t[:], dec_t[:], ALU.mult)
        nc.vector.tensor_tensor(fs_t[:], fs_t[:], mask_t[:], ALU.mult)
        nc.vector.tensor_scalar(fs_t[:], fs_t[:], float(hs_mean), None, ALU.mult)
        nc.vector.tensor_tensor(fs_t[:], fs_t[:], valid_t[:], ALU.mult)
        total_t = pp.tile([1, BL], F32, tag="total")
        nc.vector.tensor_reduce(total_t[:],
                                fs_t[:].rearrange("p (b s) -> p b s", b=BL),
                                AXL.X, ALU.add)
        den_t = pp.tile([1, BL], F32, tag="den")
        nc.vector.tensor_scalar(den_t[:], total_t[:], float(regp), None, ALU.add)
        recip_t = pp.tile([1, BL], F32, tag="recip")
        nc.vector.reciprocal(recip_t[:], den_t[:])
        fsn_t = pp.tile([1, R], F32, tag="fsn")
        for b in range(BL):
            sl_ = slice(S_active * b, S_active * (b + 1))
            nc.vector.tensor_scalar(fsn_t[:, sl_], fs_t[:, sl_],
                                    recip_t[0:1, b:b + 1], None, ALU.mult)

        # ---------------- attn active rows -------------------------------
        for b in range(BL):
            nc.sync.dma_start(
                attn_out[S - S_active:S, b:b + 1].rearrange("s x -> (s x)")
                .rearrange("(o f) -> o f", o=1),
                fsn_t[:, S_active * b:S_active * (b + 1)])

        # ---------------- feats (block-diag packed fp32 matmul) ---------
        rtp = psml.tile([BL, 1], F32, tag="psml", name="rtp")
        nc.tensor.matmul(rtp[:], recip_t[0:1, 0:BL], ident_t[0:1, 0:1],
                         is_transpose=True, start=True, stop=True)
        recipT = pp.tile([BL, 1], F32, tag="recipT")
        nc.scalar.copy(recipT[:], rtp[:])
        psf = [pacc2.tile([BL, 512], F32, tag="pacc2", name="psf")
               for _ in range(2)]
        for q in range(NQ):
            fps = ptr.tile([128, 1], F32, tag="ptr", name="fps")
            nc.tensor.matmul(fps[:], fs_t[:, 128 * q:128 * (q + 1)],
                             ident_t[0:1, 0:1], is_transpose=True,
                             start=True, stop=True)
            fsT = cy.tile([128, 1], F32, tag="fsT", name="fsT")
            nc.scalar.copy(fsT[:], fps[:])
            bd = cy.tile([128, BL], F32, tag="bd", name="bd")
            nc.vector.tensor_scalar(bd[:], bdm_t[q][:], fsT[:], None, ALU.mult)
            for h in range(2):
                nc.tensor.matmul(psf[h][:], bd[:],
                                 lx_t[q][:, 512 * h:512 * (h + 1)],
                                 start=(q == 0), stop=(q == NQ - 1))
        feats_t = pp.tile([BL, E], F32, tag="feats")
        for h in range(2):
            nc.scalar.activation(feats_t[:, 512 * h:512 * (h + 1)], psf[h][:],
                                 AF.Copy, scale=recipT[0:BL, 0:1])
        nc.sync.dma_start(feats_out[:], feats_t[:])

    nc.compile()
    return nc


def _host_consts(S_active, td, thr, b3=0.0):
    """Host-side constant tensors for one core (b-major rows). The mask is
    evaluated on the pre-bias logit z (PSUM), so rs_b3 is folded into cthr."""
    R = BL * S_active
    s_lo = S - S_active
    s_idx = np.arange(S)
    decay_full = (np.float64(td) ** (S - 1 - s_idx)).astype(np.float32)
    s_of_r = s_lo + (np.arange(R) % S_active)
    dec_r = decay_full[s_of_r]
    q = np.float64(thr) / dec_r.astype(np.float64)
    with np.errstate(divide="ignore", invalid="ignore"):
        logit = np.log(q / (1.0 - q))
    cthr = np.where(q >= 1.0, 1e30, np.where(q <= 0.0, -1e30,
                                              logit - np.float64(b3)))
    cthr = cthr.astype(np.float32)
    c2 = (S - 1 - s_of_r).astype(np.float32)
    b_of_r = np.arange(R) // S_active
    onehot = (b_of_r[None, :] == np.arange(BL)[:, None]).astype(np.float32)
    bdm = onehot.T.copy()
    ident = np.eye(128, dtype=np.float32)
    patt8 = (np.arange(128)[:, None] % BL == np.arange(BL)[None, :]).astype(
        np.float32)
    return dict(ident=ident, patt8=patt8, onehot=onehot, bdm=bdm,
                decay_r=dec_r, cthr_r=cthr, c2_r=c2)


def _uniform_fallback(npv, spans):
    """Exact outputs when no position can pass the threshold (total==0
    everywhere): attn = valid/span, feats accordingly."""
    lx = npv["long_x"].astype(np.float32)
    start = (S - spans).astype(np.int32)
    pos = np.arange(S)
    valid = (pos[:, None] >= start[None, :]).astype(np.float32)
    attn = valid / spans[None, :].astype(np.float32)
    feats = np.einsum("sb,sbe->be", attn, lx).astype(np.float32)
    return attn, feats, spans


def _numpy_full_reference(npv):
    """Exact float32 numpy replica of the reference (slow, safety net for
    near-dense threshold patterns that the sparse device kernel doesn't
    cover)."""
    from scipy.special import erf
    f32 = np.float32
    lx = npv["long_x"].astype(f32)
    ctx = npv["encoded_x"].astype(f32).mean(axis=0, dtype=f32)

    def gelu(x):
        return (0.5 * x * (1 + erf(x / np.sqrt(2)))).astype(f32)

    spans = _numpy_spans(npv)
    start = S - spans
    pos = np.arange(S)
    valid = (pos[:, None] >= start[None, :]).astype(f32)
    comb = np.concatenate([lx, np.broadcast_to(ctx[None], lx.shape)], axis=-1)
    r = gelu(comb.reshape(-1, 2 * E) @ npv["rs_w1"].astype(f32) +
             npv["rs_b1"].astype(f32))
    r = gelu(r @ npv["rs_w2"].astype(f32) + npv["rs_b2"].astype(f32))
    rel = 1 / (1 + np.exp(-(r @ npv["rs_w3"].astype(f32) +
                            npv["rs_b3"].astype(f32))))
    rel = rel[:, 0].reshape(S, B).astype(f32)
    td = f32(npv["temporal_decay"])
    decay = (np.float64(td) ** (S - 1 - pos)).astype(f32)
    fs = rel * decay[:, None]
    fs = fs * (fs > f32(npv["adaptive_threshold"]))
    fs = fs * f32(np.asarray(npv["head_scale"], np.float64).mean())
    fs = fs * valid
    total = fs.sum(axis=0, dtype=f32)
    normed = fs / (total[None, :] + f32(npv["attention_reg"]) + f32(1e-8))
    uniform = valid / spans[None, :].astype(f32)
    attn = np.where(total[None, :] > 0, normed, uniform).astype(f32)
    feats = np.einsum("sb,sbe->be", attn, lx).astype(f32)
    return attn, feats, spans


def _numpy_spans(npv):
    from scipy.special import erf
    ctx = npv["encoded_x"].astype(np.float64).mean(axis=0)

    def gelu(x):
        return 0.5 * x * (1 + erf(x / np.sqrt(2)))

    h = gelu(ctx @ npv["sp_w1"].astype(np.float64) + npv["sp_b1"])
    h = gelu(h @ npv["sp_w2"].astype(np.float64) + npv["sp_b2"])
    sl = 1 / (1 + np.exp(-(h @ npv["sp_w3"].astype(np.float64) + npv["sp_b3"])))
    sl = sl[:, 0].astype(np.float32)
    spans = np.minimum((sl * (MAX_SPAN - MIN_SPAN) + MIN_SPAN).astype(np.int32),
                       S)
    return spans


def kernel(**inputs):
    npv = {k: np.asarray(v) for k, v in inputs.items()}
    f32 = np.float32
    td = float(f32(npv["temporal_decay"]))
    thr = float(f32(npv["adaptive_threshold"]))
    reg = float(f32(npv["attention_reg"]))
    hs_mean = float(f32(np.asarray(npv["head_scale"], np.float64).mean()))
    regp = float(f32(reg) + f32(1e-8))

    decay_full = (np.float64(td) ** (S - 1 - np.arange(S))).astype(np.float32)
    passing = decay_full > f32(thr)
    if not passing.any():
        return _uniform_fallback(npv, _numpy_spans(npv))
    s_min = int(np.nonzero(passing)[0].min())
    S_active = min(S, int(np.ceil((S - s_min + 8) / 32.0)) * 32)
    if S_active > 128:
        return _numpy_full_reference(npv)
    s_lo = S - S_active

    nc = _build(S_active, hs_mean, regp)
    consts = _host_consts(S_active, td, thr,
                          float(np.float32(npv['rs_b3'].reshape(-1)[0])))

    import ml_dtypes
    bf16 = ml_dtypes.bfloat16

    def c(a, dt=np.float32):
        return np.ascontiguousarray(np.asarray(a, dtype=np.float32).astype(dt))

    rw1 = np.asarray(npv["rs_w1"], np.float32)
    shared = dict(
        w1x=c(rw1[:E]), w1c=c(rw1[E:], bf16),
        rs_w2=c(npv["rs_w2"]), rs_w3=c(npv["rs_w3"]),
        rs_b1=c(npv["rs_b1"]), rs_b2=c(npv["rs_b2"]), rs_b3=c(npv["rs_b3"]),
        sw1=c(npv["sp_w1"], bf16), sw2=c(npv["sp_w2"], bf16),
        sw3=c(npv["sp_w3"], bf16),
        sp_b1=c(npv["sp_b1"]), sp_b2=c(npv["sp_b2"]), sp_b3=c(npv["sp_b3"]),
        **{k: (c(v, bf16) if k == "patt8" else c(v)) for k, v in consts.items()})
    in_maps = []
    for i in range(NCORES):
        bs = slice(i * BL, (i + 1) * BL)
        in_maps.append(dict(
            lx=c(npv["long_x"][s_lo:, bs, :].transpose(1, 0, 2)),
            encb=c(npv["encoded_x"][:, bs, :], bf16),
            **shared))

    res = run_bass_kernel_spmd(nc, in_maps, core_ids=list(range(NCORES)))
    global LAST_RESULT
    LAST_RESULT = res
    rs = res.results
    attn = np.concatenate([r["attn_out"] for r in rs], axis=1)
    feats = np.concatenate([r["feats_out"] for r in rs], axis=0)
    spans = np.concatenate([r["spans_out"] for r in rs], axis=0).astype(np.int32)
    return attn, feats, spans


if __name__ == "__main__":
    rng = np.random.default_rng(0)
    print("smoke test requires reference inputs; use test.py")
